# revision 8
# baseline (speedup 1.0000x reference)
"""MixHop (2-hop) GNN forward on 8 TRN2 NeuronCores.

Sharding: adj and the output are row-sharded over N=8192 across 8 cores
(1024 rows each); x and all weights are replicated. Each propagation
adj_loc @ v is a local [1024,8192]@[8192,F] matmul; v is produced
row-sharded and AllGathered between hops. All matmuls run in fp32r
(full-rate reduced-precision fp32); PSUM accumulation is fp32.

Orientation notes:
- "natural"   = rows on partitions (needed for AllGather row-concat and
  as the K axis of the next propagation)
- "transposed" = features on partitions (needed as lhsT of the next
  dense layer; makes BatchNorm affine per-partition)
Pass B and D emit transposed outputs directly; pass A's t1 half and
pass C's s1 half are transposed on the PE with an identity matmul.
b1[0] (hop-0 bias of layer 1) is dropped: a per-column constant shift
is exactly cancelled by the training-mode BatchNorm that follows.
"""
import sys
from contextlib import ExitStack

sys.path.insert(0, "/opt/trn_rl_repo")

import numpy as np

N, IN, H, OUT = 8192, 128, 512, 256
NC = 8
ROWS = N // NC          # 1024 rows per core
KT = N // 128           # 64 k-tiles of the propagation contraction
HT = 3 * H // 128       # 12 feature tiles of h.T
H2T = 3 * OUT // 128    # 6 feature tiles of h2.T
EPS = 1e-5

_BUILT = {}


def build_program():
    """Build and compile the Bass program (cached)."""
    if "nc" in _BUILT:
        return _BUILT["nc"]

    import concourse.bacc as bacc
    import concourse.tile as tile
    import concourse.mybir as mybir
    from concourse.alu_op_type import AluOpType

    f32 = mybir.dt.float32
    f32r = mybir.dt.float32r
    AF = mybir.ActivationFunctionType
    AX = mybir.AxisListType

    nc = bacc.Bacc("TRN2", target_bir_lowering=False, debug=False,
                   num_devices=NC)

    # ---- external inputs (per-core values supplied by the host) ----
    adjT_d = nc.dram_tensor("adjT", [N, ROWS], f32r, kind="ExternalInput")
    xT_d = nc.dram_tensor("xT", [IN, N], f32r, kind="ExternalInput")
    xTloc_d = nc.dram_tensor("xTloc", [IN, ROWS], f32r, kind="ExternalInput")
    w1cat_d = nc.dram_tensor("w1cat", [IN, 2 * H], f32r, kind="ExternalInput")
    w1h0_d = nc.dram_tensor("w1h0", [IN, H], f32r, kind="ExternalInput")
    b1bc_d = nc.dram_tensor("b1bc", [128, 2 * H], f32, kind="ExternalInput")
    w2cat_d = nc.dram_tensor("w2cat", [3 * H, 2 * OUT], f32r, kind="ExternalInput")
    w2h0_d = nc.dram_tensor("w2h0", [3 * H, OUT], f32r, kind="ExternalInput")
    b2bc_d = nc.dram_tensor("b2bc", [128, 2 * OUT], f32, kind="ExternalInput")
    b2h0T_d = nc.dram_tensor("b2h0T", [128, 2], f32, kind="ExternalInput")
    wf_d = nc.dram_tensor("wf", [3 * OUT, OUT], f32r, kind="ExternalInput")
    bfT_d = nc.dram_tensor("bfT", [128, 2], f32, kind="ExternalInput")
    gcol_d = nc.dram_tensor("gcol", [128, HT], f32, kind="ExternalInput")
    bcol_d = nc.dram_tensor("bcol", [128, HT], f32, kind="ExternalInput")
    ident_d = nc.dram_tensor("ident", [128, 128], f32, kind="ExternalInput")

    outT_d = nc.dram_tensor("outT", [OUT, ROWS], f32, kind="ExternalOutput")

    rg = [list(range(NC))]

    with tile.TileContext(nc) as tc, ExitStack() as st:
        dram = st.enter_context(tc.tile_pool(name="dram", bufs=1, space="DRAM"))
        P = st.enter_context(tc.tile_pool(name="persist", bufs=1))
        DR = st.enter_context(tc.tile_pool(name="drain", bufs=2))

        # ---- DRAM intermediates ----
        v_dram = dram.tile([N, 2 * H], f32r, name="v_dram")
        ag1_in = dram.tile([ROWS, H], f32r, name="ag1_in")
        ag1_out = dram.tile([N, H], f32r, name="ag1_out", addr_space="Shared")
        ag2_in = dram.tile([ROWS, 2 * OUT], f32r, name="ag2_in")
        ag2_out = dram.tile([N, 2 * OUT], f32r, name="ag2_out",
                            addr_space="Shared")
        ag3_in = dram.tile([ROWS, OUT], f32r, name="ag3_in")
        ag3_out = dram.tile([N, OUT], f32r, name="ag3_out",
                            addr_space="Shared")
        ar_in = dram.tile([128, 2 * HT], f32, name="ar_in")
        ar_out = dram.tile([128, 2 * HT], f32, name="ar_out",
                           addr_space="Shared")

        # ---- small persistents (to the end) ----
        xTloc_sb = P.tile([IN, ROWS], f32r, name="xTloc_sb")
        nc.sync.dma_start(xTloc_sb[:], xTloc_d[:, :])
        w1h0_sb = P.tile([IN, H], f32r, name="w1h0_sb")
        nc.sync.dma_start(w1h0_sb[:], w1h0_d[:, :])
        b2h0T_sb = P.tile([128, 2], f32, name="b2h0T_sb")
        nc.sync.dma_start(b2h0T_sb[:], b2h0T_d[:, :])
        bfT_sb = P.tile([128, 2], f32, name="bfT_sb")
        nc.sync.dma_start(bfT_sb[:], bfT_d[:, :])
        gcol_sb = P.tile([128, HT], f32, name="gcol_sb")
        nc.sync.dma_start(gcol_sb[:], gcol_d[:, :])
        bcol_sb = P.tile([128, HT], f32, name="bcol_sb")
        nc.sync.dma_start(bcol_sb[:], bcol_d[:, :])
        ident_sb = P.tile([128, 128], f32, name="ident_sb")
        nc.sync.dma_start(ident_sb[:], ident_d[:, :])
        wf_sb = [P.tile([128, OUT], f32r, name=f"wf{k}") for k in range(H2T)]
        for k in range(H2T):
            nc.sync.dma_start(wf_sb[k][:], wf_d[k * 128:(k + 1) * 128, :])
        sumc = P.tile([128, HT], f32, name="sumc")
        sqc = P.tile([128, HT], f32, name="sqc")
        stat_g = P.tile([128, 2 * HT], f32, name="stat_g")
        scale_c = P.tile([128, HT], f32, name="scale_c")
        shift_c = P.tile([128, HT], f32, name="shift_c")
        # h2.T (fp32r): tiles 0-1 y0.T, 2-3 s1.T, 4-5 s2b.T
        h2T = [P.tile([128, ROWS], f32r, name=f"h2T{t}") for t in range(H2T)]

        # ================= T1: v = x @ [W1[1]|W1[2]] + b =====================
        with (
            tc.tile_pool(name="w1pool", bufs=1) as W1P,
            tc.tile_pool(name="t1slab", bufs=4) as T1S,
            tc.tile_pool(name="t1ps", bufs=2, space="PSUM") as T1PS,
        ):
            w1cat_sb = W1P.tile([IN, 2 * H], f32r, name="w1cat_sb")
            nc.sync.dma_start(w1cat_sb[:], w1cat_d[:, :])
            b1bc_sb = W1P.tile([128, 2 * H], f32, name="b1bc_sb")
            nc.sync.dma_start(b1bc_sb[:], b1bc_d[:, :])
            for k in range(KT):
                xk = T1S.tile([128, 128], f32r, name="xk", tag="xk")
                nc.sync.dma_start(xk[:], xT_d[:, k * 128:(k + 1) * 128])
                vps = T1PS.tile([128, 2 * H], f32, name="vps", tag="vps")
                for n in range(2):
                    nc.tensor.matmul(vps[:, n * H:(n + 1) * H], xk[:],
                                     w1cat_sb[:, n * H:(n + 1) * H],
                                     start=True, stop=True)
                vsb = DR.tile([128, 2 * H], f32r, name="vsb", tag="vsb")
                nc.vector.tensor_tensor(vsb[:], vps[:], b1bc_sb[:],
                                        AluOpType.add)
                nc.sync.dma_start(v_dram[k * 128:(k + 1) * 128, :], vsb[:])

        # ================= A-t2a: t2a = adj_loc @ x2v ========================
        with (
            tc.tile_pool(name="aslabs", bufs=3) as AS,
            tc.tile_pool(name="aps", bufs=1, space="PSUM") as APS,
        ):
            acc = [APS.tile([128, H], f32, name=f"acc{m}", tag=f"acc{m}")
                   for m in range(8)]
            for k in range(KT):
                aslab = AS.tile([128, ROWS], f32r, name="aslab", tag="aslab")
                nc.sync.dma_start(aslab[:], adjT_d[k * 128:(k + 1) * 128, :])
                v2 = AS.tile([128, H], f32r, name="v2", tag="v2")
                nc.sync.dma_start(v2[:], v_dram[k * 128:(k + 1) * 128, H:])
                for m in range(8):
                    nc.tensor.matmul(acc[m][:],
                                     aslab[:, m * 128:(m + 1) * 128],
                                     v2[:], start=(k == 0), stop=(k == KT - 1))
            for m in range(8):
                t2a = DR.tile([128, H], f32r, name="t2a", tag="t2a")
                nc.vector.tensor_copy(t2a[:], acc[m][:])
                nc.sync.dma_start(ag1_in[m * 128:(m + 1) * 128, :], t2a[:])

        nc.gpsimd.collective_compute(
            "AllGather", AluOpType.bypass, replica_groups=rg,
            ins=[ag1_in[:].opt()], outs=[ag1_out[:].opt()])

        # t1 natural (fp32), transposed right after pass B
        PT1 = st.enter_context(tc.tile_pool(name="t1nat", bufs=1))
        t1_sb = [PT1.tile([128, H], f32, name=f"t1n{m}") for m in range(8)]

        # ================= A-t1: t1 = adj_loc @ x1v (natural) ================
        with (
            tc.tile_pool(name="a2slabs", bufs=3) as AS2,
            tc.tile_pool(name="aps2", bufs=1, space="PSUM") as APS2,
        ):
            acc2 = [APS2.tile([128, H], f32, name=f"ac2{m}", tag=f"ac2{m}")
                    for m in range(8)]
            for k in range(KT):
                aslab = AS2.tile([128, ROWS], f32r, name="aslab", tag="aslab")
                nc.sync.dma_start(aslab[:], adjT_d[k * 128:(k + 1) * 128, :])
                v1 = AS2.tile([128, H], f32r, name="v1", tag="v1")
                nc.sync.dma_start(v1[:], v_dram[k * 128:(k + 1) * 128, :H])
                for m in range(8):
                    nc.tensor.matmul(acc2[m][:],
                                     aslab[:, m * 128:(m + 1) * 128],
                                     v1[:], start=(k == 0), stop=(k == KT - 1))
            for m in range(8):
                nc.vector.tensor_copy(t1_sb[m][:], acc2[m][:])

        # h.T in fp32r: tiles 0-3 hop0.T, 4-7 t1.T, 8-11 t2b.T.
        # Written pre-norm, then BN+relu normalized IN PLACE.
        PH = st.enter_context(tc.tile_pool(name="hpool", bufs=1))
        hT = [PH.tile([128, ROWS], f32r, name=f"hT{t}") for t in range(HT)]
        # layer-2 weights, loaded during pass B
        PW2 = st.enter_context(tc.tile_pool(name="w2pool", bufs=1))
        w2cat_sb = [PW2.tile([128, 2 * OUT], f32r, name=f"w2cat{k}")
                    for k in range(HT)]
        for k in range(HT):
            nc.sync.dma_start(w2cat_sb[k][:], w2cat_d[k * 128:(k + 1) * 128, :])
        w2h0_sb = [PW2.tile([128, OUT], f32r, name=f"w2h0{k}")
                   for k in range(HT)]
        for k in range(HT):
            nc.sync.dma_start(w2h0_sb[k][:], w2h0_d[k * 128:(k + 1) * 128, :])
        b2bc_sb = PW2.tile([128, 2 * OUT], f32, name="b2bc_sb")
        nc.sync.dma_start(b2bc_sb[:], b2bc_d[:, :])

        # ================= B: t2b.T = (adj_loc @ t2a_full).T =================
        with (
            tc.tile_pool(name="bslabs", bufs=3) as BS,
            tc.tile_pool(name="bps", bufs=1, space="PSUM") as BPS,
        ):
            psb = [BPS.tile([128, H], f32, name=f"psb{i}", tag=f"psb{i}")
                   for i in range(8)]  # i = mo*2+n
            for k in range(KT):
                aslab = BS.tile([128, ROWS], f32r, name="aslab", tag="aslab")
                nc.sync.dma_start(aslab[:], adjT_d[k * 128:(k + 1) * 128, :])
                tslab = BS.tile([128, H], f32r, name="tslab", tag="tslab")
                nc.sync.dma_start(tslab[:], ag1_out[k * 128:(k + 1) * 128, :])
                for mo in range(4):
                    for n in range(2):
                        nc.tensor.matmul(
                            psb[mo * 2 + n][:],
                            tslab[:, mo * 128:(mo + 1) * 128],
                            aslab[:, n * H:(n + 1) * H],
                            start=(k == 0), stop=(k == KT - 1))
            for mo in range(4):
                for n in range(2):
                    nc.vector.tensor_copy(hT[8 + mo][:, n * H:(n + 1) * H],
                                          psb[mo * 2 + n][:])

        # ========== hop0.T + t1 transposes + BN stats ========================
        with (
            tc.tile_pool(name="tps", bufs=4, space="PSUM") as TPS,
            tc.tile_pool(name="h0ps", bufs=2, space="PSUM") as H0PS,
            tc.tile_pool(name="sqps", bufs=1, space="PSUM") as SQPS,
        ):
            # hop0.T = (x_loc @ W1[0]).T  (bias dropped: BN-invariant)
            for mo in range(4):
                for n in range(2):
                    h0ps = H0PS.tile([128, H], f32, name="h0ps", tag="h0ps")
                    nc.tensor.matmul(h0ps[:],
                                     w1h0_sb[:, mo * 128:(mo + 1) * 128],
                                     xTloc_sb[:, n * H:(n + 1) * H],
                                     start=True, stop=True)
                    nc.vector.tensor_copy(hT[mo][:, n * H:(n + 1) * H],
                                          h0ps[:])
            # t1.T via PE transpose
            for c in range(4):
                for m in range(8):
                    tp = TPS.tile([128, 128], f32, name="tp", tag="tp")
                    nc.tensor.transpose(tp[:],
                                        t1_sb[m][:, c * 128:(c + 1) * 128],
                                        ident_sb[:])
                    nc.vector.tensor_copy(hT[4 + c][:, m * 128:(m + 1) * 128],
                                          tp[:])

            # local BN stats: sum and sum-of-squares along rows (free axis)
            for t in range(HT):
                nc.vector.reduce_sum(sumc[:, t:t + 1], hT[t][:], axis=AX.X)
                sq_scr = SQPS.tile([128, ROWS], f32, name="sq_scr",
                                   tag="sq_scr")
                nc.vector.scalar_tensor_tensor(
                    sq_scr[:], hT[t][:], 1.0, hT[t][:],
                    AluOpType.mult, AluOpType.mult,
                    accum_out=sqc[:, t:t + 1])
            nc.sync.dma_start(ar_in[:, :HT], sumc[:])
            nc.sync.dma_start(ar_in[:, HT:], sqc[:])

        nc.gpsimd.collective_compute(
            "AllReduce", AluOpType.add, replica_groups=rg,
            ins=[ar_in[:].opt()], outs=[ar_out[:].opt()])

        # ========== BN affine params + normalize + relu (in place) ==========
        nc.sync.dma_start(stat_g[:], ar_out[:, :])
        mu = DR.tile([128, HT], f32, name="mu", tag="mu")
        nc.vector.tensor_scalar_mul(mu[:], stat_g[:, :HT], 1.0 / N)
        # ex2 = sumsq/N + eps (eps folded in here; var+eps overall)
        ex2 = DR.tile([128, HT], f32, name="ex2", tag="ex2")
        nc.vector.tensor_scalar(ex2[:], stat_g[:, HT:], 1.0 / N, EPS,
                                AluOpType.mult, AluOpType.add)
        var = DR.tile([128, HT], f32, name="var", tag="var")
        # var = (mu * -1) * mu + ex2
        nc.vector.scalar_tensor_tensor(var[:], mu[:], -1.0, mu[:],
                                       AluOpType.mult, AluOpType.mult)
        nc.vector.tensor_add(var[:], var[:], ex2[:])
        std = DR.tile([128, HT], f32, name="std", tag="std")
        nc.scalar.activation(std[:], var[:], AF.Sqrt)
        rstd = DR.tile([128, HT], f32, name="rstd", tag="rstd")
        nc.vector.reciprocal(rstd[:], std[:])
        nc.vector.tensor_mul(scale_c[:], gcol_sb[:], rstd[:])
        # shift = bcol - mu*scale
        nc.vector.scalar_tensor_tensor(shift_c[:], mu[:], -1.0, scale_c[:],
                                       AluOpType.mult, AluOpType.mult)
        nc.vector.tensor_add(shift_c[:], shift_c[:], bcol_sb[:])
        for t in range(HT):
            nc.scalar.activation(hT[t][:], hT[t][:], AF.Relu,
                                 bias=shift_c[:, t:t + 1],
                                 scale=scale_c[:, t:t + 1])

        # ========== T2: y = [y1|y2] = hn @ [W2[1]|W2[2]] + b =================
        with (
            tc.tile_pool(name="yps", bufs=4, space="PSUM") as YPS,
            tc.tile_pool(name="y0ps", bufs=2, space="PSUM") as Y0PS,
        ):
            for m in range(8):
                yps = YPS.tile([128, 2 * OUT], f32, name="yps", tag="yps")
                for k in range(HT):
                    nc.tensor.matmul(yps[:], hT[k][:, m * 128:(m + 1) * 128],
                                     w2cat_sb[k][:],
                                     start=(k == 0), stop=(k == HT - 1))
                ysb = DR.tile([128, 2 * OUT], f32r, name="ysb", tag="ysb")
                nc.vector.tensor_tensor(ysb[:], yps[:], b2bc_sb[:],
                                        AluOpType.add)
                nc.sync.dma_start(ag2_in[m * 128:(m + 1) * 128, :], ysb[:])

            # y0.T = (hn @ W2[0]).T + b2[0] (per-partition bias)
            for mo in range(2):
                for n in range(2):
                    y0ps = Y0PS.tile([128, H], f32, name="y0ps", tag="y0ps")
                    for k in range(HT):
                        nc.tensor.matmul(
                            y0ps[:], w2h0_sb[k][:, mo * 128:(mo + 1) * 128],
                            hT[k][:, n * H:(n + 1) * H],
                            start=(k == 0), stop=(k == HT - 1))
                    nc.vector.tensor_scalar_add(h2T[mo][:, n * H:(n + 1) * H],
                                                y0ps[:],
                                                b2h0T_sb[:, mo:mo + 1])

        nc.gpsimd.collective_compute(
            "AllGather", AluOpType.bypass, replica_groups=rg,
            ins=[ag2_in[:].opt()], outs=[ag2_out[:].opt()])

        # s1 natural (fp32), transposed during pass D
        PS1 = st.enter_context(tc.tile_pool(name="s1nat", bufs=1))
        s1_sb = [PS1.tile([128, OUT], f32, name=f"s1n{m}") for m in range(8)]

        # ========== C: [s1|s2a] = adj_loc @ [y1|y2] (natural) ================
        with (
            tc.tile_pool(name="cslabs", bufs=3) as CS,
            tc.tile_pool(name="cps", bufs=1, space="PSUM") as CPS,
        ):
            psc = [CPS.tile([128, 2 * OUT], f32, name=f"psc{m}", tag=f"psc{m}")
                   for m in range(8)]
            for k in range(KT):
                aslab = CS.tile([128, ROWS], f32r, name="aslab", tag="aslab")
                nc.sync.dma_start(aslab[:], adjT_d[k * 128:(k + 1) * 128, :])
                yslab = CS.tile([128, 2 * OUT], f32r, name="yslab", tag="yslab")
                nc.sync.dma_start(yslab[:], ag2_out[k * 128:(k + 1) * 128, :])
                for m in range(8):
                    nc.tensor.matmul(psc[m][:],
                                     aslab[:, m * 128:(m + 1) * 128],
                                     yslab[:], start=(k == 0),
                                     stop=(k == KT - 1))
            for m in range(8):
                nc.vector.tensor_copy(s1_sb[m][:], psc[m][:, :OUT])
                s2a = DR.tile([128, OUT], f32r, name="s2a", tag="s2a")
                nc.vector.tensor_copy(s2a[:], psc[m][:, OUT:])
                nc.sync.dma_start(ag3_in[m * 128:(m + 1) * 128, :], s2a[:])

        nc.gpsimd.collective_compute(
            "AllGather", AluOpType.bypass, replica_groups=rg,
            ins=[ag3_in[:].opt()], outs=[ag3_out[:].opt()])

        # ========== D: s2b.T = (adj_loc @ s2a_full).T + s1 transposes ========
        with (
            tc.tile_pool(name="dslabs", bufs=3) as DS,
            tc.tile_pool(name="dps", bufs=1, space="PSUM") as DPS,
            tc.tile_pool(name="tps2", bufs=4, space="PSUM") as TPS2,
        ):
            psd = [DPS.tile([128, H], f32, name=f"psd{i}", tag=f"psd{i}")
                   for i in range(4)]  # i = mo*2+n
            for k in range(KT):
                aslab = DS.tile([128, ROWS], f32r, name="aslab", tag="aslab")
                nc.sync.dma_start(aslab[:], adjT_d[k * 128:(k + 1) * 128, :])
                sslab = DS.tile([128, OUT], f32r, name="sslab", tag="sslab")
                nc.sync.dma_start(sslab[:], ag3_out[k * 128:(k + 1) * 128, :])
                for mo in range(2):
                    for n in range(2):
                        nc.tensor.matmul(
                            psd[mo * 2 + n][:],
                            sslab[:, mo * 128:(mo + 1) * 128],
                            aslab[:, n * H:(n + 1) * H],
                            start=(k == 0), stop=(k == KT - 1))
            for mo in range(2):
                for n in range(2):
                    nc.vector.tensor_copy(h2T[4 + mo][:, n * H:(n + 1) * H],
                                          psd[mo * 2 + n][:])
            # s1.T via PE transpose
            for c in range(2):
                for m in range(8):
                    tp2 = TPS2.tile([128, 128], f32, name="tp2", tag="tp2")
                    nc.tensor.transpose(tp2[:],
                                        s1_sb[m][:, c * 128:(c + 1) * 128],
                                        ident_sb[:])
                    nc.vector.tensor_copy(h2T[2 + c][:, m * 128:(m + 1) * 128],
                                          tp2[:])

        # ========== final: out.T = (h2 @ Wf).T + bf ==========================
        with tc.tile_pool(name="fps", bufs=2, space="PSUM") as FPS:
            for mo in range(2):
                for n in range(2):
                    fps = FPS.tile([128, H], f32, name="fps", tag="fps")
                    for k in range(H2T):
                        nc.tensor.matmul(
                            fps[:], wf_sb[k][:, mo * 128:(mo + 1) * 128],
                            h2T[k][:, n * H:(n + 1) * H],
                            start=(k == 0), stop=(k == H2T - 1))
                    osb = DR.tile([128, H], f32, name="osb", tag="osb")
                    nc.vector.tensor_scalar_add(osb[:], fps[:],
                                                bfT_sb[:, mo:mo + 1])
                    nc.sync.dma_start(
                        outT_d[mo * 128:(mo + 1) * 128, n * H:(n + 1) * H],
                        osb[:])

    nc.compile()
    _BUILT["nc"] = nc
    return nc


def prep_in_maps(x, adj, W1, b1, W2, b2, gamma, beta, Wf, bf):
    """Host-side sharding / layout prep. Returns one input dict per core."""
    x = np.asarray(x, dtype=np.float32)
    adj = np.asarray(adj, dtype=np.float32)
    W1 = np.asarray(W1, dtype=np.float32)
    b1 = np.asarray(b1, dtype=np.float32)
    W2 = np.asarray(W2, dtype=np.float32)
    b2 = np.asarray(b2, dtype=np.float32)
    gamma = np.asarray(gamma, dtype=np.float32)
    beta = np.asarray(beta, dtype=np.float32)
    Wf = np.asarray(Wf, dtype=np.float32)
    bf = np.asarray(bf, dtype=np.float32)

    xT = np.ascontiguousarray(x.T)                       # [128, 8192]
    w1cat = np.ascontiguousarray(
        np.concatenate([W1[1], W1[2]], axis=1))          # [128, 1024]
    b1cat = np.concatenate([b1[1], b1[2]])               # [1024]
    b1bc = np.ascontiguousarray(
        np.broadcast_to(b1cat[None, :], (128, 2 * H)))
    w2cat = np.ascontiguousarray(
        np.concatenate([W2[1], W2[2]], axis=1))          # [1536, 512]
    b2cat = np.concatenate([b2[1], b2[2]])               # [512]
    b2bc = np.ascontiguousarray(
        np.broadcast_to(b2cat[None, :], (128, 2 * OUT)))
    gcol = np.ascontiguousarray(gamma.reshape(HT, 128).T)
    bcol = np.ascontiguousarray(beta.reshape(HT, 128).T)
    ident = np.eye(128, dtype=np.float32)

    shared = {
        "xT": xT,
        "w1cat": w1cat,
        "w1h0": np.ascontiguousarray(W1[0]),
        "b1bc": b1bc,
        "w2cat": w2cat,
        "w2h0": np.ascontiguousarray(W2[0]),
        "b2bc": b2bc,
        "b2h0T": np.ascontiguousarray(b2[0].reshape(2, 128).T),
        "wf": np.ascontiguousarray(Wf),
        "bfT": np.ascontiguousarray(bf.reshape(2, 128).T),
        "gcol": gcol,
        "bcol": bcol,
        "ident": ident,
    }
    in_maps = []
    for d in range(NC):
        r0, r1 = d * ROWS, (d + 1) * ROWS
        m = dict(shared)
        m["adjT"] = np.ascontiguousarray(adj[r0:r1].T)   # [8192, 1024]
        m["xTloc"] = np.ascontiguousarray(x[r0:r1].T)    # [128, 1024]
        in_maps.append(m)
    return in_maps


def run_on_hw(in_maps, trace=False):
    from concourse import bass_utils
    nc = build_program()
    return bass_utils.run_bass_kernel_spmd(
        nc, in_maps, core_ids=list(range(NC)), trace=trace)


def kernel(x, adj, W1, b1, W2, b2, gamma, beta, Wf, bf):
    in_maps = prep_in_maps(x, adj, W1, b1, W2, b2, gamma, beta, Wf, bf)
    res = run_on_hw(in_maps)
    out = np.concatenate(
        [np.ascontiguousarray(res.results[d]["outT"].T) for d in range(NC)],
        axis=0)
    return out.astype(np.float32)


# revision 9
# speedup vs baseline: 1.3252x; 1.3252x over previous
"""MixHop (2-hop) GNN forward on 8 TRN2 NeuronCores.

Sharding: adj and the output are row-sharded over N=8192 across 8 cores
(1024 rows each); x and all weights are replicated. Each propagation
adj_loc @ v is a local [1024,8192]@[8192,F] matmul; v is produced
row-sharded and AllGathered between hops.

Precision: propagation matmuls (adj-sided, the bulk of bytes+flops) run
in bf16 with fp32 PSUM accumulation; dense-layer transforms and BN run
in fp32r (full-rate reduced fp32). Measured end-to-end relative error
~2e-3.

Orientation notes:
- "natural"   = rows on partitions (needed for AllGather row-concat and
  as the K axis of the next propagation)
- "transposed" = features on partitions (needed as lhsT of the next
  dense layer; makes BatchNorm affine per-partition)
Pass B and D emit transposed outputs directly; pass A's t1 half and
pass C's s1 half are transposed on the PE with an identity matmul.
b1[0] (hop-0 bias of layer 1) is dropped: a per-column constant shift
is exactly cancelled by the training-mode BatchNorm that follows.

DMA: adj slabs alternate between the two HWDGE rings (sync/scalar);
rhs slabs ride the opposite ring; SBUF->DRAM drains go via SWDGE
(gpsimd) to keep the HWDGE rings free for loads.
"""
import sys
from contextlib import ExitStack

sys.path.insert(0, "/opt/trn_rl_repo")

import numpy as np

N, IN, H, OUT = 8192, 128, 512, 256
NC = 8
ROWS = N // NC          # 1024 rows per core
KT = N // 128           # 64 k-tiles of the propagation contraction
HT = 3 * H // 128       # 12 feature tiles of h.T
H2T = 3 * OUT // 128    # 6 feature tiles of h2.T
EPS = 1e-5

_BUILT = {}


def build_program():
    """Build and compile the Bass program (cached)."""
    if "nc" in _BUILT:
        return _BUILT["nc"]

    import concourse.bacc as bacc
    import concourse.tile as tile
    import concourse.mybir as mybir
    from concourse.alu_op_type import AluOpType

    f32 = mybir.dt.float32
    f32r = mybir.dt.float32r
    bf16 = mybir.dt.bfloat16
    AF = mybir.ActivationFunctionType
    AX = mybir.AxisListType

    nc = bacc.Bacc("TRN2", target_bir_lowering=False, debug=False,
                   num_devices=NC)

    # ---- external inputs (per-core values supplied by the host) ----
    adjT_d = nc.dram_tensor("adjT", [N, ROWS], bf16, kind="ExternalInput")
    xT_d = nc.dram_tensor("xT", [IN, N], f32r, kind="ExternalInput")
    xTloc_d = nc.dram_tensor("xTloc", [IN, ROWS], f32r, kind="ExternalInput")
    w1cat_d = nc.dram_tensor("w1cat", [IN, 2 * H], f32r, kind="ExternalInput")
    w1h0_d = nc.dram_tensor("w1h0", [IN, H], f32r, kind="ExternalInput")
    b1bc_d = nc.dram_tensor("b1bc", [128, 2 * H], f32, kind="ExternalInput")
    w2cat_d = nc.dram_tensor("w2cat", [3 * H, 2 * OUT], f32r, kind="ExternalInput")
    w2h0_d = nc.dram_tensor("w2h0", [3 * H, OUT], f32r, kind="ExternalInput")
    b2bc_d = nc.dram_tensor("b2bc", [128, 2 * OUT], f32, kind="ExternalInput")
    b2h0T_d = nc.dram_tensor("b2h0T", [128, 2], f32, kind="ExternalInput")
    wf_d = nc.dram_tensor("wf", [3 * OUT, OUT], f32r, kind="ExternalInput")
    bfT_d = nc.dram_tensor("bfT", [128, 2], f32, kind="ExternalInput")
    gcol_d = nc.dram_tensor("gcol", [128, HT], f32, kind="ExternalInput")
    bcol_d = nc.dram_tensor("bcol", [128, HT], f32, kind="ExternalInput")
    ident_d = nc.dram_tensor("ident", [128, 128], f32, kind="ExternalInput")

    outT_d = nc.dram_tensor("outT", [OUT, ROWS], f32, kind="ExternalOutput")

    rg = [list(range(NC))]

    with tile.TileContext(nc) as tc, ExitStack() as st:
        dram = st.enter_context(tc.tile_pool(name="dram", bufs=1, space="DRAM"))
        P = st.enter_context(tc.tile_pool(name="persist", bufs=1))
        DR = st.enter_context(tc.tile_pool(name="drain", bufs=2))

        # ---- DRAM intermediates ----
        v_dram = dram.tile([N, 2 * H], bf16, name="v_dram")
        ag1_in = dram.tile([ROWS, H], bf16, name="ag1_in")
        ag1_out = dram.tile([N, H], bf16, name="ag1_out", addr_space="Shared")
        ag2_in = dram.tile([ROWS, 2 * OUT], bf16, name="ag2_in")
        ag2_out = dram.tile([N, 2 * OUT], bf16, name="ag2_out",
                            addr_space="Shared")
        ag3_in = dram.tile([ROWS, OUT], bf16, name="ag3_in")
        ag3_out = dram.tile([N, OUT], bf16, name="ag3_out",
                            addr_space="Shared")
        ar_in = dram.tile([128, 2 * HT], f32, name="ar_in")
        ar_out = dram.tile([128, 2 * HT], f32, name="ar_out",
                           addr_space="Shared")

        # ---- small persistents (to the end) ----
        xTloc_sb = P.tile([IN, ROWS], f32r, name="xTloc_sb")
        nc.scalar.dma_start(xTloc_sb[:], xTloc_d[:, :])
        w1h0_sb = P.tile([IN, H], f32r, name="w1h0_sb")
        nc.scalar.dma_start(w1h0_sb[:], w1h0_d[:, :])
        b2h0T_sb = P.tile([128, 2], f32, name="b2h0T_sb")
        nc.scalar.dma_start(b2h0T_sb[:], b2h0T_d[:, :])
        bfT_sb = P.tile([128, 2], f32, name="bfT_sb")
        nc.scalar.dma_start(bfT_sb[:], bfT_d[:, :])
        gcol_sb = P.tile([128, HT], f32, name="gcol_sb")
        nc.scalar.dma_start(gcol_sb[:], gcol_d[:, :])
        bcol_sb = P.tile([128, HT], f32, name="bcol_sb")
        nc.scalar.dma_start(bcol_sb[:], bcol_d[:, :])
        ident_sb = P.tile([128, 128], f32, name="ident_sb")
        nc.scalar.dma_start(ident_sb[:], ident_d[:, :])
        wf_sb = [P.tile([128, OUT], f32r, name=f"wf{k}") for k in range(H2T)]
        for k in range(H2T):
            nc.scalar.dma_start(wf_sb[k][:], wf_d[k * 128:(k + 1) * 128, :])
        sumc = P.tile([128, HT], f32, name="sumc")
        sqc = P.tile([128, HT], f32, name="sqc")
        stat_g = P.tile([128, 2 * HT], f32, name="stat_g")
        scale_c = P.tile([128, HT], f32, name="scale_c")
        shift_c = P.tile([128, HT], f32, name="shift_c")
        # h2.T (fp32r): tiles 0-1 y0.T, 2-3 s1.T, 4-5 s2b.T
        h2T = [P.tile([128, ROWS], f32r, name=f"h2T{t}") for t in range(H2T)]

        # ================= T1: v = x @ [W1[1]|W1[2]] + b =====================
        with (
            tc.tile_pool(name="w1pool", bufs=1) as W1P,
            tc.tile_pool(name="t1slab", bufs=4) as T1S,
            tc.tile_pool(name="t1ps", bufs=2, space="PSUM") as T1PS,
        ):
            w1cat_sb = W1P.tile([IN, 2 * H], f32r, name="w1cat_sb")
            nc.scalar.dma_start(w1cat_sb[:], w1cat_d[:, :])
            b1bc_sb = W1P.tile([128, 2 * H], f32, name="b1bc_sb")
            nc.scalar.dma_start(b1bc_sb[:], b1bc_d[:, :])
            for k in range(KT):
                xk = T1S.tile([128, 128], f32r, name="xk", tag="xk")
                nc.sync.dma_start(xk[:], xT_d[:, k * 128:(k + 1) * 128])
                vps = T1PS.tile([128, 2 * H], f32, name="vps", tag="vps")
                for n in range(2):
                    nc.tensor.matmul(vps[:, n * H:(n + 1) * H], xk[:],
                                     w1cat_sb[:, n * H:(n + 1) * H],
                                     start=True, stop=True)
                vsb = DR.tile([128, 2 * H], bf16, name="vsb", tag="vsb")
                nc.vector.tensor_tensor(vsb[:], vps[:], b1bc_sb[:],
                                        AluOpType.add)
                nc.gpsimd.dma_start(v_dram[k * 128:(k + 1) * 128, :], vsb[:])

        # ================= A-t2a: t2a = adj_loc @ x2v ========================
        with (
            tc.tile_pool(name="aslabs", bufs=3) as AS,
            tc.tile_pool(name="aps", bufs=1, space="PSUM") as APS,
        ):
            acc = [APS.tile([128, H], f32, name=f"acc{m}", tag=f"acc{m}")
                   for m in range(8)]
            for k in range(KT):
                aslab = AS.tile([128, ROWS], bf16, name="aslab", tag="aslab")
                eng = nc.sync if k % 2 == 0 else nc.scalar
                oth = nc.scalar if k % 2 == 0 else nc.sync
                eng.dma_start(aslab[:], adjT_d[k * 128:(k + 1) * 128, :])
                v2 = AS.tile([128, H], bf16, name="v2", tag="v2")
                oth.dma_start(v2[:], v_dram[k * 128:(k + 1) * 128, H:])
                for m in range(8):
                    nc.tensor.matmul(acc[m][:],
                                     aslab[:, m * 128:(m + 1) * 128],
                                     v2[:], start=(k == 0), stop=(k == KT - 1))
            for m in range(8):
                t2a = DR.tile([128, H], bf16, name="t2a", tag="t2a")
                nc.vector.tensor_copy(t2a[:], acc[m][:])
                nc.gpsimd.dma_start(ag1_in[m * 128:(m + 1) * 128, :], t2a[:])

        nc.gpsimd.collective_compute(
            "AllGather", AluOpType.bypass, replica_groups=rg,
            ins=[ag1_in[:].opt()], outs=[ag1_out[:].opt()])

        # t1 natural (fp32), transposed right after A-t1
        PT1 = st.enter_context(tc.tile_pool(name="t1nat", bufs=1))
        t1_sb = [PT1.tile([128, H], f32, name=f"t1n{m}") for m in range(8)]

        # ================= A-t1: t1 = adj_loc @ x1v (natural) ================
        with (
            tc.tile_pool(name="a2slabs", bufs=3) as AS2,
            tc.tile_pool(name="aps2", bufs=1, space="PSUM") as APS2,
        ):
            acc2 = [APS2.tile([128, H], f32, name=f"ac2{m}", tag=f"ac2{m}")
                    for m in range(8)]
            for k in range(KT):
                aslab = AS2.tile([128, ROWS], bf16, name="aslab", tag="aslab")
                eng = nc.sync if k % 2 == 0 else nc.scalar
                oth = nc.scalar if k % 2 == 0 else nc.sync
                eng.dma_start(aslab[:], adjT_d[k * 128:(k + 1) * 128, :])
                v1 = AS2.tile([128, H], bf16, name="v1", tag="v1")
                oth.dma_start(v1[:], v_dram[k * 128:(k + 1) * 128, :H])
                for m in range(8):
                    nc.tensor.matmul(acc2[m][:],
                                     aslab[:, m * 128:(m + 1) * 128],
                                     v1[:], start=(k == 0), stop=(k == KT - 1))
            for m in range(8):
                nc.vector.tensor_copy(t1_sb[m][:], acc2[m][:])

        # h.T in fp32r: tiles 0-3 hop0.T, 4-7 t1.T, 8-11 t2b.T.
        # Written pre-norm, then BN+relu normalized IN PLACE.
        PH = st.enter_context(tc.tile_pool(name="hpool", bufs=1))
        hT = [PH.tile([128, ROWS], f32r, name=f"hT{t}") for t in range(HT)]
        # layer-2 weights, loaded during pass B
        PW2 = st.enter_context(tc.tile_pool(name="w2pool", bufs=1))
        w2cat_sb = [PW2.tile([128, 2 * OUT], f32r, name=f"w2cat{k}")
                    for k in range(HT)]
        for k in range(HT):
            nc.scalar.dma_start(w2cat_sb[k][:],
                                w2cat_d[k * 128:(k + 1) * 128, :])
        w2h0_sb = [PW2.tile([128, OUT], f32r, name=f"w2h0{k}")
                   for k in range(HT)]
        for k in range(HT):
            nc.scalar.dma_start(w2h0_sb[k][:],
                                w2h0_d[k * 128:(k + 1) * 128, :])
        b2bc_sb = PW2.tile([128, 2 * OUT], f32, name="b2bc_sb")
        nc.scalar.dma_start(b2bc_sb[:], b2bc_d[:, :])

        # ========== hop0.T + t1 transposes + stats for tiles 0..7 ============
        # (placed before pass B so only t2b stats remain on the critical path)
        with (
            tc.tile_pool(name="tps", bufs=4, space="PSUM") as TPS,
            tc.tile_pool(name="h0ps", bufs=2, space="PSUM") as H0PS,
            tc.tile_pool(name="sqps", bufs=1, space="PSUM") as SQPS,
        ):
            # hop0.T = (x_loc @ W1[0]).T  (bias dropped: BN-invariant)
            for mo in range(4):
                for n in range(2):
                    h0ps = H0PS.tile([128, H], f32, name="h0ps", tag="h0ps")
                    nc.tensor.matmul(h0ps[:],
                                     w1h0_sb[:, mo * 128:(mo + 1) * 128],
                                     xTloc_sb[:, n * H:(n + 1) * H],
                                     start=True, stop=True)
                    nc.vector.tensor_copy(hT[mo][:, n * H:(n + 1) * H],
                                          h0ps[:])
            # t1.T via PE transpose
            for c in range(4):
                for m in range(8):
                    tp = TPS.tile([128, 128], f32, name="tp", tag="tp")
                    nc.tensor.transpose(tp[:],
                                        t1_sb[m][:, c * 128:(c + 1) * 128],
                                        ident_sb[:])
                    nc.vector.tensor_copy(hT[4 + c][:, m * 128:(m + 1) * 128],
                                          tp[:])
            for t in range(8):
                nc.vector.reduce_sum(sumc[:, t:t + 1], hT[t][:], axis=AX.X)
                sq_scr = SQPS.tile([128, ROWS], f32, name="sq_scr",
                                   tag="sq_scr")
                nc.vector.scalar_tensor_tensor(
                    sq_scr[:], hT[t][:], 1.0, hT[t][:],
                    AluOpType.mult, AluOpType.mult,
                    accum_out=sqc[:, t:t + 1])

        # ================= B: t2b.T = (adj_loc @ t2a_full).T =================
        with (
            tc.tile_pool(name="bslabs", bufs=3) as BS,
            tc.tile_pool(name="bps", bufs=1, space="PSUM") as BPS,
        ):
            psb = [BPS.tile([128, H], f32, name=f"psb{i}", tag=f"psb{i}")
                   for i in range(8)]  # i = mo*2+n
            for k in range(KT):
                aslab = BS.tile([128, ROWS], bf16, name="aslab", tag="aslab")
                eng = nc.sync if k % 2 == 0 else nc.scalar
                oth = nc.scalar if k % 2 == 0 else nc.sync
                eng.dma_start(aslab[:], adjT_d[k * 128:(k + 1) * 128, :])
                tslab = BS.tile([128, H], bf16, name="tslab", tag="tslab")
                oth.dma_start(tslab[:], ag1_out[k * 128:(k + 1) * 128, :])
                for mo in range(4):
                    for n in range(2):
                        nc.tensor.matmul(
                            psb[mo * 2 + n][:],
                            tslab[:, mo * 128:(mo + 1) * 128],
                            aslab[:, n * H:(n + 1) * H],
                            start=(k == 0), stop=(k == KT - 1))
            for mo in range(4):
                for n in range(2):
                    nc.vector.tensor_copy(hT[8 + mo][:, n * H:(n + 1) * H],
                                          psb[mo * 2 + n][:])

        # ========== stats for t2b tiles + AllReduce ==========================
        with tc.tile_pool(name="sqps2", bufs=1, space="PSUM") as SQPS2:
            for t in range(8, HT):
                nc.vector.reduce_sum(sumc[:, t:t + 1], hT[t][:], axis=AX.X)
                sq_scr2 = SQPS2.tile([128, ROWS], f32, name="sq_scr2",
                                     tag="sq_scr2")
                nc.vector.scalar_tensor_tensor(
                    sq_scr2[:], hT[t][:], 1.0, hT[t][:],
                    AluOpType.mult, AluOpType.mult,
                    accum_out=sqc[:, t:t + 1])
        nc.gpsimd.dma_start(ar_in[:, :HT], sumc[:])
        nc.gpsimd.dma_start(ar_in[:, HT:], sqc[:])

        nc.gpsimd.collective_compute(
            "AllReduce", AluOpType.add, replica_groups=rg,
            ins=[ar_in[:].opt()], outs=[ar_out[:].opt()])

        # ========== BN affine params + normalize + relu (in place) ==========
        nc.sync.dma_start(stat_g[:], ar_out[:, :])
        mu = DR.tile([128, HT], f32, name="mu", tag="mu")
        nc.vector.tensor_scalar_mul(mu[:], stat_g[:, :HT], 1.0 / N)
        # ex2 = sumsq/N + eps (eps folded in here; var+eps overall)
        ex2 = DR.tile([128, HT], f32, name="ex2", tag="ex2")
        nc.vector.tensor_scalar(ex2[:], stat_g[:, HT:], 1.0 / N, EPS,
                                AluOpType.mult, AluOpType.add)
        var = DR.tile([128, HT], f32, name="var", tag="var")
        # var = (mu * -1) * mu + ex2
        nc.vector.scalar_tensor_tensor(var[:], mu[:], -1.0, mu[:],
                                       AluOpType.mult, AluOpType.mult)
        nc.vector.tensor_add(var[:], var[:], ex2[:])
        std = DR.tile([128, HT], f32, name="std", tag="std")
        nc.scalar.activation(std[:], var[:], AF.Sqrt)
        rstd = DR.tile([128, HT], f32, name="rstd", tag="rstd")
        nc.vector.reciprocal(rstd[:], std[:])
        nc.vector.tensor_mul(scale_c[:], gcol_sb[:], rstd[:])
        # shift = bcol - mu*scale
        nc.vector.scalar_tensor_tensor(shift_c[:], mu[:], -1.0, scale_c[:],
                                       AluOpType.mult, AluOpType.mult)
        nc.vector.tensor_add(shift_c[:], shift_c[:], bcol_sb[:])
        for t in range(HT):
            nc.scalar.activation(hT[t][:], hT[t][:], AF.Relu,
                                 bias=shift_c[:, t:t + 1],
                                 scale=scale_c[:, t:t + 1])

        # ========== T2: y = [y1|y2] = hn @ [W2[1]|W2[2]] + b =================
        with (
            tc.tile_pool(name="yps", bufs=4, space="PSUM") as YPS,
            tc.tile_pool(name="y0ps", bufs=2, space="PSUM") as Y0PS,
        ):
            for m in range(8):
                yps = YPS.tile([128, 2 * OUT], f32, name="yps", tag="yps")
                for k in range(HT):
                    nc.tensor.matmul(yps[:], hT[k][:, m * 128:(m + 1) * 128],
                                     w2cat_sb[k][:],
                                     start=(k == 0), stop=(k == HT - 1))
                ysb = DR.tile([128, 2 * OUT], bf16, name="ysb", tag="ysb")
                nc.vector.tensor_tensor(ysb[:], yps[:], b2bc_sb[:],
                                        AluOpType.add)
                nc.gpsimd.dma_start(ag2_in[m * 128:(m + 1) * 128, :], ysb[:])

            # y0.T = (hn @ W2[0]).T + b2[0] (per-partition bias)
            for mo in range(2):
                for n in range(2):
                    y0ps = Y0PS.tile([128, H], f32, name="y0ps", tag="y0ps")
                    for k in range(HT):
                        nc.tensor.matmul(
                            y0ps[:], w2h0_sb[k][:, mo * 128:(mo + 1) * 128],
                            hT[k][:, n * H:(n + 1) * H],
                            start=(k == 0), stop=(k == HT - 1))
                    nc.vector.tensor_scalar_add(h2T[mo][:, n * H:(n + 1) * H],
                                                y0ps[:],
                                                b2h0T_sb[:, mo:mo + 1])

        nc.gpsimd.collective_compute(
            "AllGather", AluOpType.bypass, replica_groups=rg,
            ins=[ag2_in[:].opt()], outs=[ag2_out[:].opt()])

        # s1 natural (fp32), transposed during pass D
        PS1 = st.enter_context(tc.tile_pool(name="s1nat", bufs=1))
        s1_sb = [PS1.tile([128, OUT], f32, name=f"s1n{m}") for m in range(8)]

        # ========== C: [s1|s2a] = adj_loc @ [y1|y2] (natural) ================
        with (
            tc.tile_pool(name="cslabs", bufs=3) as CS,
            tc.tile_pool(name="cps", bufs=1, space="PSUM") as CPS,
        ):
            psc = [CPS.tile([128, 2 * OUT], f32, name=f"psc{m}", tag=f"psc{m}")
                   for m in range(8)]
            for k in range(KT):
                aslab = CS.tile([128, ROWS], bf16, name="aslab", tag="aslab")
                eng = nc.sync if k % 2 == 0 else nc.scalar
                oth = nc.scalar if k % 2 == 0 else nc.sync
                eng.dma_start(aslab[:], adjT_d[k * 128:(k + 1) * 128, :])
                yslab = CS.tile([128, 2 * OUT], bf16, name="yslab", tag="yslab")
                oth.dma_start(yslab[:], ag2_out[k * 128:(k + 1) * 128, :])
                for m in range(8):
                    nc.tensor.matmul(psc[m][:],
                                     aslab[:, m * 128:(m + 1) * 128],
                                     yslab[:], start=(k == 0),
                                     stop=(k == KT - 1))
            for m in range(8):
                nc.vector.tensor_copy(s1_sb[m][:], psc[m][:, :OUT])
                s2a = DR.tile([128, OUT], bf16, name="s2a", tag="s2a")
                nc.vector.tensor_copy(s2a[:], psc[m][:, OUT:])
                nc.gpsimd.dma_start(ag3_in[m * 128:(m + 1) * 128, :], s2a[:])

        nc.gpsimd.collective_compute(
            "AllGather", AluOpType.bypass, replica_groups=rg,
            ins=[ag3_in[:].opt()], outs=[ag3_out[:].opt()])

        # ========== D: s2b.T = (adj_loc @ s2a_full).T + s1 transposes ========
        with (
            tc.tile_pool(name="dslabs", bufs=3) as DS,
            tc.tile_pool(name="dps", bufs=1, space="PSUM") as DPS,
            tc.tile_pool(name="tps2", bufs=4, space="PSUM") as TPS2,
        ):
            psd = [DPS.tile([128, H], f32, name=f"psd{i}", tag=f"psd{i}")
                   for i in range(4)]  # i = mo*2+n
            for k in range(KT):
                aslab = DS.tile([128, ROWS], bf16, name="aslab", tag="aslab")
                eng = nc.sync if k % 2 == 0 else nc.scalar
                oth = nc.scalar if k % 2 == 0 else nc.sync
                eng.dma_start(aslab[:], adjT_d[k * 128:(k + 1) * 128, :])
                sslab = DS.tile([128, OUT], bf16, name="sslab", tag="sslab")
                oth.dma_start(sslab[:], ag3_out[k * 128:(k + 1) * 128, :])
                for mo in range(2):
                    for n in range(2):
                        nc.tensor.matmul(
                            psd[mo * 2 + n][:],
                            sslab[:, mo * 128:(mo + 1) * 128],
                            aslab[:, n * H:(n + 1) * H],
                            start=(k == 0), stop=(k == KT - 1))
            for mo in range(2):
                for n in range(2):
                    nc.vector.tensor_copy(h2T[4 + mo][:, n * H:(n + 1) * H],
                                          psd[mo * 2 + n][:])
            # s1.T via PE transpose
            for c in range(2):
                for m in range(8):
                    tp2 = TPS2.tile([128, 128], f32, name="tp2", tag="tp2")
                    nc.tensor.transpose(tp2[:],
                                        s1_sb[m][:, c * 128:(c + 1) * 128],
                                        ident_sb[:])
                    nc.vector.tensor_copy(h2T[2 + c][:, m * 128:(m + 1) * 128],
                                          tp2[:])

        # ========== final: out.T = (h2 @ Wf).T + bf ==========================
        with tc.tile_pool(name="fps", bufs=2, space="PSUM") as FPS:
            for mo in range(2):
                for n in range(2):
                    fps = FPS.tile([128, H], f32, name="fps", tag="fps")
                    for k in range(H2T):
                        nc.tensor.matmul(
                            fps[:], wf_sb[k][:, mo * 128:(mo + 1) * 128],
                            h2T[k][:, n * H:(n + 1) * H],
                            start=(k == 0), stop=(k == H2T - 1))
                    osb = DR.tile([128, H], f32, name="osb", tag="osb")
                    nc.vector.tensor_scalar_add(osb[:], fps[:],
                                                bfT_sb[:, mo:mo + 1])
                    nc.sync.dma_start(
                        outT_d[mo * 128:(mo + 1) * 128, n * H:(n + 1) * H],
                        osb[:])

    nc.compile()
    _BUILT["nc"] = nc
    return nc


def prep_in_maps(x, adj, W1, b1, W2, b2, gamma, beta, Wf, bf):
    """Host-side sharding / layout prep. Returns one input dict per core."""
    import ml_dtypes

    x = np.asarray(x, dtype=np.float32)
    adj = np.asarray(adj, dtype=np.float32)
    W1 = np.asarray(W1, dtype=np.float32)
    b1 = np.asarray(b1, dtype=np.float32)
    W2 = np.asarray(W2, dtype=np.float32)
    b2 = np.asarray(b2, dtype=np.float32)
    gamma = np.asarray(gamma, dtype=np.float32)
    beta = np.asarray(beta, dtype=np.float32)
    Wf = np.asarray(Wf, dtype=np.float32)
    bf = np.asarray(bf, dtype=np.float32)

    xT = np.ascontiguousarray(x.T)                       # [128, 8192]
    w1cat = np.ascontiguousarray(
        np.concatenate([W1[1], W1[2]], axis=1))          # [128, 1024]
    b1cat = np.concatenate([b1[1], b1[2]])               # [1024]
    b1bc = np.ascontiguousarray(
        np.broadcast_to(b1cat[None, :], (128, 2 * H)))
    w2cat = np.ascontiguousarray(
        np.concatenate([W2[1], W2[2]], axis=1))          # [1536, 512]
    b2cat = np.concatenate([b2[1], b2[2]])               # [512]
    b2bc = np.ascontiguousarray(
        np.broadcast_to(b2cat[None, :], (128, 2 * OUT)))
    gcol = np.ascontiguousarray(gamma.reshape(HT, 128).T)
    bcol = np.ascontiguousarray(beta.reshape(HT, 128).T)
    ident = np.eye(128, dtype=np.float32)

    shared = {
        "xT": xT,
        "w1cat": w1cat,
        "w1h0": np.ascontiguousarray(W1[0]),
        "b1bc": b1bc,
        "w2cat": w2cat,
        "w2h0": np.ascontiguousarray(W2[0]),
        "b2bc": b2bc,
        "b2h0T": np.ascontiguousarray(b2[0].reshape(2, 128).T),
        "wf": np.ascontiguousarray(Wf),
        "bfT": np.ascontiguousarray(bf.reshape(2, 128).T),
        "gcol": gcol,
        "bcol": bcol,
        "ident": ident,
    }
    in_maps = []
    for d in range(NC):
        r0, r1 = d * ROWS, (d + 1) * ROWS
        m = dict(shared)
        m["adjT"] = np.ascontiguousarray(
            adj[r0:r1].T.astype(ml_dtypes.bfloat16))     # [8192, 1024] bf16
        m["xTloc"] = np.ascontiguousarray(x[r0:r1].T)    # [128, 1024]
        in_maps.append(m)
    return in_maps


def run_on_hw(in_maps, trace=False):
    from concourse import bass_utils
    nc = build_program()
    return bass_utils.run_bass_kernel_spmd(
        nc, in_maps, core_ids=list(range(NC)), trace=trace)


def kernel(x, adj, W1, b1, W2, b2, gamma, beta, Wf, bf):
    in_maps = prep_in_maps(x, adj, W1, b1, W2, b2, gamma, beta, Wf, bf)
    res = run_on_hw(in_maps)
    out = np.concatenate(
        [np.ascontiguousarray(res.results[d]["outT"].T) for d in range(NC)],
        axis=0)
    return out.astype(np.float32)


# revision 12
# speedup vs baseline: 1.3477x; 1.0170x over previous
"""MixHop (2-hop) GNN forward on 8 TRN2 NeuronCores.

Sharding: adj and the output are row-sharded over N=8192 across 8 cores
(1024 rows each); x and all weights are replicated. Each propagation
adj_loc @ v is a local [1024,8192]@[8192,F] matmul; v is produced
row-sharded and AllGathered between hops.

Precision: propagation matmuls (adj-sided, the bulk of bytes+flops) run
in bf16 with fp32 PSUM accumulation; dense-layer transforms and BN run
in fp32r (full-rate reduced fp32). Measured end-to-end relative error
~2e-3.

Orientation notes:
- "natural"   = rows on partitions (needed for AllGather row-concat and
  as the K axis of the next propagation)
- "transposed" = features on partitions (needed as lhsT of the next
  dense layer; makes BatchNorm affine per-partition)
Pass B and D emit transposed outputs directly; pass A's t1 half and
pass C's s1 half are transposed on the PE with an identity matmul.
b1[0] (hop-0 bias of layer 1) is dropped: a per-column constant shift
is exactly cancelled by the training-mode BatchNorm that follows.

Scheduling: x2-transform output stays SBUF-resident and feeds pass
A-t2a, which runs as two 4-PSUM-bank row-groups so it overlaps the
transform (total PSUM demand of overlapping stages <= 8 banks). The
x1 transform, hop-0 transform and its BN stats run under AllGather1;
s1 transposes run under AllGather3. DMA loads alternate between the
two HWDGE rings (sync/scalar); SBUF->DRAM drains go via SWDGE (gpsimd).
"""
import sys
from contextlib import ExitStack

sys.path.insert(0, "/opt/trn_rl_repo")

import numpy as np

N, IN, H, OUT = 8192, 128, 512, 256
NC = 8
ROWS = N // NC          # 1024 rows per core
KT = N // 128           # 64 k-tiles of the propagation contraction
HT = 3 * H // 128       # 12 feature tiles of h.T
H2T = 3 * OUT // 128    # 6 feature tiles of h2.T
EPS = 1e-5

_BUILT = {}


def build_program():
    """Build and compile the Bass program (cached)."""
    if "nc" in _BUILT:
        return _BUILT["nc"]

    import concourse.bacc as bacc
    import concourse.tile as tile
    import concourse.mybir as mybir
    from concourse.alu_op_type import AluOpType

    f32 = mybir.dt.float32
    f32r = mybir.dt.float32r
    bf16 = mybir.dt.bfloat16
    AF = mybir.ActivationFunctionType
    AX = mybir.AxisListType

    nc = bacc.Bacc("TRN2", target_bir_lowering=False, debug=False,
                   num_devices=NC)

    # ---- external inputs (per-core values supplied by the host) ----
    adjT_d = nc.dram_tensor("adjT", [N, ROWS], bf16, kind="ExternalInput")
    xT_d = nc.dram_tensor("xT", [IN, N], f32r, kind="ExternalInput")
    xTloc_d = nc.dram_tensor("xTloc", [IN, ROWS], f32r, kind="ExternalInput")
    w1cat_d = nc.dram_tensor("w1cat", [IN, 2 * H], f32r, kind="ExternalInput")
    w1h0_d = nc.dram_tensor("w1h0", [IN, H], f32r, kind="ExternalInput")
    b1bc_d = nc.dram_tensor("b1bc", [128, 2 * H], f32, kind="ExternalInput")
    w2cat_d = nc.dram_tensor("w2cat", [3 * H, 2 * OUT], f32r, kind="ExternalInput")
    w2h0_d = nc.dram_tensor("w2h0", [3 * H, OUT], f32r, kind="ExternalInput")
    b2bc_d = nc.dram_tensor("b2bc", [128, 2 * OUT], f32, kind="ExternalInput")
    b2h0T_d = nc.dram_tensor("b2h0T", [128, 2], f32, kind="ExternalInput")
    wf_d = nc.dram_tensor("wf", [3 * OUT, OUT], f32r, kind="ExternalInput")
    bfT_d = nc.dram_tensor("bfT", [128, 2], f32, kind="ExternalInput")
    gcol_d = nc.dram_tensor("gcol", [128, HT], f32, kind="ExternalInput")
    bcol_d = nc.dram_tensor("bcol", [128, HT], f32, kind="ExternalInput")
    ident_d = nc.dram_tensor("ident", [128, 128], f32, kind="ExternalInput")

    outT_d = nc.dram_tensor("outT", [OUT, ROWS], f32, kind="ExternalOutput")

    rg = [list(range(NC))]

    def ring(k):
        return nc.sync if k % 2 == 0 else nc.scalar

    def ring2(k):
        return nc.scalar if k % 2 == 0 else nc.sync

    with tile.TileContext(nc) as tc, ExitStack() as st:
        dram = st.enter_context(tc.tile_pool(name="dram", bufs=1, space="DRAM"))
        P = st.enter_context(tc.tile_pool(name="persist", bufs=1))
        DR = st.enter_context(tc.tile_pool(name="drain", bufs=2))

        # ---- DRAM intermediates ----
        v_dram = dram.tile([N, H], bf16, name="v_dram")   # x1 transform only
        ag1_in = dram.tile([ROWS, H], bf16, name="ag1_in")
        ag1_out = dram.tile([N, H], bf16, name="ag1_out", addr_space="Shared")
        ag2_in = dram.tile([ROWS, 2 * OUT], bf16, name="ag2_in")
        ag2_out = dram.tile([N, 2 * OUT], bf16, name="ag2_out",
                            addr_space="Shared")
        ag3_in = dram.tile([ROWS, OUT], bf16, name="ag3_in")
        ag3_out = dram.tile([N, OUT], bf16, name="ag3_out",
                            addr_space="Shared")
        ar_in = dram.tile([128, 2 * HT], f32, name="ar_in")
        ar_out = dram.tile([128, 2 * HT], f32, name="ar_out",
                           addr_space="Shared")

        # ---- small persistents (to the end) ----
        xTloc_sb = P.tile([IN, ROWS], f32r, name="xTloc_sb")
        nc.scalar.dma_start(xTloc_sb[:], xTloc_d[:, :])
        w1h0_sb = P.tile([IN, H], f32r, name="w1h0_sb")
        nc.scalar.dma_start(w1h0_sb[:], w1h0_d[:, :])
        b2h0T_sb = P.tile([128, 2], f32, name="b2h0T_sb")
        nc.scalar.dma_start(b2h0T_sb[:], b2h0T_d[:, :])
        bfT_sb = P.tile([128, 2], f32, name="bfT_sb")
        nc.scalar.dma_start(bfT_sb[:], bfT_d[:, :])
        gcol_sb = P.tile([128, HT], f32, name="gcol_sb")
        nc.scalar.dma_start(gcol_sb[:], gcol_d[:, :])
        bcol_sb = P.tile([128, HT], f32, name="bcol_sb")
        nc.scalar.dma_start(bcol_sb[:], bcol_d[:, :])
        ident_sb = P.tile([128, 128], f32, name="ident_sb")
        nc.scalar.dma_start(ident_sb[:], ident_d[:, :])
        wf_sb = [P.tile([128, OUT], f32r, name=f"wf{k}") for k in range(H2T)]
        for k in range(H2T):
            nc.scalar.dma_start(wf_sb[k][:], wf_d[k * 128:(k + 1) * 128, :])
        sumc = P.tile([128, HT], f32, name="sumc")
        sqc = P.tile([128, HT], f32, name="sqc")
        stat_g = P.tile([128, 2 * HT], f32, name="stat_g")
        scale_c = P.tile([128, HT], f32, name="scale_c")
        shift_c = P.tile([128, HT], f32, name="shift_c")
        # h2.T (fp32r): tiles 0-1 y0.T, 2-3 s1.T, 4-5 s2b.T
        h2T = [P.tile([128, ROWS], f32r, name=f"h2T{t}") for t in range(H2T)]

        # ============ T1 + pass A (t2a), pipelined ===========================
        # x2v stays SBUF-resident; A-t2a runs as two 4-bank row-groups so it
        # overlaps the transform producing its rhs.
        with (
            tc.tile_pool(name="v2pool", bufs=1) as V2P,
            tc.tile_pool(name="xkpool", bufs=1) as XKP,
            tc.tile_pool(name="w1pool", bufs=1) as W1P,
        ):
            v2sb = [V2P.tile([128, H], bf16, name=f"v2s{k}")
                    for k in range(KT)]
            xk_sb = [XKP.tile([128, 128], f32r, name=f"xk{k}")
                     for k in range(KT)]
            w1cat_sb = W1P.tile([IN, 2 * H], f32r, name="w1cat_sb")
            nc.scalar.dma_start(w1cat_sb[:], w1cat_d[:, :])
            b1bc_sb = W1P.tile([128, 2 * H], f32, name="b1bc_sb")
            nc.scalar.dma_start(b1bc_sb[:], b1bc_d[:, :])

            # T1-x2: v2 = x @ W1[2] + b1[2]  (SBUF-resident, bf16)
            with tc.tile_pool(name="t1ps2", bufs=2, space="PSUM") as T1PS2:
                for k in range(KT):
                    nc.sync.dma_start(xk_sb[k][:],
                                      xT_d[:, k * 128:(k + 1) * 128])
                    vps2 = T1PS2.tile([128, H], f32, name="vps2", tag="vps2")
                    nc.tensor.matmul(vps2[:], xk_sb[k][:],
                                     w1cat_sb[:, H:2 * H],
                                     start=True, stop=True)
                    nc.vector.tensor_tensor(v2sb[k][:], vps2[:],
                                            b1bc_sb[:, H:2 * H],
                                            AluOpType.add)

            # A-t2a group 0: rows m=0..3
            with (
                tc.tile_pool(name="ag0slab", bufs=3) as AS0,
                tc.tile_pool(name="ag0ps", bufs=1, space="PSUM") as APS0,
            ):
                acc0 = [APS0.tile([128, H], f32, name=f"acc0{m}",
                                  tag=f"acc0{m}") for m in range(4)]
                for k in range(KT):
                    aslab = AS0.tile([128, H], bf16, name="aslab", tag="aslab")
                    ring(k).dma_start(aslab[:],
                                      adjT_d[k * 128:(k + 1) * 128, 0:H])
                    for m in range(4):
                        nc.tensor.matmul(acc0[m][:],
                                         aslab[:, m * 128:(m + 1) * 128],
                                         v2sb[k][:], start=(k == 0),
                                         stop=(k == KT - 1))
                for m in range(4):
                    t2a = DR.tile([128, H], bf16, name="t2a", tag="t2a")
                    nc.vector.tensor_copy(t2a[:], acc0[m][:])
                    nc.gpsimd.dma_start(ag1_in[m * 128:(m + 1) * 128, :],
                                        t2a[:])

            # A-t2a group 1: rows m=4..7
            with (
                tc.tile_pool(name="ag1slab", bufs=3) as AS1,
                tc.tile_pool(name="ag1ps", bufs=1, space="PSUM") as APS1,
            ):
                acc1 = [APS1.tile([128, H], f32, name=f"acc1{m}",
                                  tag=f"acc1{m}") for m in range(4)]
                for k in range(KT):
                    aslab = AS1.tile([128, H], bf16, name="aslab", tag="aslab")
                    ring(k).dma_start(aslab[:],
                                      adjT_d[k * 128:(k + 1) * 128, H:ROWS])
                    for m in range(4):
                        nc.tensor.matmul(acc1[m][:],
                                         aslab[:, m * 128:(m + 1) * 128],
                                         v2sb[k][:], start=(k == 0),
                                         stop=(k == KT - 1))
                for m in range(4):
                    t2a = DR.tile([128, H], bf16, name="t2a", tag="t2a")
                    nc.vector.tensor_copy(t2a[:], acc1[m][:])
                    nc.gpsimd.dma_start(
                        ag1_in[(4 + m) * 128:(5 + m) * 128, :], t2a[:])

            nc.gpsimd.collective_compute(
                "AllGather", AluOpType.bypass, replica_groups=rg,
                ins=[ag1_in[:].opt()], outs=[ag1_out[:].opt()])

            # T1-x1: v1 = x @ W1[1] + b1[1] -> DRAM (runs under AllGather1)
            with tc.tile_pool(name="t1ps1", bufs=2, space="PSUM") as T1PS1:
                for k in range(KT):
                    vps1 = T1PS1.tile([128, H], f32, name="vps1", tag="vps1")
                    nc.tensor.matmul(vps1[:], xk_sb[k][:], w1cat_sb[:, 0:H],
                                     start=True, stop=True)
                    v1sb = DR.tile([128, H], bf16, name="v1sb", tag="v1sb")
                    nc.vector.tensor_tensor(v1sb[:], vps1[:],
                                            b1bc_sb[:, 0:H], AluOpType.add)
                    nc.gpsimd.dma_start(v_dram[k * 128:(k + 1) * 128, :],
                                        v1sb[:])

        # hT in fp32r: tiles 0-3 hop0.T, 4-7 t1.T, 8-11 t2b.T.
        # Written pre-norm, then BN+relu normalized IN PLACE.
        PH = st.enter_context(tc.tile_pool(name="hpool", bufs=1))
        hT = [PH.tile([128, ROWS], f32r, name=f"hT{t}") for t in range(HT)]

        # ========== hop0.T + its stats (runs under AllGather1) ===============
        with (
            tc.tile_pool(name="h0ps", bufs=2, space="PSUM") as H0PS,
            tc.tile_pool(name="sqps0", bufs=1, space="PSUM") as SQPS0,
        ):
            for mo in range(4):
                for n in range(2):
                    h0ps = H0PS.tile([128, H], f32, name="h0ps", tag="h0ps")
                    nc.tensor.matmul(h0ps[:],
                                     w1h0_sb[:, mo * 128:(mo + 1) * 128],
                                     xTloc_sb[:, n * H:(n + 1) * H],
                                     start=True, stop=True)
                    nc.vector.tensor_copy(hT[mo][:, n * H:(n + 1) * H],
                                          h0ps[:])
            for t in range(4):
                nc.vector.reduce_sum(sumc[:, t:t + 1], hT[t][:], axis=AX.X)
                sq0 = SQPS0.tile([128, ROWS], f32, name="sq0", tag="sq0")
                nc.vector.scalar_tensor_tensor(
                    sq0[:], hT[t][:], 1.0, hT[t][:],
                    AluOpType.mult, AluOpType.mult,
                    accum_out=sqc[:, t:t + 1])

        # t1 natural (fp32), transposed right after A-t1
        PT1 = st.enter_context(tc.tile_pool(name="t1nat", bufs=1))
        t1_sb = [PT1.tile([128, H], f32, name=f"t1n{m}") for m in range(8)]

        # ================= A-t1: t1 = adj_loc @ x1v (natural) ================
        with (
            tc.tile_pool(name="a2slabs", bufs=3) as AS2,
            tc.tile_pool(name="aps2", bufs=1, space="PSUM") as APS2,
        ):
            acc2 = [APS2.tile([128, H], f32, name=f"ac2{m}", tag=f"ac2{m}")
                    for m in range(8)]
            for k in range(KT):
                aslab = AS2.tile([128, ROWS], bf16, name="aslab", tag="aslab")
                ring(k).dma_start(aslab[:], adjT_d[k * 128:(k + 1) * 128, :])
                v1 = AS2.tile([128, H], bf16, name="v1", tag="v1")
                ring2(k).dma_start(v1[:], v_dram[k * 128:(k + 1) * 128, :])
                for m in range(8):
                    nc.tensor.matmul(acc2[m][:],
                                     aslab[:, m * 128:(m + 1) * 128],
                                     v1[:], start=(k == 0), stop=(k == KT - 1))
            for m in range(8):
                nc.vector.tensor_copy(t1_sb[m][:], acc2[m][:])

        # layer-2 weights, loaded during pass B
        PW2 = st.enter_context(tc.tile_pool(name="w2pool", bufs=1))
        w2cat_sb = [PW2.tile([128, 2 * OUT], f32r, name=f"w2cat{k}")
                    for k in range(HT)]
        for k in range(HT):
            nc.scalar.dma_start(w2cat_sb[k][:],
                                w2cat_d[k * 128:(k + 1) * 128, :])
        w2h0_sb = [PW2.tile([128, OUT], f32r, name=f"w2h0{k}")
                   for k in range(HT)]
        for k in range(HT):
            nc.scalar.dma_start(w2h0_sb[k][:],
                                w2h0_d[k * 128:(k + 1) * 128, :])
        b2bc_sb = PW2.tile([128, 2 * OUT], f32, name="b2bc_sb")
        nc.scalar.dma_start(b2bc_sb[:], b2bc_d[:, :])

        # ========== t1 transposes + stats for t1 tiles =======================
        with (
            tc.tile_pool(name="tps", bufs=4, space="PSUM") as TPS,
            tc.tile_pool(name="sqps1", bufs=1, space="PSUM") as SQPS1,
        ):
            for c in range(4):
                for m in range(8):
                    tp = TPS.tile([128, 128], f32, name="tp", tag="tp")
                    nc.tensor.transpose(tp[:],
                                        t1_sb[m][:, c * 128:(c + 1) * 128],
                                        ident_sb[:])
                    nc.vector.tensor_copy(hT[4 + c][:, m * 128:(m + 1) * 128],
                                          tp[:])
            for t in range(4, 8):
                nc.vector.reduce_sum(sumc[:, t:t + 1], hT[t][:], axis=AX.X)
                sq1 = SQPS1.tile([128, ROWS], f32, name="sq1", tag="sq1")
                nc.vector.scalar_tensor_tensor(
                    sq1[:], hT[t][:], 1.0, hT[t][:],
                    AluOpType.mult, AluOpType.mult,
                    accum_out=sqc[:, t:t + 1])

        # ================= B: t2b.T = (adj_loc @ t2a_full).T =================
        with (
            tc.tile_pool(name="bslabs", bufs=3) as BS,
            tc.tile_pool(name="bps", bufs=1, space="PSUM") as BPS,
        ):
            psb = [BPS.tile([128, H], f32, name=f"psb{i}", tag=f"psb{i}")
                   for i in range(8)]  # i = mo*2+n
            for k in range(KT):
                aslab = BS.tile([128, ROWS], bf16, name="aslab", tag="aslab")
                ring(k).dma_start(aslab[:], adjT_d[k * 128:(k + 1) * 128, :])
                tslab = BS.tile([128, H], bf16, name="tslab", tag="tslab")
                ring2(k).dma_start(tslab[:], ag1_out[k * 128:(k + 1) * 128, :])
                for mo in range(4):
                    for n in range(2):
                        nc.tensor.matmul(
                            psb[mo * 2 + n][:],
                            tslab[:, mo * 128:(mo + 1) * 128],
                            aslab[:, n * H:(n + 1) * H],
                            start=(k == 0), stop=(k == KT - 1))
            for mo in range(4):
                for n in range(2):
                    nc.vector.tensor_copy(hT[8 + mo][:, n * H:(n + 1) * H],
                                          psb[mo * 2 + n][:])

        # ========== stats for t2b tiles + AllReduce ==========================
        with tc.tile_pool(name="sqps2", bufs=1, space="PSUM") as SQPS2:
            for t in range(8, HT):
                nc.vector.reduce_sum(sumc[:, t:t + 1], hT[t][:], axis=AX.X)
                sq2 = SQPS2.tile([128, ROWS], f32, name="sq2", tag="sq2")
                nc.vector.scalar_tensor_tensor(
                    sq2[:], hT[t][:], 1.0, hT[t][:],
                    AluOpType.mult, AluOpType.mult,
                    accum_out=sqc[:, t:t + 1])
        nc.gpsimd.dma_start(ar_in[:, :HT], sumc[:])
        nc.gpsimd.dma_start(ar_in[:, HT:], sqc[:])

        nc.gpsimd.collective_compute(
            "AllReduce", AluOpType.add, replica_groups=rg,
            ins=[ar_in[:].opt()], outs=[ar_out[:].opt()])

        # ========== BN affine params + normalize + relu (in place) ==========
        nc.sync.dma_start(stat_g[:], ar_out[:, :])
        mu = DR.tile([128, HT], f32, name="mu", tag="mu")
        nc.vector.tensor_scalar_mul(mu[:], stat_g[:, :HT], 1.0 / N)
        # ex2 = sumsq/N + eps (eps folded in here; var+eps overall)
        ex2 = DR.tile([128, HT], f32, name="ex2", tag="ex2")
        nc.vector.tensor_scalar(ex2[:], stat_g[:, HT:], 1.0 / N, EPS,
                                AluOpType.mult, AluOpType.add)
        var = DR.tile([128, HT], f32, name="var", tag="var")
        # var = (mu * -1) * mu + ex2
        nc.vector.scalar_tensor_tensor(var[:], mu[:], -1.0, mu[:],
                                       AluOpType.mult, AluOpType.mult)
        nc.vector.tensor_add(var[:], var[:], ex2[:])
        std = DR.tile([128, HT], f32, name="std", tag="std")
        nc.scalar.activation(std[:], var[:], AF.Sqrt)
        rstd = DR.tile([128, HT], f32, name="rstd", tag="rstd")
        nc.vector.reciprocal(rstd[:], std[:])
        nc.vector.tensor_mul(scale_c[:], gcol_sb[:], rstd[:])
        # shift = bcol - mu*scale
        nc.vector.scalar_tensor_tensor(shift_c[:], mu[:], -1.0, scale_c[:],
                                       AluOpType.mult, AluOpType.mult)
        nc.vector.tensor_add(shift_c[:], shift_c[:], bcol_sb[:])
        for t in range(HT):
            nc.scalar.activation(hT[t][:], hT[t][:], AF.Relu,
                                 bias=shift_c[:, t:t + 1],
                                 scale=scale_c[:, t:t + 1])

        # ========== T2: y = [y1|y2] = hn @ [W2[1]|W2[2]] + b =================
        with tc.tile_pool(name="yps", bufs=4, space="PSUM") as YPS:
            for m in range(8):
                yps = YPS.tile([128, 2 * OUT], f32, name="yps", tag="yps")
                for k in range(HT):
                    nc.tensor.matmul(yps[:], hT[k][:, m * 128:(m + 1) * 128],
                                     w2cat_sb[k][:],
                                     start=(k == 0), stop=(k == HT - 1))
                ysb = DR.tile([128, 2 * OUT], bf16, name="ysb", tag="ysb")
                nc.vector.tensor_tensor(ysb[:], yps[:], b2bc_sb[:],
                                        AluOpType.add)
                nc.gpsimd.dma_start(ag2_in[m * 128:(m + 1) * 128, :], ysb[:])

        nc.gpsimd.collective_compute(
            "AllGather", AluOpType.bypass, replica_groups=rg,
            ins=[ag2_in[:].opt()], outs=[ag2_out[:].opt()])

        # y0.T = (hn @ W2[0]).T + b2[0]  (runs under AllGather2)
        with tc.tile_pool(name="y0ps", bufs=2, space="PSUM") as Y0PS:
            for mo in range(2):
                for n in range(2):
                    y0ps = Y0PS.tile([128, H], f32, name="y0ps", tag="y0ps")
                    for k in range(HT):
                        nc.tensor.matmul(
                            y0ps[:], w2h0_sb[k][:, mo * 128:(mo + 1) * 128],
                            hT[k][:, n * H:(n + 1) * H],
                            start=(k == 0), stop=(k == HT - 1))
                    nc.vector.tensor_scalar_add(h2T[mo][:, n * H:(n + 1) * H],
                                                y0ps[:],
                                                b2h0T_sb[:, mo:mo + 1])

        # s1 natural (fp32), transposed under AllGather3
        PS1 = st.enter_context(tc.tile_pool(name="s1nat", bufs=1))
        s1_sb = [PS1.tile([128, OUT], f32, name=f"s1n{m}") for m in range(8)]

        # ========== C: [s1|s2a] = adj_loc @ [y1|y2] (natural) ================
        with (
            tc.tile_pool(name="cslabs", bufs=3) as CS,
            tc.tile_pool(name="cps", bufs=1, space="PSUM") as CPS,
        ):
            psc = [CPS.tile([128, 2 * OUT], f32, name=f"psc{m}", tag=f"psc{m}")
                   for m in range(8)]
            for k in range(KT):
                aslab = CS.tile([128, ROWS], bf16, name="aslab", tag="aslab")
                ring(k).dma_start(aslab[:], adjT_d[k * 128:(k + 1) * 128, :])
                yslab = CS.tile([128, 2 * OUT], bf16, name="yslab", tag="yslab")
                ring2(k).dma_start(yslab[:], ag2_out[k * 128:(k + 1) * 128, :])
                for m in range(8):
                    nc.tensor.matmul(psc[m][:],
                                     aslab[:, m * 128:(m + 1) * 128],
                                     yslab[:], start=(k == 0),
                                     stop=(k == KT - 1))
            for m in range(8):
                nc.vector.tensor_copy(s1_sb[m][:], psc[m][:, :OUT])
                s2a = DR.tile([128, OUT], bf16, name="s2a", tag="s2a")
                nc.vector.tensor_copy(s2a[:], psc[m][:, OUT:])
                nc.gpsimd.dma_start(ag3_in[m * 128:(m + 1) * 128, :], s2a[:])

        nc.gpsimd.collective_compute(
            "AllGather", AluOpType.bypass, replica_groups=rg,
            ins=[ag3_in[:].opt()], outs=[ag3_out[:].opt()])

        # ========== s1 transposes (under AllGather3) + D ====================
        with (
            tc.tile_pool(name="tps2", bufs=4, space="PSUM") as TPS2,
            tc.tile_pool(name="dslabs", bufs=3) as DS,
            tc.tile_pool(name="dps", bufs=1, space="PSUM") as DPS,
        ):
            # s1.T via PE transpose (independent of AllGather3)
            for c in range(2):
                for m in range(8):
                    tp2 = TPS2.tile([128, 128], f32, name="tp2", tag="tp2")
                    nc.tensor.transpose(tp2[:],
                                        s1_sb[m][:, c * 128:(c + 1) * 128],
                                        ident_sb[:])
                    nc.vector.tensor_copy(h2T[2 + c][:, m * 128:(m + 1) * 128],
                                          tp2[:])
            # D: s2b.T = (adj_loc @ s2a_full).T; sslab pair-loaded (2 k-slabs)
            psd = [DPS.tile([128, H], f32, name=f"psd{i}", tag=f"psd{i}")
                   for i in range(4)]  # i = mo*2+n
            ag3_pairs = ag3_out[:].rearrange("(a two p) f -> a p two f",
                                             two=2, p=128)
            for kk in range(KT // 2):
                aslab = DS.tile([128, ROWS], bf16, name="aslab", tag="aslab")
                ring(kk).dma_start(
                    aslab[:],
                    adjT_d[2 * kk * 128:(2 * kk + 1) * 128, :])
                aslab2 = DS.tile([128, ROWS], bf16, name="aslab2", tag="aslab2")
                ring2(kk).dma_start(
                    aslab2[:],
                    adjT_d[(2 * kk + 1) * 128:(2 * kk + 2) * 128, :])
                spair = DS.tile([128, 2 * OUT], bf16, name="spair", tag="spair")
                ring(kk + 1).dma_start(
                    spair[:].rearrange("p (two f) -> p two f", two=2),
                    ag3_pairs[kk])
                for t, asl in ((0, aslab), (1, aslab2)):
                    k = 2 * kk + t
                    for mo in range(2):
                        for n in range(2):
                            nc.tensor.matmul(
                                psd[mo * 2 + n][:],
                                spair[:, t * OUT + mo * 128:
                                      t * OUT + (mo + 1) * 128],
                                asl[:, n * H:(n + 1) * H],
                                start=(k == 0), stop=(k == KT - 1))
            for mo in range(2):
                for n in range(2):
                    nc.vector.tensor_copy(h2T[4 + mo][:, n * H:(n + 1) * H],
                                          psd[mo * 2 + n][:])

        # ========== final: out.T = (h2 @ Wf).T + bf ==========================
        with tc.tile_pool(name="fps", bufs=2, space="PSUM") as FPS:
            for mo in range(2):
                for n in range(2):
                    fps = FPS.tile([128, H], f32, name="fps", tag="fps")
                    for k in range(H2T):
                        nc.tensor.matmul(
                            fps[:], wf_sb[k][:, mo * 128:(mo + 1) * 128],
                            h2T[k][:, n * H:(n + 1) * H],
                            start=(k == 0), stop=(k == H2T - 1))
                    osb = DR.tile([128, H], f32, name="osb", tag="osb")
                    nc.vector.tensor_scalar_add(osb[:], fps[:],
                                                bfT_sb[:, mo:mo + 1])
                    nc.sync.dma_start(
                        outT_d[mo * 128:(mo + 1) * 128, n * H:(n + 1) * H],
                        osb[:])

    nc.compile()
    _BUILT["nc"] = nc
    return nc


def prep_in_maps(x, adj, W1, b1, W2, b2, gamma, beta, Wf, bf):
    """Host-side sharding / layout prep. Returns one input dict per core."""
    import ml_dtypes

    x = np.asarray(x, dtype=np.float32)
    adj = np.asarray(adj, dtype=np.float32)
    W1 = np.asarray(W1, dtype=np.float32)
    b1 = np.asarray(b1, dtype=np.float32)
    W2 = np.asarray(W2, dtype=np.float32)
    b2 = np.asarray(b2, dtype=np.float32)
    gamma = np.asarray(gamma, dtype=np.float32)
    beta = np.asarray(beta, dtype=np.float32)
    Wf = np.asarray(Wf, dtype=np.float32)
    bf = np.asarray(bf, dtype=np.float32)

    xT = np.ascontiguousarray(x.T)                       # [128, 8192]
    w1cat = np.ascontiguousarray(
        np.concatenate([W1[1], W1[2]], axis=1))          # [128, 1024]
    b1cat = np.concatenate([b1[1], b1[2]])               # [1024]
    b1bc = np.ascontiguousarray(
        np.broadcast_to(b1cat[None, :], (128, 2 * H)))
    w2cat = np.ascontiguousarray(
        np.concatenate([W2[1], W2[2]], axis=1))          # [1536, 512]
    b2cat = np.concatenate([b2[1], b2[2]])               # [512]
    b2bc = np.ascontiguousarray(
        np.broadcast_to(b2cat[None, :], (128, 2 * OUT)))
    gcol = np.ascontiguousarray(gamma.reshape(HT, 128).T)
    bcol = np.ascontiguousarray(beta.reshape(HT, 128).T)
    ident = np.eye(128, dtype=np.float32)

    shared = {
        "xT": xT,
        "w1cat": w1cat,
        "w1h0": np.ascontiguousarray(W1[0]),
        "b1bc": b1bc,
        "w2cat": w2cat,
        "w2h0": np.ascontiguousarray(W2[0]),
        "b2bc": b2bc,
        "b2h0T": np.ascontiguousarray(b2[0].reshape(2, 128).T),
        "wf": np.ascontiguousarray(Wf),
        "bfT": np.ascontiguousarray(bf.reshape(2, 128).T),
        "gcol": gcol,
        "bcol": bcol,
        "ident": ident,
    }
    in_maps = []
    for d in range(NC):
        r0, r1 = d * ROWS, (d + 1) * ROWS
        m = dict(shared)
        m["adjT"] = np.ascontiguousarray(
            adj[r0:r1].T.astype(ml_dtypes.bfloat16))     # [8192, 1024] bf16
        m["xTloc"] = np.ascontiguousarray(x[r0:r1].T)    # [128, 1024]
        in_maps.append(m)
    return in_maps


def run_on_hw(in_maps, trace=False):
    from concourse import bass_utils
    nc = build_program()
    return bass_utils.run_bass_kernel_spmd(
        nc, in_maps, core_ids=list(range(NC)), trace=trace)


def kernel(x, adj, W1, b1, W2, b2, gamma, beta, Wf, bf):
    in_maps = prep_in_maps(x, adj, W1, b1, W2, b2, gamma, beta, Wf, bf)
    res = run_on_hw(in_maps)
    out = np.concatenate(
        [np.ascontiguousarray(res.results[d]["outT"].T) for d in range(NC)],
        axis=0)
    return out.astype(np.float32)


# revision 15
# speedup vs baseline: 1.3803x; 1.0242x over previous
"""MixHop (2-hop) GNN forward on 8 TRN2 NeuronCores.

Sharding: adj and the output are row-sharded over N=8192 across 8 cores
(1024 rows each); x and all weights are replicated. Each propagation
adj_loc @ v is a local [1024,8192]@[8192,F] matmul; v is produced
row-sharded and AllGathered between hops.

Precision: propagation matmuls (adj-sided, the bulk of bytes+flops) run
in bf16 with fp32 PSUM accumulation; dense-layer transforms and BN run
in fp32r (full-rate reduced fp32). Measured end-to-end relative error
~2e-3.

Orientation notes:
- "natural"   = rows on partitions (needed for AllGather row-concat and
  as the K axis of the next propagation)
- "transposed" = features on partitions (needed as lhsT of the next
  dense layer; makes BatchNorm affine per-partition)
Pass B and D emit transposed outputs directly; pass A's t1 half and
pass C's s1 half are transposed on the PE with an identity matmul.
b1[0] (hop-0 bias of layer 1) is dropped: a per-column constant shift
is exactly cancelled by the training-mode BatchNorm that follows.

Scheduling notes:
- Propagation k-loops process 4 k-slabs per iteration -> ~4.3us
  contiguous matmul bursts, long enough to hold the PE HAM clock-gate
  at full rate (short bursts leave the PE throttled to 1.2 GHz).
- x2-transform output stays SBUF-resident and feeds A-t2a, which runs
  as two 4-PSUM-bank row-groups so it overlaps the transform.
- The x1 transform + hop-0 transform + A-t1 row-group 0 run under
  AllGather1; BN stats/AllReduce/normalize for h tiles 0..7 run under
  pass B (only the t2b tiles' stats remain on the critical path);
  y0.T runs under AllGather2; s1 transposes run under AllGather3.
- DMA loads alternate between the two HWDGE rings (sync/scalar);
  SBUF->DRAM drains go via SWDGE (gpsimd).
"""
import sys
from contextlib import ExitStack

sys.path.insert(0, "/opt/trn_rl_repo")

import numpy as np

N, IN, H, OUT = 8192, 128, 512, 256
NC = 8
ROWS = N // NC          # 1024 rows per core
KT = N // 128           # 64 k-tiles of the propagation contraction
HT = 3 * H // 128       # 12 feature tiles of h.T
H2T = 3 * OUT // 128    # 6 feature tiles of h2.T
EPS = 1e-5

_BUILT = {}


def build_program():
    """Build and compile the Bass program (cached)."""
    if "nc" in _BUILT:
        return _BUILT["nc"]

    import concourse.bacc as bacc
    import concourse.tile as tile
    import concourse.mybir as mybir
    from concourse.alu_op_type import AluOpType

    f32 = mybir.dt.float32
    f32r = mybir.dt.float32r
    bf16 = mybir.dt.bfloat16
    AF = mybir.ActivationFunctionType
    AX = mybir.AxisListType

    nc = bacc.Bacc("TRN2", target_bir_lowering=False, debug=False,
                   num_devices=NC)

    # ---- external inputs (per-core values supplied by the host) ----
    adjT_d = nc.dram_tensor("adjT", [N, ROWS], bf16, kind="ExternalInput")
    xT_d = nc.dram_tensor("xT", [IN, N], f32r, kind="ExternalInput")
    xTloc_d = nc.dram_tensor("xTloc", [IN, ROWS], f32r, kind="ExternalInput")
    w1cat_d = nc.dram_tensor("w1cat", [IN, 2 * H], f32r, kind="ExternalInput")
    w1h0_d = nc.dram_tensor("w1h0", [IN, H], f32r, kind="ExternalInput")
    b1bc_d = nc.dram_tensor("b1bc", [128, 2 * H], f32, kind="ExternalInput")
    w2cat_d = nc.dram_tensor("w2cat", [3 * H, 2 * OUT], f32r, kind="ExternalInput")
    w2h0_d = nc.dram_tensor("w2h0", [3 * H, OUT], f32r, kind="ExternalInput")
    b2bc_d = nc.dram_tensor("b2bc", [128, 2 * OUT], f32, kind="ExternalInput")
    b2h0T_d = nc.dram_tensor("b2h0T", [128, 2], f32, kind="ExternalInput")
    wf_d = nc.dram_tensor("wf", [3 * OUT, OUT], f32r, kind="ExternalInput")
    bfT_d = nc.dram_tensor("bfT", [128, 2], f32, kind="ExternalInput")
    gcol_d = nc.dram_tensor("gcol", [128, HT], f32, kind="ExternalInput")
    bcol_d = nc.dram_tensor("bcol", [128, HT], f32, kind="ExternalInput")
    ident_d = nc.dram_tensor("ident", [128, 128], f32, kind="ExternalInput")

    outT_d = nc.dram_tensor("outT", [OUT, ROWS], f32, kind="ExternalOutput")

    rg = [list(range(NC))]

    def ring(k):
        return nc.sync if k % 2 == 0 else nc.scalar

    def ring2(k):
        return nc.scalar if k % 2 == 0 else nc.sync

    with tile.TileContext(nc) as tc, ExitStack() as st:
        dram = st.enter_context(tc.tile_pool(name="dram", bufs=1, space="DRAM"))
        P = st.enter_context(tc.tile_pool(name="persist", bufs=1))
        DR = st.enter_context(tc.tile_pool(name="drain", bufs=2))

        # ---- DRAM intermediates ----
        v_dram = dram.tile([N, H], bf16, name="v_dram")   # x1 transform only
        ag1_in = dram.tile([ROWS, H], bf16, name="ag1_in")
        ag1_out = dram.tile([N, H], bf16, name="ag1_out", addr_space="Shared")
        ag2_in = dram.tile([ROWS, 2 * OUT], bf16, name="ag2_in")
        ag2_out = dram.tile([N, 2 * OUT], bf16, name="ag2_out",
                            addr_space="Shared")
        ag3_in = dram.tile([ROWS, OUT], bf16, name="ag3_in")
        ag3_out = dram.tile([N, OUT], bf16, name="ag3_out",
                            addr_space="Shared")
        ar_a_in = dram.tile([128, 16], f32, name="ar_a_in")
        ar_a_out = dram.tile([128, 16], f32, name="ar_a_out",
                             addr_space="Shared")
        ar_b_in = dram.tile([128, 8], f32, name="ar_b_in")
        ar_b_out = dram.tile([128, 8], f32, name="ar_b_out",
                             addr_space="Shared")

        # ---- small persistents (to the end) ----
        xTloc_sb = P.tile([IN, ROWS], f32r, name="xTloc_sb")
        nc.scalar.dma_start(xTloc_sb[:], xTloc_d[:, :])
        w1h0_sb = P.tile([IN, H], f32r, name="w1h0_sb")
        nc.scalar.dma_start(w1h0_sb[:], w1h0_d[:, :])
        b2h0T_sb = P.tile([128, 2], f32, name="b2h0T_sb")
        nc.scalar.dma_start(b2h0T_sb[:], b2h0T_d[:, :])
        bfT_sb = P.tile([128, 2], f32, name="bfT_sb")
        nc.scalar.dma_start(bfT_sb[:], bfT_d[:, :])
        gcol_sb = P.tile([128, HT], f32, name="gcol_sb")
        nc.scalar.dma_start(gcol_sb[:], gcol_d[:, :])
        bcol_sb = P.tile([128, HT], f32, name="bcol_sb")
        nc.scalar.dma_start(bcol_sb[:], bcol_d[:, :])
        ident_sb = P.tile([128, 128], f32, name="ident_sb")
        nc.scalar.dma_start(ident_sb[:], ident_d[:, :])
        wf_sb = [P.tile([128, OUT], f32r, name=f"wf{k}") for k in range(H2T)]
        for k in range(H2T):
            nc.scalar.dma_start(wf_sb[k][:], wf_d[k * 128:(k + 1) * 128, :])
        sumc = P.tile([128, HT], f32, name="sumc")
        sqc = P.tile([128, HT], f32, name="sqc")
        scale_c = P.tile([128, HT], f32, name="scale_c")
        shift_c = P.tile([128, HT], f32, name="shift_c")
        stat_a = P.tile([128, 16], f32, name="stat_a")
        stat_b = P.tile([128, 8], f32, name="stat_b")
        # h2.T (fp32r): tiles 0-1 y0.T, 2-3 s1.T, 4-5 s2b.T
        h2T = [P.tile([128, ROWS], f32r, name=f"h2T{t}") for t in range(H2T)]

        def bn_affine(stat, lo, hi):
            """Compute scale/shift columns [lo,hi) from gathered stats
            (stat holds [sum | sumsq] for hi-lo tiles)."""
            w = hi - lo
            mu = DR.tile([128, w], f32, name="mu", tag=f"mu{lo}")
            nc.vector.tensor_scalar_mul(mu[:], stat[:, :w], 1.0 / N)
            ex2 = DR.tile([128, w], f32, name="ex2", tag=f"ex2{lo}")
            nc.vector.tensor_scalar(ex2[:], stat[:, w:2 * w], 1.0 / N, EPS,
                                    AluOpType.mult, AluOpType.add)
            var = DR.tile([128, w], f32, name="var", tag=f"var{lo}")
            nc.vector.scalar_tensor_tensor(var[:], mu[:], -1.0, mu[:],
                                           AluOpType.mult, AluOpType.mult)
            nc.vector.tensor_add(var[:], var[:], ex2[:])
            std = DR.tile([128, w], f32, name="std", tag=f"std{lo}")
            nc.scalar.activation(std[:], var[:], AF.Sqrt)
            rstd = DR.tile([128, w], f32, name="rstd", tag=f"rstd{lo}")
            nc.vector.reciprocal(rstd[:], std[:])
            nc.vector.tensor_mul(scale_c[:, lo:hi], gcol_sb[:, lo:hi], rstd[:])
            nc.vector.scalar_tensor_tensor(shift_c[:, lo:hi], mu[:], -1.0,
                                           scale_c[:, lo:hi],
                                           AluOpType.mult, AluOpType.mult)
            nc.vector.tensor_add(shift_c[:, lo:hi], shift_c[:, lo:hi],
                                 bcol_sb[:, lo:hi])

        # ============ T1 + pass A (t2a), pipelined ===========================
        with (
            tc.tile_pool(name="v2pool", bufs=1) as V2P,
            tc.tile_pool(name="xkpool", bufs=1) as XKP,
            tc.tile_pool(name="w1pool", bufs=1) as W1P,
        ):
            v2sb = [V2P.tile([128, H], bf16, name=f"v2s{k}")
                    for k in range(KT)]
            xk_sb = [XKP.tile([128, 128], f32r, name=f"xk{k}")
                     for k in range(KT)]
            w1cat_sb = W1P.tile([IN, 2 * H], f32r, name="w1cat_sb")
            nc.scalar.dma_start(w1cat_sb[:], w1cat_d[:, :])
            b1bc_sb = W1P.tile([128, 2 * H], f32, name="b1bc_sb")
            nc.scalar.dma_start(b1bc_sb[:], b1bc_d[:, :])

            # T1-x2: v2 = x @ W1[2] + b1[2]  (SBUF-resident, bf16)
            with tc.tile_pool(name="t1ps2", bufs=2, space="PSUM") as T1PS2:
                for k in range(KT):
                    nc.sync.dma_start(xk_sb[k][:],
                                      xT_d[:, k * 128:(k + 1) * 128])
                    vps2 = T1PS2.tile([128, H], f32, name="vps2", tag="vps2")
                    nc.tensor.matmul(vps2[:], xk_sb[k][:],
                                     w1cat_sb[:, H:2 * H],
                                     start=True, stop=True)
                    nc.vector.tensor_tensor(v2sb[k][:], vps2[:],
                                            b1bc_sb[:, H:2 * H],
                                            AluOpType.add)

            # A-t2a row-groups: 4 k-slabs per iteration (16-MM bursts)
            for g in range(2):
                with (
                    tc.tile_pool(name=f"ag{g}slab", bufs=2) as AS,
                    tc.tile_pool(name=f"ag{g}ps", bufs=1, space="PSUM") as APS,
                ):
                    acc = [APS.tile([128, H], f32, name=f"acc{g}{m}",
                                    tag=f"acc{g}{m}") for m in range(4)]
                    for q in range(KT // 4):
                        slabs = []
                        for t in range(4):
                            k = 4 * q + t
                            asl = AS.tile([128, H], bf16, name=f"asl{t}",
                                          tag=f"asl{t}")
                            ring(t).dma_start(
                                asl[:],
                                adjT_d[k * 128:(k + 1) * 128,
                                       g * H:(g + 1) * H])
                            slabs.append(asl)
                        for t in range(4):
                            k = 4 * q + t
                            for m in range(4):
                                nc.tensor.matmul(
                                    acc[m][:],
                                    slabs[t][:, m * 128:(m + 1) * 128],
                                    v2sb[k][:], start=(k == 0),
                                    stop=(k == KT - 1))
                    for m in range(4):
                        t2a = DR.tile([128, H], bf16, name="t2a", tag="t2a")
                        nc.vector.tensor_copy(t2a[:], acc[m][:])
                        nc.gpsimd.dma_start(
                            ag1_in[(4 * g + m) * 128:(4 * g + m + 1) * 128, :],
                            t2a[:])

            nc.gpsimd.collective_compute(
                "AllGather", AluOpType.bypass, replica_groups=rg,
                ins=[ag1_in[:].opt()], outs=[ag1_out[:].opt()])

            # T1-x1: v1 = x @ W1[1] + b1[1] -> DRAM (runs under AllGather1)
            with tc.tile_pool(name="t1ps1", bufs=2, space="PSUM") as T1PS1:
                for k in range(KT):
                    vps1 = T1PS1.tile([128, H], f32, name="vps1", tag="vps1")
                    nc.tensor.matmul(vps1[:], xk_sb[k][:], w1cat_sb[:, 0:H],
                                     start=True, stop=True)
                    v1sb = DR.tile([128, H], bf16, name="v1sb", tag="v1sb")
                    nc.vector.tensor_tensor(v1sb[:], vps1[:],
                                            b1bc_sb[:, 0:H], AluOpType.add)
                    nc.gpsimd.dma_start(v_dram[k * 128:(k + 1) * 128, :],
                                        v1sb[:])

        # hT in fp32r: tiles 0-3 hop0.T, 4-7 t1.T, 8-11 t2b.T.
        # Written pre-norm, then BN+relu normalized IN PLACE.
        PH = st.enter_context(tc.tile_pool(name="hpool", bufs=1))
        hT = [PH.tile([128, ROWS], f32r, name=f"hT{t}") for t in range(HT)]

        # ========== hop0.T + its stats (runs under AllGather1) ===============
        with (
            tc.tile_pool(name="h0ps", bufs=2, space="PSUM") as H0PS,
            tc.tile_pool(name="sqps0", bufs=1, space="PSUM") as SQPS0,
        ):
            for mo in range(4):
                for n in range(2):
                    h0ps = H0PS.tile([128, H], f32, name="h0ps", tag="h0ps")
                    nc.tensor.matmul(h0ps[:],
                                     w1h0_sb[:, mo * 128:(mo + 1) * 128],
                                     xTloc_sb[:, n * H:(n + 1) * H],
                                     start=True, stop=True)
                    nc.vector.tensor_copy(hT[mo][:, n * H:(n + 1) * H],
                                          h0ps[:])
            for t in range(4):
                nc.vector.reduce_sum(sumc[:, t:t + 1], hT[t][:], axis=AX.X)
                sq0 = SQPS0.tile([128, ROWS], f32, name="sq0", tag="sq0")
                nc.vector.scalar_tensor_tensor(
                    sq0[:], hT[t][:], 1.0, hT[t][:],
                    AluOpType.mult, AluOpType.mult,
                    accum_out=sqc[:, t:t + 1])

        # t1 natural (fp32), transposed as soon as each row-group lands
        PT1 = st.enter_context(tc.tile_pool(name="t1nat", bufs=1))
        t1_sb = [PT1.tile([128, H], f32, name=f"t1n{m}") for m in range(8)]

        # ========= A-t1 row-groups (g0 under AllGather1) + transposes ========
        for g in range(2):
            with (
                tc.tile_pool(name=f"a2slab{g}", bufs=2) as AS2,
                tc.tile_pool(name=f"aps2{g}", bufs=1, space="PSUM") as APS2,
            ):
                acc2 = [APS2.tile([128, H], f32, name=f"ac2{g}{m}",
                                  tag=f"ac2{g}{m}") for m in range(4)]
                for q in range(KT // 4):
                    slabs = []
                    for t in range(4):
                        k = 4 * q + t
                        asl = AS2.tile([128, H], bf16, name=f"a2s{t}",
                                       tag=f"a2s{t}")
                        ring(t).dma_start(
                            asl[:], adjT_d[k * 128:(k + 1) * 128,
                                           g * H:(g + 1) * H])
                        slabs.append(asl)
                    v1p = AS2.tile([128, 2 * H], bf16, name="v1p", tag="v1p")
                    nc.scalar.dma_start(
                        v1p[:].rearrange("p (two f) -> p two f", two=2),
                        v_dram[:].rearrange("(a two p) f -> a p two f",
                                            two=2, p=128)[2 * q])
                    v1p2 = AS2.tile([128, 2 * H], bf16, name="v1p2",
                                    tag="v1p2")
                    nc.sync.dma_start(
                        v1p2[:].rearrange("p (two f) -> p two f", two=2),
                        v_dram[:].rearrange("(a two p) f -> a p two f",
                                            two=2, p=128)[2 * q + 1])
                    vv = [v1p[:, 0:H], v1p[:, H:2 * H],
                          v1p2[:, 0:H], v1p2[:, H:2 * H]]
                    for t in range(4):
                        k = 4 * q + t
                        for m in range(4):
                            nc.tensor.matmul(
                                acc2[m][:],
                                slabs[t][:, m * 128:(m + 1) * 128],
                                vv[t], start=(k == 0), stop=(k == KT - 1))
                for m in range(4):
                    nc.vector.tensor_copy(t1_sb[4 * g + m][:], acc2[m][:])
            # transposes for this row-group's t1 tiles
            with tc.tile_pool(name=f"tps{g}", bufs=4, space="PSUM") as TPS:
                for c in range(4):
                    for m in range(4 * g, 4 * g + 4):
                        tp = TPS.tile([128, 128], f32, name="tp", tag="tp")
                        nc.tensor.transpose(
                            tp[:], t1_sb[m][:, c * 128:(c + 1) * 128],
                            ident_sb[:])
                        nc.vector.tensor_copy(
                            hT[4 + c][:, m * 128:(m + 1) * 128], tp[:])

        # ========= t1 stats + AllReduce-a + normalize tiles 0..7 =============
        # (everything below until pass-B's drain runs concurrently with B)
        with tc.tile_pool(name="sqps1", bufs=1, space="PSUM") as SQPS1:
            for t in range(4, 8):
                nc.vector.reduce_sum(sumc[:, t:t + 1], hT[t][:], axis=AX.X)
                sq1 = SQPS1.tile([128, ROWS], f32, name="sq1", tag="sq1")
                nc.vector.scalar_tensor_tensor(
                    sq1[:], hT[t][:], 1.0, hT[t][:],
                    AluOpType.mult, AluOpType.mult,
                    accum_out=sqc[:, t:t + 1])
        nc.gpsimd.dma_start(ar_a_in[:, 0:8], sumc[:, 0:8])
        nc.gpsimd.dma_start(ar_a_in[:, 8:16], sqc[:, 0:8])
        nc.gpsimd.collective_compute(
            "AllReduce", AluOpType.add, replica_groups=rg,
            ins=[ar_a_in[:].opt()], outs=[ar_a_out[:].opt()])
        nc.sync.dma_start(stat_a[:], ar_a_out[:, :])
        bn_affine(stat_a, 0, 8)
        for t in range(8):
            nc.scalar.activation(hT[t][:], hT[t][:], AF.Relu,
                                 bias=shift_c[:, t:t + 1],
                                 scale=scale_c[:, t:t + 1])

        # layer-2 weights, loaded during pass B
        PW2 = st.enter_context(tc.tile_pool(name="w2pool", bufs=1))
        w2cat_sb = [PW2.tile([128, 2 * OUT], f32r, name=f"w2cat{k}")
                    for k in range(HT)]
        for k in range(HT):
            nc.scalar.dma_start(w2cat_sb[k][:],
                                w2cat_d[k * 128:(k + 1) * 128, :])
        w2h0_sb = [PW2.tile([128, OUT], f32r, name=f"w2h0{k}")
                   for k in range(HT)]
        for k in range(HT):
            nc.scalar.dma_start(w2h0_sb[k][:],
                                w2h0_d[k * 128:(k + 1) * 128, :])
        b2bc_sb = PW2.tile([128, 2 * OUT], f32, name="b2bc_sb")
        nc.scalar.dma_start(b2bc_sb[:], b2bc_d[:, :])

        # ================= B: t2b.T = (adj_loc @ t2a_full).T =================
        with (
            tc.tile_pool(name="bslabs", bufs=2) as BS,
            tc.tile_pool(name="bps", bufs=1, space="PSUM") as BPS,
        ):
            psb = [BPS.tile([128, H], f32, name=f"psb{i}", tag=f"psb{i}")
                   for i in range(8)]  # i = mo*2+n
            for q in range(KT // 2):
                aslab = BS.tile([128, ROWS], bf16, name="aslab", tag="aslab")
                nc.sync.dma_start(aslab[:],
                                  adjT_d[2 * q * 128:(2 * q + 1) * 128, :])
                aslab2 = BS.tile([128, ROWS], bf16, name="aslab2",
                                 tag="aslab2")
                nc.scalar.dma_start(
                    aslab2[:], adjT_d[(2 * q + 1) * 128:(2 * q + 2) * 128, :])
                tsp = BS.tile([128, 2 * H], bf16, name="tsp", tag="tsp")
                nc.scalar.dma_start(
                    tsp[:].rearrange("p (two f) -> p two f", two=2),
                    ag1_out[:].rearrange("(a two p) f -> a p two f",
                                         two=2, p=128)[q])
                for t, asl in ((0, aslab), (1, aslab2)):
                    k = 2 * q + t
                    for mo in range(4):
                        for n in range(2):
                            nc.tensor.matmul(
                                psb[mo * 2 + n][:],
                                tsp[:, t * H + mo * 128:
                                    t * H + (mo + 1) * 128],
                                asl[:, n * H:(n + 1) * H],
                                start=(k == 0), stop=(k == KT - 1))
            for mo in range(4):
                for n in range(2):
                    nc.vector.tensor_copy(hT[8 + mo][:, n * H:(n + 1) * H],
                                          psb[mo * 2 + n][:])

        # ========== stats for t2b + AllReduce-b + normalize 8..11 ============
        with tc.tile_pool(name="sqps2", bufs=1, space="PSUM") as SQPS2:
            for t in range(8, HT):
                nc.vector.reduce_sum(sumc[:, t:t + 1], hT[t][:], axis=AX.X)
                sq2 = SQPS2.tile([128, ROWS], f32, name="sq2", tag="sq2")
                nc.vector.scalar_tensor_tensor(
                    sq2[:], hT[t][:], 1.0, hT[t][:],
                    AluOpType.mult, AluOpType.mult,
                    accum_out=sqc[:, t:t + 1])
        nc.gpsimd.dma_start(ar_b_in[:, 0:4], sumc[:, 8:12])
        nc.gpsimd.dma_start(ar_b_in[:, 4:8], sqc[:, 8:12])
        nc.gpsimd.collective_compute(
            "AllReduce", AluOpType.add, replica_groups=rg,
            ins=[ar_b_in[:].opt()], outs=[ar_b_out[:].opt()])
        nc.sync.dma_start(stat_b[:], ar_b_out[:, :])
        bn_affine(stat_b, 8, HT)
        for t in range(8, HT):
            nc.scalar.activation(hT[t][:], hT[t][:], AF.Relu,
                                 bias=shift_c[:, t:t + 1],
                                 scale=scale_c[:, t:t + 1])

        # ========== T2: y = [y1|y2] = hn @ [W2[1]|W2[2]] + b =================
        with tc.tile_pool(name="yps", bufs=4, space="PSUM") as YPS:
            for m in range(8):
                yps = YPS.tile([128, 2 * OUT], f32, name="yps", tag="yps")
                for k in range(HT):
                    nc.tensor.matmul(yps[:], hT[k][:, m * 128:(m + 1) * 128],
                                     w2cat_sb[k][:],
                                     start=(k == 0), stop=(k == HT - 1))
                ysb = DR.tile([128, 2 * OUT], bf16, name="ysb", tag="ysb")
                nc.vector.tensor_tensor(ysb[:], yps[:], b2bc_sb[:],
                                        AluOpType.add)
                nc.gpsimd.dma_start(ag2_in[m * 128:(m + 1) * 128, :], ysb[:])

        nc.gpsimd.collective_compute(
            "AllGather", AluOpType.bypass, replica_groups=rg,
            ins=[ag2_in[:].opt()], outs=[ag2_out[:].opt()])

        # y0.T = (hn @ W2[0]).T + b2[0]  (runs under AllGather2)
        with tc.tile_pool(name="y0ps", bufs=2, space="PSUM") as Y0PS:
            for mo in range(2):
                for n in range(2):
                    y0ps = Y0PS.tile([128, H], f32, name="y0ps", tag="y0ps")
                    for k in range(HT):
                        nc.tensor.matmul(
                            y0ps[:], w2h0_sb[k][:, mo * 128:(mo + 1) * 128],
                            hT[k][:, n * H:(n + 1) * H],
                            start=(k == 0), stop=(k == HT - 1))
                    nc.vector.tensor_scalar_add(h2T[mo][:, n * H:(n + 1) * H],
                                                y0ps[:],
                                                b2h0T_sb[:, mo:mo + 1])

        # s1 natural (fp32), transposed under AllGather3
        PS1 = st.enter_context(tc.tile_pool(name="s1nat", bufs=1))
        s1_sb = [PS1.tile([128, OUT], f32, name=f"s1n{m}") for m in range(8)]

        # ========== C: [s1|s2a] = adj_loc @ [y1|y2] (natural) ================
        with (
            tc.tile_pool(name="cslabs", bufs=2) as CS,
            tc.tile_pool(name="cps", bufs=1, space="PSUM") as CPS,
        ):
            psc = [CPS.tile([128, 2 * OUT], f32, name=f"psc{m}", tag=f"psc{m}")
                   for m in range(8)]
            for q in range(KT // 2):
                aslab = CS.tile([128, ROWS], bf16, name="aslab", tag="aslab")
                nc.sync.dma_start(aslab[:],
                                  adjT_d[2 * q * 128:(2 * q + 1) * 128, :])
                aslab2 = CS.tile([128, ROWS], bf16, name="aslab2",
                                 tag="aslab2")
                nc.scalar.dma_start(
                    aslab2[:], adjT_d[(2 * q + 1) * 128:(2 * q + 2) * 128, :])
                ysp = CS.tile([128, 4 * OUT], bf16, name="ysp", tag="ysp")
                nc.sync.dma_start(
                    ysp[:].rearrange("p (two f) -> p two f", two=2),
                    ag2_out[:].rearrange("(a two p) f -> a p two f",
                                         two=2, p=128)[q])
                for t, asl in ((0, aslab), (1, aslab2)):
                    for m in range(8):
                        nc.tensor.matmul(
                            psc[m][:], asl[:, m * 128:(m + 1) * 128],
                            ysp[:, t * 2 * OUT:(t + 1) * 2 * OUT],
                            start=(2 * q + t == 0),
                            stop=(2 * q + t == KT - 1))
            for m in range(8):
                nc.vector.tensor_copy(s1_sb[m][:], psc[m][:, :OUT])
                s2a = DR.tile([128, OUT], bf16, name="s2a", tag="s2a")
                nc.vector.tensor_copy(s2a[:], psc[m][:, OUT:])
                nc.gpsimd.dma_start(ag3_in[m * 128:(m + 1) * 128, :], s2a[:])

        nc.gpsimd.collective_compute(
            "AllGather", AluOpType.bypass, replica_groups=rg,
            ins=[ag3_in[:].opt()], outs=[ag3_out[:].opt()])

        # ========== s1 transposes (under AllGather3) + D ====================
        with (
            tc.tile_pool(name="tps2", bufs=4, space="PSUM") as TPS2,
            tc.tile_pool(name="dslabs", bufs=2) as DS,
            tc.tile_pool(name="dps", bufs=1, space="PSUM") as DPS,
        ):
            for c in range(2):
                for m in range(8):
                    tp2 = TPS2.tile([128, 128], f32, name="tp2", tag="tp2")
                    nc.tensor.transpose(tp2[:],
                                        s1_sb[m][:, c * 128:(c + 1) * 128],
                                        ident_sb[:])
                    nc.vector.tensor_copy(h2T[2 + c][:, m * 128:(m + 1) * 128],
                                          tp2[:])
            # D: s2b.T = (adj_loc @ s2a_full).T; 4 k-slabs per iteration
            psd = [DPS.tile([128, H], f32, name=f"psd{i}", tag=f"psd{i}")
                   for i in range(4)]  # i = mo*2+n
            ag3_r = ag3_out[:].rearrange("(a two p) f -> a p two f",
                                         two=2, p=128)
            for q in range(KT // 4):
                slabs = []
                for t in range(4):
                    k = 4 * q + t
                    asl = DS.tile([128, ROWS], bf16, name=f"dsl{t}",
                                  tag=f"dsl{t}")
                    ring(t).dma_start(asl[:],
                                      adjT_d[k * 128:(k + 1) * 128, :])
                    slabs.append(asl)
                sp1 = DS.tile([128, 2 * OUT], bf16, name="sp1", tag="sp1")
                nc.sync.dma_start(
                    sp1[:].rearrange("p (two f) -> p two f", two=2),
                    ag3_r[2 * q])
                sp2 = DS.tile([128, 2 * OUT], bf16, name="sp2", tag="sp2")
                nc.scalar.dma_start(
                    sp2[:].rearrange("p (two f) -> p two f", two=2),
                    ag3_r[2 * q + 1])
                ss = [sp1[:, 0:OUT], sp1[:, OUT:2 * OUT],
                      sp2[:, 0:OUT], sp2[:, OUT:2 * OUT]]
                for t in range(4):
                    k = 4 * q + t
                    for mo in range(2):
                        for n in range(2):
                            nc.tensor.matmul(
                                psd[mo * 2 + n][:],
                                ss[t][:, mo * 128:(mo + 1) * 128],
                                slabs[t][:, n * H:(n + 1) * H],
                                start=(k == 0), stop=(k == KT - 1))
            for mo in range(2):
                for n in range(2):
                    nc.vector.tensor_copy(h2T[4 + mo][:, n * H:(n + 1) * H],
                                          psd[mo * 2 + n][:])

        # ========== final: out.T = (h2 @ Wf).T + bf ==========================
        with tc.tile_pool(name="fps", bufs=2, space="PSUM") as FPS:
            for mo in range(2):
                for n in range(2):
                    fps = FPS.tile([128, H], f32, name="fps", tag="fps")
                    for k in range(H2T):
                        nc.tensor.matmul(
                            fps[:], wf_sb[k][:, mo * 128:(mo + 1) * 128],
                            h2T[k][:, n * H:(n + 1) * H],
                            start=(k == 0), stop=(k == H2T - 1))
                    osb = DR.tile([128, H], f32, name="osb", tag="osb")
                    nc.vector.tensor_scalar_add(osb[:], fps[:],
                                                bfT_sb[:, mo:mo + 1])
                    nc.sync.dma_start(
                        outT_d[mo * 128:(mo + 1) * 128, n * H:(n + 1) * H],
                        osb[:])

    nc.compile()
    _BUILT["nc"] = nc
    return nc


def prep_in_maps(x, adj, W1, b1, W2, b2, gamma, beta, Wf, bf):
    """Host-side sharding / layout prep. Returns one input dict per core."""
    import ml_dtypes

    x = np.asarray(x, dtype=np.float32)
    adj = np.asarray(adj, dtype=np.float32)
    W1 = np.asarray(W1, dtype=np.float32)
    b1 = np.asarray(b1, dtype=np.float32)
    W2 = np.asarray(W2, dtype=np.float32)
    b2 = np.asarray(b2, dtype=np.float32)
    gamma = np.asarray(gamma, dtype=np.float32)
    beta = np.asarray(beta, dtype=np.float32)
    Wf = np.asarray(Wf, dtype=np.float32)
    bf = np.asarray(bf, dtype=np.float32)

    xT = np.ascontiguousarray(x.T)                       # [128, 8192]
    w1cat = np.ascontiguousarray(
        np.concatenate([W1[1], W1[2]], axis=1))          # [128, 1024]
    b1cat = np.concatenate([b1[1], b1[2]])               # [1024]
    b1bc = np.ascontiguousarray(
        np.broadcast_to(b1cat[None, :], (128, 2 * H)))
    w2cat = np.ascontiguousarray(
        np.concatenate([W2[1], W2[2]], axis=1))          # [1536, 512]
    b2cat = np.concatenate([b2[1], b2[2]])               # [512]
    b2bc = np.ascontiguousarray(
        np.broadcast_to(b2cat[None, :], (128, 2 * OUT)))
    gcol = np.ascontiguousarray(gamma.reshape(HT, 128).T)
    bcol = np.ascontiguousarray(beta.reshape(HT, 128).T)
    ident = np.eye(128, dtype=np.float32)

    shared = {
        "xT": xT,
        "w1cat": w1cat,
        "w1h0": np.ascontiguousarray(W1[0]),
        "b1bc": b1bc,
        "w2cat": w2cat,
        "w2h0": np.ascontiguousarray(W2[0]),
        "b2bc": b2bc,
        "b2h0T": np.ascontiguousarray(b2[0].reshape(2, 128).T),
        "wf": np.ascontiguousarray(Wf),
        "bfT": np.ascontiguousarray(bf.reshape(2, 128).T),
        "gcol": gcol,
        "bcol": bcol,
        "ident": ident,
    }
    in_maps = []
    for d in range(NC):
        r0, r1 = d * ROWS, (d + 1) * ROWS
        m = dict(shared)
        m["adjT"] = np.ascontiguousarray(
            adj[r0:r1].T.astype(ml_dtypes.bfloat16))     # [8192, 1024] bf16
        m["xTloc"] = np.ascontiguousarray(x[r0:r1].T)    # [128, 1024]
        in_maps.append(m)
    return in_maps


def run_on_hw(in_maps, trace=False):
    from concourse import bass_utils
    nc = build_program()
    return bass_utils.run_bass_kernel_spmd(
        nc, in_maps, core_ids=list(range(NC)), trace=trace)


def kernel(x, adj, W1, b1, W2, b2, gamma, beta, Wf, bf):
    in_maps = prep_in_maps(x, adj, W1, b1, W2, b2, gamma, beta, Wf, bf)
    res = run_on_hw(in_maps)
    out = np.concatenate(
        [np.ascontiguousarray(res.results[d]["outT"].T) for d in range(NC)],
        axis=0)
    return out.astype(np.float32)


# revision 16
# speedup vs baseline: 1.5669x; 1.1352x over previous
"""MixHop (2-hop) GNN forward on 8 TRN2 NeuronCores.

Sharding: adj and the output are row-sharded over N=8192 across 8 cores
(1024 rows each); x and all weights are replicated. Each propagation
adj_loc @ v is a local [1024,8192]@[8192,F] matmul; v is produced
row-sharded and AllGathered between hops.

Precision: propagation matmuls (adj-sided, the bulk of bytes+flops) run
in bf16 with fp32 PSUM accumulation; dense-layer transforms and BN run
in fp32r (full-rate reduced fp32). Measured end-to-end relative error
~2e-3.

Orientation notes:
- "natural"   = rows on partitions (needed for AllGather row-concat and
  as the K axis of the next propagation)
- "transposed" = features on partitions (needed as lhsT of the next
  dense layer; makes BatchNorm affine per-partition)
Pass B and D emit transposed outputs directly; pass A's t1 half and
pass C's s1 half are transposed on the PE with an identity matmul.
b1[0] (hop-0 bias of layer 1) is dropped: a per-column constant shift
is exactly cancelled by the training-mode BatchNorm that follows.

Scheduling notes:
- Every AllGather is split into two row-half chunks. Producer passes
  emit their first row-half, trigger chunk-a, and compute the second
  half under it; consumer passes contract chunk-a's rows while chunk-b
  is still gathering. To keep the contraction k-axis contiguous per
  chunk, the host permutes adj's columns (and x's rows) into
  "half-major" order: [r0 rows0:512 | r1 rows0:512 | ... | r0 rows
  512:1024 | ...]. Local row order (outputs) is unpermuted.
- Propagation k-loops process 4 k-slabs per iteration -> ~4.3us
  contiguous matmul bursts, which hold the PE HAM clock-gate at full
  rate (short bursts leave the PE throttled to 1.2 GHz).
- BN stats/AllReduce/normalize for h tiles 0..7 run under pass B; T2's
  first 8 k-tiles accumulate under AllReduce-b so only a short tail
  waits on it. y0.T runs under AllGather2b; s1 transposes run before
  AllGather3.
- DMA loads alternate between the two HWDGE rings (sync/scalar);
  SBUF->DRAM drains go via SWDGE (gpsimd).
"""
import sys
from contextlib import ExitStack

sys.path.insert(0, "/opt/trn_rl_repo")

import numpy as np

N, IN, H, OUT = 8192, 128, 512, 256
NC = 8
ROWS = N // NC          # 1024 rows per core
KT = N // 128           # 64 k-tiles of the propagation contraction
KH = KT // 2            # 32 k-tiles per gather chunk
HT = 3 * H // 128       # 12 feature tiles of h.T
H2T = 3 * OUT // 128    # 6 feature tiles of h2.T
EPS = 1e-5

_BUILT = {}


def build_program():
    """Build and compile the Bass program (cached)."""
    if "nc" in _BUILT:
        return _BUILT["nc"]

    import concourse.bacc as bacc
    import concourse.tile as tile
    import concourse.mybir as mybir
    from concourse.alu_op_type import AluOpType

    f32 = mybir.dt.float32
    f32r = mybir.dt.float32r
    bf16 = mybir.dt.bfloat16
    AF = mybir.ActivationFunctionType
    AX = mybir.AxisListType

    nc = bacc.Bacc("TRN2", target_bir_lowering=False, debug=False,
                   num_devices=NC)

    # ---- external inputs (per-core values supplied by the host) ----
    adjT_d = nc.dram_tensor("adjT", [N, ROWS], bf16, kind="ExternalInput")
    xT_d = nc.dram_tensor("xT", [IN, N], f32r, kind="ExternalInput")
    xTloc_d = nc.dram_tensor("xTloc", [IN, ROWS], f32r, kind="ExternalInput")
    w1cat_d = nc.dram_tensor("w1cat", [IN, 2 * H], f32r, kind="ExternalInput")
    w1h0_d = nc.dram_tensor("w1h0", [IN, H], f32r, kind="ExternalInput")
    b1bc_d = nc.dram_tensor("b1bc", [128, 2 * H], f32, kind="ExternalInput")
    w2cat_d = nc.dram_tensor("w2cat", [3 * H, 2 * OUT], f32r, kind="ExternalInput")
    w2h0_d = nc.dram_tensor("w2h0", [3 * H, OUT], f32r, kind="ExternalInput")
    b2bc_d = nc.dram_tensor("b2bc", [128, 2 * OUT], f32, kind="ExternalInput")
    b2h0T_d = nc.dram_tensor("b2h0T", [128, 2], f32, kind="ExternalInput")
    wf_d = nc.dram_tensor("wf", [3 * OUT, OUT], f32r, kind="ExternalInput")
    bfT_d = nc.dram_tensor("bfT", [128, 2], f32, kind="ExternalInput")
    gcol_d = nc.dram_tensor("gcol", [128, HT], f32, kind="ExternalInput")
    bcol_d = nc.dram_tensor("bcol", [128, HT], f32, kind="ExternalInput")
    ident_d = nc.dram_tensor("ident", [128, 128], f32, kind="ExternalInput")

    outT_d = nc.dram_tensor("outT", [OUT, ROWS], f32, kind="ExternalOutput")

    rg = [list(range(NC))]

    def ring(k):
        return nc.sync if k % 2 == 0 else nc.scalar

    with tile.TileContext(nc) as tc, ExitStack() as st:
        dram = st.enter_context(tc.tile_pool(name="dram", bufs=1, space="DRAM"))
        P = st.enter_context(tc.tile_pool(name="persist", bufs=1))
        DR = st.enter_context(tc.tile_pool(name="drain", bufs=2))

        # ---- DRAM intermediates ----
        v_dram = dram.tile([N, H], bf16, name="v_dram")   # x1 transform only
        agi = {}
        ago = {}
        for nm, rows, cols in (("ag1", ROWS // 2, H), ("ag2", ROWS // 2,
                                                       2 * OUT),
                               ("ag3", ROWS // 2, OUT)):
            for ch in "ab":
                agi[nm + ch] = dram.tile([rows, cols], bf16,
                                         name=f"{nm}{ch}_in")
                ago[nm + ch] = dram.tile([rows * NC, cols], bf16,
                                         name=f"{nm}{ch}_out",
                                         addr_space="Shared")
        ar_a_in = dram.tile([128, 16], f32, name="ar_a_in")
        ar_a_out = dram.tile([128, 16], f32, name="ar_a_out",
                             addr_space="Shared")
        ar_b_in = dram.tile([128, 8], f32, name="ar_b_in")
        ar_b_out = dram.tile([128, 8], f32, name="ar_b_out",
                             addr_space="Shared")

        def gather(nm, ch):
            nc.gpsimd.collective_compute(
                "AllGather", AluOpType.bypass, replica_groups=rg,
                ins=[agi[nm + ch][:].opt()], outs=[ago[nm + ch][:].opt()])

        def gsrc(nm, k):
            """k-slab [128, cols] of the gathered tensor, half-major order."""
            t = ago[nm + ("a" if k < KH else "b")]
            kk = k if k < KH else k - KH
            return t[kk * 128:(kk + 1) * 128, :]

        def gsrc_pair(nm, q):
            """Pair-slab AP [128, 2, cols] for k-slabs 2q, 2q+1."""
            t = ago[nm + ("a" if 2 * q < KH else "b")]
            qq = q if 2 * q < KH else q - KH // 2
            return t[:].rearrange("(a two p) f -> a p two f",
                                  two=2, p=128)[qq]

        # ---- small persistents (to the end) ----
        xTloc_sb = P.tile([IN, ROWS], f32r, name="xTloc_sb")
        nc.scalar.dma_start(xTloc_sb[:], xTloc_d[:, :])
        w1h0_sb = P.tile([IN, H], f32r, name="w1h0_sb")
        nc.scalar.dma_start(w1h0_sb[:], w1h0_d[:, :])
        b2h0T_sb = P.tile([128, 2], f32, name="b2h0T_sb")
        nc.scalar.dma_start(b2h0T_sb[:], b2h0T_d[:, :])
        bfT_sb = P.tile([128, 2], f32, name="bfT_sb")
        nc.scalar.dma_start(bfT_sb[:], bfT_d[:, :])
        gcol_sb = P.tile([128, HT], f32, name="gcol_sb")
        nc.scalar.dma_start(gcol_sb[:], gcol_d[:, :])
        bcol_sb = P.tile([128, HT], f32, name="bcol_sb")
        nc.scalar.dma_start(bcol_sb[:], bcol_d[:, :])
        ident_sb = P.tile([128, 128], f32, name="ident_sb")
        nc.scalar.dma_start(ident_sb[:], ident_d[:, :])
        wf_sb = [P.tile([128, OUT], f32r, name=f"wf{k}") for k in range(H2T)]
        for k in range(H2T):
            nc.scalar.dma_start(wf_sb[k][:], wf_d[k * 128:(k + 1) * 128, :])
        sumc = P.tile([128, HT], f32, name="sumc")
        sqc = P.tile([128, HT], f32, name="sqc")
        scale_c = P.tile([128, HT], f32, name="scale_c")
        shift_c = P.tile([128, HT], f32, name="shift_c")
        stat_a = P.tile([128, 16], f32, name="stat_a")
        stat_b = P.tile([128, 8], f32, name="stat_b")
        # h2.T (fp32r): tiles 0-1 y0.T, 2-3 s1.T, 4-5 s2b.T
        h2T = [P.tile([128, ROWS], f32r, name=f"h2T{t}") for t in range(H2T)]

        def bn_affine(stat, lo, hi):
            """Compute scale/shift columns [lo,hi) from gathered stats."""
            w = hi - lo
            mu = DR.tile([128, w], f32, name="mu", tag=f"mu{lo}")
            nc.vector.tensor_scalar_mul(mu[:], stat[:, :w], 1.0 / N)
            ex2 = DR.tile([128, w], f32, name="ex2", tag=f"ex2{lo}")
            nc.vector.tensor_scalar(ex2[:], stat[:, w:2 * w], 1.0 / N, EPS,
                                    AluOpType.mult, AluOpType.add)
            var = DR.tile([128, w], f32, name="var", tag=f"var{lo}")
            nc.vector.scalar_tensor_tensor(var[:], mu[:], -1.0, mu[:],
                                           AluOpType.mult, AluOpType.mult)
            nc.vector.tensor_add(var[:], var[:], ex2[:])
            std = DR.tile([128, w], f32, name="std", tag=f"std{lo}")
            nc.scalar.activation(std[:], var[:], AF.Sqrt)
            rstd = DR.tile([128, w], f32, name="rstd", tag=f"rstd{lo}")
            nc.vector.reciprocal(rstd[:], std[:])
            nc.vector.tensor_mul(scale_c[:, lo:hi], gcol_sb[:, lo:hi], rstd[:])
            nc.vector.scalar_tensor_tensor(shift_c[:, lo:hi], mu[:], -1.0,
                                           scale_c[:, lo:hi],
                                           AluOpType.mult, AluOpType.mult)
            nc.vector.tensor_add(shift_c[:, lo:hi], shift_c[:, lo:hi],
                                 bcol_sb[:, lo:hi])

        def stats_for(tiles, sq_pool, tag):
            for t in tiles:
                nc.vector.reduce_sum(sumc[:, t:t + 1], hT[t][:], axis=AX.X)
                sq = sq_pool.tile([128, ROWS], f32, name=tag, tag=tag)
                nc.vector.scalar_tensor_tensor(
                    sq[:], hT[t][:], 1.0, hT[t][:],
                    AluOpType.mult, AluOpType.mult,
                    accum_out=sqc[:, t:t + 1])

        # ============ T1 + pass A (t2a), pipelined ===========================
        with (
            tc.tile_pool(name="v2pool", bufs=1) as V2P,
            tc.tile_pool(name="xkpool", bufs=1) as XKP,
            tc.tile_pool(name="w1pool", bufs=1) as W1P,
        ):
            v2sb = [V2P.tile([128, H], bf16, name=f"v2s{k}")
                    for k in range(KT)]
            xk_sb = [XKP.tile([128, 128], f32r, name=f"xk{k}")
                     for k in range(KT)]
            w1cat_sb = W1P.tile([IN, 2 * H], f32r, name="w1cat_sb")
            nc.scalar.dma_start(w1cat_sb[:], w1cat_d[:, :])
            b1bc_sb = W1P.tile([128, 2 * H], f32, name="b1bc_sb")
            nc.scalar.dma_start(b1bc_sb[:], b1bc_d[:, :])

            # T1-x2: v2 = x @ W1[2] + b1[2]  (SBUF-resident, bf16)
            with tc.tile_pool(name="t1ps2", bufs=2, space="PSUM") as T1PS2:
                for k in range(KT):
                    nc.sync.dma_start(xk_sb[k][:],
                                      xT_d[:, k * 128:(k + 1) * 128])
                    vps2 = T1PS2.tile([128, H], f32, name="vps2", tag="vps2")
                    nc.tensor.matmul(vps2[:], xk_sb[k][:],
                                     w1cat_sb[:, H:2 * H],
                                     start=True, stop=True)
                    nc.vector.tensor_tensor(v2sb[k][:], vps2[:],
                                            b1bc_sb[:, H:2 * H],
                                            AluOpType.add)

            # A-t2a group 0 with T1-x1 interleaved into its dense PE stream
            with (
                tc.tile_pool(name="ag0slab", bufs=2) as AS,
                tc.tile_pool(name="ag0ps", bufs=1, space="PSUM") as APS,
                tc.tile_pool(name="t1ps1", bufs=2, space="PSUM") as T1PS1,
            ):
                acc = [APS.tile([128, H], f32, name=f"acc0{m}",
                                tag=f"acc0{m}") for m in range(4)]
                for q in range(KT // 4):
                    slabs = []
                    for t in range(4):
                        k = 4 * q + t
                        asl = AS.tile([128, H], bf16, name=f"asl{t}",
                                      tag=f"asl{t}")
                        ring(t).dma_start(asl[:],
                                          adjT_d[k * 128:(k + 1) * 128, 0:H])
                        slabs.append(asl)
                    for t in range(4):
                        k = 4 * q + t
                        for m in range(4):
                            nc.tensor.matmul(
                                acc[m][:], slabs[t][:, m * 128:(m + 1) * 128],
                                v2sb[k][:], start=(k == 0),
                                stop=(k == KT - 1))
                    # T1-x1 for 4 slabs, slotted into the same stream
                    for t in range(4):
                        k = 4 * q + t
                        vps1 = T1PS1.tile([128, H], f32, name="vps1",
                                          tag="vps1")
                        nc.tensor.matmul(vps1[:], xk_sb[k][:],
                                         w1cat_sb[:, 0:H],
                                         start=True, stop=True)
                        v1sb = DR.tile([128, H], bf16, name="v1sb",
                                       tag="v1sb")
                        nc.vector.tensor_tensor(v1sb[:], vps1[:],
                                                b1bc_sb[:, 0:H],
                                                AluOpType.add)
                        nc.gpsimd.dma_start(
                            v_dram[k * 128:(k + 1) * 128, :], v1sb[:])
                for m in range(4):
                    t2a = DR.tile([128, H], bf16, name="t2a", tag="t2a")
                    nc.vector.tensor_copy(t2a[:], acc[m][:])
                    nc.gpsimd.dma_start(
                        agi["ag1a"][m * 128:(m + 1) * 128, :], t2a[:])

            gather("ag1", "a")

            # A-t2a group 1
            with (
                tc.tile_pool(name="ag1slab", bufs=2) as AS1,
                tc.tile_pool(name="ag1ps", bufs=1, space="PSUM") as APS1,
            ):
                acc1 = [APS1.tile([128, H], f32, name=f"acc1{m}",
                                  tag=f"acc1{m}") for m in range(4)]
                for q in range(KT // 4):
                    slabs = []
                    for t in range(4):
                        k = 4 * q + t
                        asl = AS1.tile([128, H], bf16, name=f"bsl{t}",
                                       tag=f"bsl{t}")
                        ring(t).dma_start(asl[:],
                                          adjT_d[k * 128:(k + 1) * 128,
                                                 H:ROWS])
                        slabs.append(asl)
                    for t in range(4):
                        k = 4 * q + t
                        for m in range(4):
                            nc.tensor.matmul(
                                acc1[m][:], slabs[t][:, m * 128:(m + 1) * 128],
                                v2sb[k][:], start=(k == 0),
                                stop=(k == KT - 1))
                for m in range(4):
                    t2a = DR.tile([128, H], bf16, name="t2a", tag="t2a")
                    nc.vector.tensor_copy(t2a[:], acc1[m][:])
                    nc.gpsimd.dma_start(
                        agi["ag1b"][m * 128:(m + 1) * 128, :], t2a[:])

            gather("ag1", "b")

        # hT in fp32r: tiles 0-3 hop0.T, 4-7 t1.T, 8-11 t2b.T.
        PH = st.enter_context(tc.tile_pool(name="hpool", bufs=1))
        hT = [PH.tile([128, ROWS], f32r, name=f"hT{t}") for t in range(HT)]

        # ========== hop0.T + its stats (runs under AllGather1) ===============
        with (
            tc.tile_pool(name="h0ps", bufs=2, space="PSUM") as H0PS,
            tc.tile_pool(name="sqps0", bufs=1, space="PSUM") as SQPS0,
        ):
            for mo in range(4):
                for n in range(2):
                    h0ps = H0PS.tile([128, H], f32, name="h0ps", tag="h0ps")
                    nc.tensor.matmul(h0ps[:],
                                     w1h0_sb[:, mo * 128:(mo + 1) * 128],
                                     xTloc_sb[:, n * H:(n + 1) * H],
                                     start=True, stop=True)
                    nc.vector.tensor_copy(hT[mo][:, n * H:(n + 1) * H],
                                          h0ps[:])
            stats_for(range(4), SQPS0, "sq0")

        # t1 natural (fp32), transposed as soon as each row-group lands
        PT1 = st.enter_context(tc.tile_pool(name="t1nat", bufs=1))
        t1_sb = [PT1.tile([128, H], f32, name=f"t1n{m}") for m in range(8)]

        # ========= A-t1 row-groups + transposes ==============================
        for g in range(2):
            with (
                tc.tile_pool(name=f"a2slab{g}", bufs=2) as AS2,
                tc.tile_pool(name=f"aps2{g}", bufs=1, space="PSUM") as APS2,
            ):
                acc2 = [APS2.tile([128, H], f32, name=f"ac2{g}{m}",
                                  tag=f"ac2{g}{m}") for m in range(4)]
                for q in range(KT // 4):
                    slabs = []
                    for t in range(4):
                        k = 4 * q + t
                        asl = AS2.tile([128, H], bf16, name=f"a2s{t}",
                                       tag=f"a2s{t}")
                        ring(t).dma_start(
                            asl[:], adjT_d[k * 128:(k + 1) * 128,
                                           g * H:(g + 1) * H])
                        slabs.append(asl)
                    v1p = AS2.tile([128, 2 * H], bf16, name="v1p", tag="v1p")
                    nc.scalar.dma_start(
                        v1p[:].rearrange("p (two f) -> p two f", two=2),
                        v_dram[:].rearrange("(a two p) f -> a p two f",
                                            two=2, p=128)[2 * q])
                    v1p2 = AS2.tile([128, 2 * H], bf16, name="v1p2",
                                    tag="v1p2")
                    nc.sync.dma_start(
                        v1p2[:].rearrange("p (two f) -> p two f", two=2),
                        v_dram[:].rearrange("(a two p) f -> a p two f",
                                            two=2, p=128)[2 * q + 1])
                    vv = [v1p[:, 0:H], v1p[:, H:2 * H],
                          v1p2[:, 0:H], v1p2[:, H:2 * H]]
                    for t in range(4):
                        k = 4 * q + t
                        for m in range(4):
                            nc.tensor.matmul(
                                acc2[m][:],
                                slabs[t][:, m * 128:(m + 1) * 128],
                                vv[t], start=(k == 0), stop=(k == KT - 1))
                for m in range(4):
                    nc.vector.tensor_copy(t1_sb[4 * g + m][:], acc2[m][:])
            with tc.tile_pool(name=f"tps{g}", bufs=4, space="PSUM") as TPS:
                for c in range(4):
                    for m in range(4 * g, 4 * g + 4):
                        tp = TPS.tile([128, 128], f32, name="tp", tag="tp")
                        nc.tensor.transpose(
                            tp[:], t1_sb[m][:, c * 128:(c + 1) * 128],
                            ident_sb[:])
                        nc.vector.tensor_copy(
                            hT[4 + c][:, m * 128:(m + 1) * 128], tp[:])

        # ========= t1 stats + AllReduce-a + normalize tiles 0..7 =============
        with tc.tile_pool(name="sqps1", bufs=1, space="PSUM") as SQPS1:
            stats_for(range(4, 8), SQPS1, "sq1")
        nc.gpsimd.dma_start(ar_a_in[:, 0:8], sumc[:, 0:8])
        nc.gpsimd.dma_start(ar_a_in[:, 8:16], sqc[:, 0:8])
        nc.gpsimd.collective_compute(
            "AllReduce", AluOpType.add, replica_groups=rg,
            ins=[ar_a_in[:].opt()], outs=[ar_a_out[:].opt()])
        nc.sync.dma_start(stat_a[:], ar_a_out[:, :])
        bn_affine(stat_a, 0, 8)
        for t in range(8):
            nc.scalar.activation(hT[t][:], hT[t][:], AF.Relu,
                                 bias=shift_c[:, t:t + 1],
                                 scale=scale_c[:, t:t + 1])

        # layer-2 weights, loaded during pass B
        PW2 = st.enter_context(tc.tile_pool(name="w2pool", bufs=1))
        w2cat_sb = [PW2.tile([128, 2 * OUT], f32r, name=f"w2cat{k}")
                    for k in range(HT)]
        for k in range(HT):
            nc.scalar.dma_start(w2cat_sb[k][:],
                                w2cat_d[k * 128:(k + 1) * 128, :])
        w2h0_sb = [PW2.tile([128, OUT], f32r, name=f"w2h0{k}")
                   for k in range(HT)]
        for k in range(HT):
            nc.scalar.dma_start(w2h0_sb[k][:],
                                w2h0_d[k * 128:(k + 1) * 128, :])
        b2bc_sb = PW2.tile([128, 2 * OUT], f32, name="b2bc_sb")
        nc.scalar.dma_start(b2bc_sb[:], b2bc_d[:, :])

        # ================= B: t2b.T = (adj_loc @ t2a_full).T =================
        with (
            tc.tile_pool(name="bslabs", bufs=2) as BS,
            tc.tile_pool(name="bps", bufs=1, space="PSUM") as BPS,
        ):
            psb = [BPS.tile([128, H], f32, name=f"psb{i}", tag=f"psb{i}")
                   for i in range(8)]  # i = mo*2+n
            for q in range(KT // 2):
                aslab = BS.tile([128, ROWS], bf16, name="aslab", tag="aslab")
                nc.sync.dma_start(aslab[:],
                                  adjT_d[2 * q * 128:(2 * q + 1) * 128, :])
                aslab2 = BS.tile([128, ROWS], bf16, name="aslab2",
                                 tag="aslab2")
                nc.scalar.dma_start(
                    aslab2[:], adjT_d[(2 * q + 1) * 128:(2 * q + 2) * 128, :])
                tsp = BS.tile([128, 2 * H], bf16, name="tsp", tag="tsp")
                nc.scalar.dma_start(
                    tsp[:].rearrange("p (two f) -> p two f", two=2),
                    gsrc_pair("ag1", q))
                for t, asl in ((0, aslab), (1, aslab2)):
                    k = 2 * q + t
                    for mo in range(4):
                        for n in range(2):
                            nc.tensor.matmul(
                                psb[mo * 2 + n][:],
                                tsp[:, t * H + mo * 128:
                                    t * H + (mo + 1) * 128],
                                asl[:, n * H:(n + 1) * H],
                                start=(k == 0), stop=(k == KT - 1))
            for mo in range(4):
                for n in range(2):
                    nc.vector.tensor_copy(hT[8 + mo][:, n * H:(n + 1) * H],
                                          psb[mo * 2 + n][:])

        # ========== stats for t2b + AllReduce-b ==============================
        with tc.tile_pool(name="sqps2", bufs=1, space="PSUM") as SQPS2:
            stats_for(range(8, HT), SQPS2, "sq2")
        nc.gpsimd.dma_start(ar_b_in[:, 0:4], sumc[:, 8:12])
        nc.gpsimd.dma_start(ar_b_in[:, 4:8], sqc[:, 8:12])
        nc.gpsimd.collective_compute(
            "AllReduce", AluOpType.add, replica_groups=rg,
            ins=[ar_b_in[:].opt()], outs=[ar_b_out[:].opt()])

        # ========== T2 phase 1 (k=0..7) under AllReduce-b ====================
        with tc.tile_pool(name="yps", bufs=1, space="PSUM") as YPS:
            ypss = [YPS.tile([128, 2 * OUT], f32, name=f"yps{m}",
                             tag=f"yps{m}") for m in range(8)]
            for m in range(8):
                for k in range(8):
                    nc.tensor.matmul(ypss[m][:],
                                     hT[k][:, m * 128:(m + 1) * 128],
                                     w2cat_sb[k][:],
                                     start=(k == 0), stop=False)

            # AllReduce-b lands: finish BN for t2b tiles
            nc.sync.dma_start(stat_b[:], ar_b_out[:, :])
            bn_affine(stat_b, 8, HT)
            for t in range(8, HT):
                nc.scalar.activation(hT[t][:], hT[t][:], AF.Relu,
                                     bias=shift_c[:, t:t + 1],
                                     scale=scale_c[:, t:t + 1])

            # ========== T2 phase 2 (k=8..11) + drains, chunked gathers =======
            for half in range(2):
                for m in range(4 * half, 4 * half + 4):
                    for k in range(8, HT):
                        nc.tensor.matmul(ypss[m][:],
                                         hT[k][:, m * 128:(m + 1) * 128],
                                         w2cat_sb[k][:],
                                         start=False, stop=(k == HT - 1))
                    ysb = DR.tile([128, 2 * OUT], bf16, name="ysb", tag="ysb")
                    nc.vector.tensor_tensor(ysb[:], ypss[m][:], b2bc_sb[:],
                                            AluOpType.add)
                    nc.gpsimd.dma_start(
                        agi["ag2" + "ab"[half]][
                            (m - 4 * half) * 128:(m - 4 * half + 1) * 128, :],
                        ysb[:])
                gather("ag2", "ab"[half])

        # y0.T = (hn @ W2[0]).T + b2[0]  (runs under AllGather2)
        with tc.tile_pool(name="y0ps", bufs=2, space="PSUM") as Y0PS:
            for mo in range(2):
                for n in range(2):
                    y0ps = Y0PS.tile([128, H], f32, name="y0ps", tag="y0ps")
                    for k in range(HT):
                        nc.tensor.matmul(
                            y0ps[:], w2h0_sb[k][:, mo * 128:(mo + 1) * 128],
                            hT[k][:, n * H:(n + 1) * H],
                            start=(k == 0), stop=(k == HT - 1))
                    nc.vector.tensor_scalar_add(h2T[mo][:, n * H:(n + 1) * H],
                                                y0ps[:],
                                                b2h0T_sb[:, mo:mo + 1])

        # s1 natural (fp32), transposed before AllGather3 completes
        PS1 = st.enter_context(tc.tile_pool(name="s1nat", bufs=1))
        s1_sb = [PS1.tile([128, OUT], f32, name=f"s1n{m}") for m in range(8)]

        # ========== C: [s1|s2a] = adj_loc @ [y1|y2] (natural) ================
        with (
            tc.tile_pool(name="cslabs", bufs=2) as CS,
            tc.tile_pool(name="cps", bufs=1, space="PSUM") as CPS,
        ):
            psc = [CPS.tile([128, 2 * OUT], f32, name=f"psc{m}", tag=f"psc{m}")
                   for m in range(8)]
            for q in range(KT // 2):
                aslab = CS.tile([128, ROWS], bf16, name="aslab", tag="aslab")
                nc.sync.dma_start(aslab[:],
                                  adjT_d[2 * q * 128:(2 * q + 1) * 128, :])
                aslab2 = CS.tile([128, ROWS], bf16, name="aslab2",
                                 tag="aslab2")
                nc.scalar.dma_start(
                    aslab2[:], adjT_d[(2 * q + 1) * 128:(2 * q + 2) * 128, :])
                ysp = CS.tile([128, 4 * OUT], bf16, name="ysp", tag="ysp")
                nc.sync.dma_start(
                    ysp[:].rearrange("p (two f) -> p two f", two=2),
                    gsrc_pair("ag2", q))
                for t, asl in ((0, aslab), (1, aslab2)):
                    for m in range(8):
                        nc.tensor.matmul(
                            psc[m][:], asl[:, m * 128:(m + 1) * 128],
                            ysp[:, t * 2 * OUT:(t + 1) * 2 * OUT],
                            start=(2 * q + t == 0),
                            stop=(2 * q + t == KT - 1))
            # drains: s1 + s2a, chunked AllGather3, s1 transposes interleaved
            for half in range(2):
                for m in range(4 * half, 4 * half + 4):
                    nc.vector.tensor_copy(s1_sb[m][:], psc[m][:, :OUT])
                    s2a = DR.tile([128, OUT], bf16, name="s2a", tag="s2a")
                    nc.vector.tensor_copy(s2a[:], psc[m][:, OUT:])
                    nc.gpsimd.dma_start(
                        agi["ag3" + "ab"[half]][
                            (m - 4 * half) * 128:(m - 4 * half + 1) * 128, :],
                        s2a[:])
                gather("ag3", "ab"[half])

        # ========== s1 transposes (under AllGather3) + D ====================
        with (
            tc.tile_pool(name="tps2", bufs=4, space="PSUM") as TPS2,
            tc.tile_pool(name="dslabs", bufs=2) as DS,
            tc.tile_pool(name="dps", bufs=1, space="PSUM") as DPS,
        ):
            for c in range(2):
                for m in range(8):
                    tp2 = TPS2.tile([128, 128], f32, name="tp2", tag="tp2")
                    nc.tensor.transpose(tp2[:],
                                        s1_sb[m][:, c * 128:(c + 1) * 128],
                                        ident_sb[:])
                    nc.vector.tensor_copy(h2T[2 + c][:, m * 128:(m + 1) * 128],
                                          tp2[:])
            # D: s2b.T = (adj_loc @ s2a_full).T; 4 k-slabs per iteration
            psd = [DPS.tile([128, H], f32, name=f"psd{i}", tag=f"psd{i}")
                   for i in range(4)]  # i = mo*2+n
            for q in range(KT // 4):
                slabs = []
                for t in range(4):
                    k = 4 * q + t
                    asl = DS.tile([128, ROWS], bf16, name=f"dsl{t}",
                                  tag=f"dsl{t}")
                    ring(t).dma_start(asl[:],
                                      adjT_d[k * 128:(k + 1) * 128, :])
                    slabs.append(asl)
                sp1 = DS.tile([128, 2 * OUT], bf16, name="sp1", tag="sp1")
                nc.sync.dma_start(
                    sp1[:].rearrange("p (two f) -> p two f", two=2),
                    gsrc_pair("ag3", 2 * q))
                sp2 = DS.tile([128, 2 * OUT], bf16, name="sp2", tag="sp2")
                nc.scalar.dma_start(
                    sp2[:].rearrange("p (two f) -> p two f", two=2),
                    gsrc_pair("ag3", 2 * q + 1))
                ss = [sp1[:, 0:OUT], sp1[:, OUT:2 * OUT],
                      sp2[:, 0:OUT], sp2[:, OUT:2 * OUT]]
                for t in range(4):
                    k = 4 * q + t
                    for mo in range(2):
                        for n in range(2):
                            nc.tensor.matmul(
                                psd[mo * 2 + n][:],
                                ss[t][:, mo * 128:(mo + 1) * 128],
                                slabs[t][:, n * H:(n + 1) * H],
                                start=(k == 0), stop=(k == KT - 1))
            for mo in range(2):
                for n in range(2):
                    nc.vector.tensor_copy(h2T[4 + mo][:, n * H:(n + 1) * H],
                                          psd[mo * 2 + n][:])

        # ========== final: out.T = (h2 @ Wf).T + bf ==========================
        with tc.tile_pool(name="fps", bufs=2, space="PSUM") as FPS:
            for mo in range(2):
                for n in range(2):
                    fps = FPS.tile([128, H], f32, name="fps", tag="fps")
                    for k in range(H2T):
                        nc.tensor.matmul(
                            fps[:], wf_sb[k][:, mo * 128:(mo + 1) * 128],
                            h2T[k][:, n * H:(n + 1) * H],
                            start=(k == 0), stop=(k == H2T - 1))
                    osb = DR.tile([128, H], f32, name="osb", tag="osb")
                    nc.vector.tensor_scalar_add(osb[:], fps[:],
                                                bfT_sb[:, mo:mo + 1])
                    nc.sync.dma_start(
                        outT_d[mo * 128:(mo + 1) * 128, n * H:(n + 1) * H],
                        osb[:])

    nc.compile()
    _BUILT["nc"] = nc
    return nc


def _half_major_perm():
    """Slab permutation: k' -> global 128-row slab index, half-major order:
    [r0 rows0:512 | r1 rows0:512 | ... | r7 rows0:512 | r0 rows512:1024...]"""
    perm = []
    for g in range(2):
        for r in range(NC):
            for j in range(4):
                perm.append(r * 8 + g * 4 + j)
    return perm


def prep_in_maps(x, adj, W1, b1, W2, b2, gamma, beta, Wf, bf):
    """Host-side sharding / layout prep. Returns one input dict per core."""
    import ml_dtypes

    x = np.asarray(x, dtype=np.float32)
    adj = np.asarray(adj, dtype=np.float32)
    W1 = np.asarray(W1, dtype=np.float32)
    b1 = np.asarray(b1, dtype=np.float32)
    W2 = np.asarray(W2, dtype=np.float32)
    b2 = np.asarray(b2, dtype=np.float32)
    gamma = np.asarray(gamma, dtype=np.float32)
    beta = np.asarray(beta, dtype=np.float32)
    Wf = np.asarray(Wf, dtype=np.float32)
    bf = np.asarray(bf, dtype=np.float32)

    perm = _half_major_perm()
    row_perm = np.concatenate(
        [np.arange(s * 128, (s + 1) * 128) for s in perm])

    xTp = np.ascontiguousarray(x.T[:, row_perm])         # [128, 8192]
    w1cat = np.ascontiguousarray(
        np.concatenate([W1[1], W1[2]], axis=1))          # [128, 1024]
    b1cat = np.concatenate([b1[1], b1[2]])               # [1024]
    b1bc = np.ascontiguousarray(
        np.broadcast_to(b1cat[None, :], (128, 2 * H)))
    w2cat = np.ascontiguousarray(
        np.concatenate([W2[1], W2[2]], axis=1))          # [1536, 512]
    b2cat = np.concatenate([b2[1], b2[2]])               # [512]
    b2bc = np.ascontiguousarray(
        np.broadcast_to(b2cat[None, :], (128, 2 * OUT)))
    gcol = np.ascontiguousarray(gamma.reshape(HT, 128).T)
    bcol = np.ascontiguousarray(beta.reshape(HT, 128).T)
    ident = np.eye(128, dtype=np.float32)

    shared = {
        "xT": xTp,
        "w1cat": w1cat,
        "w1h0": np.ascontiguousarray(W1[0]),
        "b1bc": b1bc,
        "w2cat": w2cat,
        "w2h0": np.ascontiguousarray(W2[0]),
        "b2bc": b2bc,
        "b2h0T": np.ascontiguousarray(b2[0].reshape(2, 128).T),
        "wf": np.ascontiguousarray(Wf),
        "bfT": np.ascontiguousarray(bf.reshape(2, 128).T),
        "gcol": gcol,
        "bcol": bcol,
        "ident": ident,
    }
    in_maps = []
    for d in range(NC):
        r0, r1 = d * ROWS, (d + 1) * ROWS
        m = dict(shared)
        adjT = adj[r0:r1].T[row_perm]                    # [8192, 1024]
        m["adjT"] = np.ascontiguousarray(adjT.astype(ml_dtypes.bfloat16))
        m["xTloc"] = np.ascontiguousarray(x[r0:r1].T)    # [128, 1024]
        in_maps.append(m)
    return in_maps


def run_on_hw(in_maps, trace=False):
    from concourse import bass_utils
    nc = build_program()
    return bass_utils.run_bass_kernel_spmd(
        nc, in_maps, core_ids=list(range(NC)), trace=trace)


def kernel(x, adj, W1, b1, W2, b2, gamma, beta, Wf, bf):
    in_maps = prep_in_maps(x, adj, W1, b1, W2, b2, gamma, beta, Wf, bf)
    res = run_on_hw(in_maps)
    out = np.concatenate(
        [np.ascontiguousarray(res.results[d]["outT"].T) for d in range(NC)],
        axis=0)
    return out.astype(np.float32)


# revision 20
# speedup vs baseline: 1.5721x; 1.0033x over previous
"""MixHop (2-hop) GNN forward on 8 TRN2 NeuronCores.

Sharding: adj and the output are row-sharded over N=8192 across 8 cores
(1024 rows each); x and all weights are replicated. Each propagation
adj_loc @ v is a local [1024,8192]@[8192,F] matmul; v is produced
row-sharded and AllGathered between hops.

Precision: propagation matmuls (adj-sided, the bulk of bytes+flops) run
in bf16 with fp32 PSUM accumulation; dense-layer transforms and BN run
in fp32r (full-rate reduced fp32). Measured end-to-end relative error
~2e-3.

Orientation notes:
- "natural"   = rows on partitions (needed for AllGather row-concat and
  as the K axis of the next propagation)
- "transposed" = features on partitions (needed as lhsT of the next
  dense layer; makes BatchNorm affine per-partition)
Pass B and D emit transposed outputs directly; pass A's t1 half and
pass C's s1 half are transposed on the PE with an identity matmul.
b1[0] (hop-0 bias of layer 1) is dropped: a per-column constant shift
is exactly cancelled by the training-mode BatchNorm that follows.

Scheduling notes:
- Every AllGather is split into two row-half chunks. Producer passes
  emit their first row-half, trigger chunk-a, and compute the second
  half under it; consumer passes contract chunk-a's rows while chunk-b
  is still gathering. To keep the contraction k-axis contiguous per
  chunk, the host permutes adj's columns (and x's rows) into
  "half-major" order: [r0 rows0:512 | r1 rows0:512 | ... | r0 rows
  512:1024 | ...]. Local row order (outputs) is unpermuted.
- Propagation k-loops process 4 k-slabs per iteration -> ~4.3us
  contiguous matmul bursts, which hold the PE HAM clock-gate at full
  rate (short bursts leave the PE throttled to 1.2 GHz).
- BN stats/AllReduce/normalize for h tiles 0..7 run under pass B; T2's
  first 8 k-tiles accumulate under AllReduce-b so only a short tail
  waits on it. y0.T runs under AllGather2b; s1 transposes run before
  AllGather3.
- DMA loads alternate between the two HWDGE rings (sync/scalar);
  SBUF->DRAM drains go via SWDGE (gpsimd).
"""
import sys
from contextlib import ExitStack

sys.path.insert(0, "/opt/trn_rl_repo")

import numpy as np

N, IN, H, OUT = 8192, 128, 512, 256
NC = 8
ROWS = N // NC          # 1024 rows per core
KT = N // 128           # 64 k-tiles of the propagation contraction
KH = KT // 2            # 32 k-tiles per gather chunk
HT = 3 * H // 128       # 12 feature tiles of h.T
H2T = 3 * OUT // 128    # 6 feature tiles of h2.T
EPS = 1e-5

_BUILT = {}


def build_program():
    """Build and compile the Bass program (cached)."""
    if "nc" in _BUILT:
        return _BUILT["nc"]

    import concourse.bacc as bacc
    import concourse.tile as tile
    import concourse.mybir as mybir
    from concourse.alu_op_type import AluOpType

    f32 = mybir.dt.float32
    f32r = mybir.dt.float32r
    bf16 = mybir.dt.bfloat16
    AF = mybir.ActivationFunctionType
    AX = mybir.AxisListType

    nc = bacc.Bacc("TRN2", target_bir_lowering=False, debug=False,
                   num_devices=NC)

    # ---- external inputs (per-core values supplied by the host) ----
    adjT_d = nc.dram_tensor("adjT", [N, ROWS], bf16, kind="ExternalInput")
    xT_d = nc.dram_tensor("xT", [IN, N], f32r, kind="ExternalInput")
    xTloc_d = nc.dram_tensor("xTloc", [IN, ROWS], f32r, kind="ExternalInput")
    w1cat_d = nc.dram_tensor("w1cat", [IN, 2 * H], f32r, kind="ExternalInput")
    w1h0_d = nc.dram_tensor("w1h0", [IN, H], f32r, kind="ExternalInput")
    b1bc_d = nc.dram_tensor("b1bc", [128, 2 * H], f32, kind="ExternalInput")
    w2cat_d = nc.dram_tensor("w2cat", [3 * H, 2 * OUT], f32r, kind="ExternalInput")
    w2h0_d = nc.dram_tensor("w2h0", [3 * H, OUT], f32r, kind="ExternalInput")
    b2bc_d = nc.dram_tensor("b2bc", [128, 2 * OUT], f32, kind="ExternalInput")
    b2h0T_d = nc.dram_tensor("b2h0T", [128, 2], f32, kind="ExternalInput")
    wf_d = nc.dram_tensor("wf", [3 * OUT, OUT], f32r, kind="ExternalInput")
    bfT_d = nc.dram_tensor("bfT", [128, 2], f32, kind="ExternalInput")
    gcol_d = nc.dram_tensor("gcol", [128, HT], f32, kind="ExternalInput")
    bcol_d = nc.dram_tensor("bcol", [128, HT], f32, kind="ExternalInput")
    ident_d = nc.dram_tensor("ident", [128, 128], f32, kind="ExternalInput")

    outT_d = nc.dram_tensor("outT", [OUT, ROWS], f32, kind="ExternalOutput")

    rg = [list(range(NC))]

    def ring(k):
        return nc.sync if k % 2 == 0 else nc.scalar

    with tile.TileContext(nc) as tc, ExitStack() as st:
        dram = st.enter_context(tc.tile_pool(name="dram", bufs=1, space="DRAM"))
        P = st.enter_context(tc.tile_pool(name="persist", bufs=1))
        DR = st.enter_context(tc.tile_pool(name="drain", bufs=2))

        # ---- DRAM intermediates ----
        v_dram = dram.tile([N, H], bf16, name="v_dram")   # x1 transform only
        agi = {}
        ago = {}
        for nm, rows, cols in (("ag1", ROWS // 2, H), ("ag2", ROWS // 2,
                                                       2 * OUT),
                               ("ag3", ROWS // 2, OUT)):
            for ch in "ab":
                agi[nm + ch] = dram.tile([rows, cols], bf16,
                                         name=f"{nm}{ch}_in")
                ago[nm + ch] = dram.tile([rows * NC, cols], bf16,
                                         name=f"{nm}{ch}_out",
                                         addr_space="Shared")
        ar_a_in = dram.tile([128, 16], f32, name="ar_a_in")
        ar_a_out = dram.tile([128, 16], f32, name="ar_a_out",
                             addr_space="Shared")
        ar_b_in = dram.tile([128, 8], f32, name="ar_b_in")
        ar_b_out = dram.tile([128, 8], f32, name="ar_b_out",
                             addr_space="Shared")

        def gather(nm, ch):
            nc.gpsimd.collective_compute(
                "AllGather", AluOpType.bypass, replica_groups=rg,
                ins=[agi[nm + ch][:].opt()], outs=[ago[nm + ch][:].opt()])

        def gsrc(nm, k):
            """k-slab [128, cols] of the gathered tensor, half-major order."""
            t = ago[nm + ("a" if k < KH else "b")]
            kk = k if k < KH else k - KH
            return t[kk * 128:(kk + 1) * 128, :]

        def gsrc_pair(nm, q):
            """Pair-slab AP [128, 2, cols] for k-slabs 2q, 2q+1."""
            t = ago[nm + ("a" if 2 * q < KH else "b")]
            qq = q if 2 * q < KH else q - KH // 2
            return t[:].rearrange("(a two p) f -> a p two f",
                                  two=2, p=128)[qq]

        # ---- small persistents (to the end) ----
        xTloc_sb = P.tile([IN, ROWS], f32r, name="xTloc_sb")
        nc.scalar.dma_start(xTloc_sb[:], xTloc_d[:, :])
        w1h0_sb = P.tile([IN, H], f32r, name="w1h0_sb")
        nc.scalar.dma_start(w1h0_sb[:], w1h0_d[:, :])
        b2h0T_sb = P.tile([128, 2], f32, name="b2h0T_sb")
        nc.scalar.dma_start(b2h0T_sb[:], b2h0T_d[:, :])
        bfT_sb = P.tile([128, 2], f32, name="bfT_sb")
        nc.scalar.dma_start(bfT_sb[:], bfT_d[:, :])
        gcol_sb = P.tile([128, HT], f32, name="gcol_sb")
        nc.scalar.dma_start(gcol_sb[:], gcol_d[:, :])
        bcol_sb = P.tile([128, HT], f32, name="bcol_sb")
        nc.scalar.dma_start(bcol_sb[:], bcol_d[:, :])
        ident_sb = P.tile([128, 128], f32, name="ident_sb")
        nc.scalar.dma_start(ident_sb[:], ident_d[:, :])
        wf_sb = [P.tile([128, OUT], f32r, name=f"wf{k}") for k in range(H2T)]
        for k in range(H2T):
            nc.scalar.dma_start(wf_sb[k][:], wf_d[k * 128:(k + 1) * 128, :])
        sumc = P.tile([128, HT], f32, name="sumc")
        sqc = P.tile([128, HT], f32, name="sqc")
        scale_c = P.tile([128, HT], f32, name="scale_c")
        shift_c = P.tile([128, HT], f32, name="shift_c")
        stat_a = P.tile([128, 16], f32, name="stat_a")
        stat_b = P.tile([128, 8], f32, name="stat_b")
        # h2.T (fp32r): tiles 0-1 y0.T, 2-3 s1.T, 4-5 s2b.T
        h2T = [P.tile([128, ROWS], f32r, name=f"h2T{t}") for t in range(H2T)]

        def bn_affine(stat, lo, hi):
            """Compute scale/shift columns [lo,hi) from gathered stats."""
            w = hi - lo
            mu = DR.tile([128, w], f32, name="mu", tag=f"mu{lo}")
            nc.vector.tensor_scalar_mul(mu[:], stat[:, :w], 1.0 / N)
            ex2 = DR.tile([128, w], f32, name="ex2", tag=f"ex2{lo}")
            nc.vector.tensor_scalar(ex2[:], stat[:, w:2 * w], 1.0 / N, EPS,
                                    AluOpType.mult, AluOpType.add)
            var = DR.tile([128, w], f32, name="var", tag=f"var{lo}")
            nc.vector.scalar_tensor_tensor(var[:], mu[:], -1.0, mu[:],
                                           AluOpType.mult, AluOpType.mult)
            nc.vector.tensor_add(var[:], var[:], ex2[:])
            std = DR.tile([128, w], f32, name="std", tag=f"std{lo}")
            nc.scalar.activation(std[:], var[:], AF.Sqrt)
            rstd = DR.tile([128, w], f32, name="rstd", tag=f"rstd{lo}")
            nc.vector.reciprocal(rstd[:], std[:])
            nc.vector.tensor_mul(scale_c[:, lo:hi], gcol_sb[:, lo:hi], rstd[:])
            nc.vector.scalar_tensor_tensor(shift_c[:, lo:hi], mu[:], -1.0,
                                           scale_c[:, lo:hi],
                                           AluOpType.mult, AluOpType.mult)
            nc.vector.tensor_add(shift_c[:, lo:hi], shift_c[:, lo:hi],
                                 bcol_sb[:, lo:hi])

        def stats_for(tiles, sq_pool, tag):
            for t in tiles:
                nc.vector.reduce_sum(sumc[:, t:t + 1], hT[t][:], axis=AX.X)
                sq = sq_pool.tile([128, ROWS], f32, name=tag, tag=tag)
                nc.vector.scalar_tensor_tensor(
                    sq[:], hT[t][:], 1.0, hT[t][:],
                    AluOpType.mult, AluOpType.mult,
                    accum_out=sqc[:, t:t + 1])

        # ============ T1 + pass A (t2a), pipelined ===========================
        with (
            tc.tile_pool(name="v2pool", bufs=1) as V2P,
            tc.tile_pool(name="xkpool", bufs=1) as XKP,
            tc.tile_pool(name="w1pool", bufs=1) as W1P,
        ):
            v2sb = [V2P.tile([128, H], bf16, name=f"v2s{k}")
                    for k in range(KT)]
            xk_sb = [XKP.tile([128, 128], f32r, name=f"xk{k}")
                     for k in range(KT)]
            w1cat_sb = W1P.tile([IN, 2 * H], f32r, name="w1cat_sb")
            nc.scalar.dma_start(w1cat_sb[:], w1cat_d[:, :])
            b1bc_sb = W1P.tile([128, 2 * H], f32, name="b1bc_sb")
            nc.scalar.dma_start(b1bc_sb[:], b1bc_d[:, :])

            # T1-x2: v2 = x @ W1[2] + b1[2]  (SBUF-resident, bf16)
            with tc.tile_pool(name="t1ps2", bufs=3, space="PSUM") as T1PS2:
                for k in range(KT):
                    nc.gpsimd.dma_start(xk_sb[k][:],
                                        xT_d[:, k * 128:(k + 1) * 128])
                    vps2 = T1PS2.tile([128, H], f32, name="vps2", tag="vps2")
                    nc.tensor.matmul(vps2[:], xk_sb[k][:],
                                     w1cat_sb[:, H:2 * H],
                                     start=True, stop=True)
                    nc.vector.tensor_tensor(v2sb[k][:], vps2[:],
                                            b1bc_sb[:, H:2 * H],
                                            AluOpType.add)

            # A-t2a group 0 with T1-x1 interleaved into its dense PE stream
            with (
                tc.tile_pool(name="ag0slab", bufs=2) as AS,
                tc.tile_pool(name="ag0ps", bufs=1, space="PSUM") as APS,
                tc.tile_pool(name="t1ps1", bufs=2, space="PSUM") as T1PS1,
            ):
                acc = [APS.tile([128, H], f32, name=f"acc0{m}",
                                tag=f"acc0{m}") for m in range(4)]
                for q in range(KT // 4):
                    slabs = []
                    for t in range(4):
                        k = 4 * q + t
                        asl = AS.tile([128, H], bf16, name=f"asl{t}",
                                      tag=f"asl{t}")
                        ring(t).dma_start(asl[:],
                                          adjT_d[k * 128:(k + 1) * 128, 0:H])
                        slabs.append(asl)
                    for t in range(4):
                        k = 4 * q + t
                        for m in range(4):
                            nc.tensor.matmul(
                                acc[m][:], slabs[t][:, m * 128:(m + 1) * 128],
                                v2sb[k][:], start=(k == 0),
                                stop=(k == KT - 1))
                    # T1-x1 for 4 slabs, slotted into the same stream
                    for t in range(4):
                        k = 4 * q + t
                        vps1 = T1PS1.tile([128, H], f32, name="vps1",
                                          tag="vps1")
                        nc.tensor.matmul(vps1[:], xk_sb[k][:],
                                         w1cat_sb[:, 0:H],
                                         start=True, stop=True)
                        v1sb = DR.tile([128, H], bf16, name="v1sb",
                                       tag="v1sb")
                        nc.vector.tensor_tensor(v1sb[:], vps1[:],
                                                b1bc_sb[:, 0:H],
                                                AluOpType.add)
                        nc.gpsimd.dma_start(
                            v_dram[k * 128:(k + 1) * 128, :], v1sb[:])
                for m in range(4):
                    t2a = DR.tile([128, H], bf16, name="t2a", tag="t2a")
                    nc.vector.tensor_copy(t2a[:], acc[m][:])
                    nc.gpsimd.dma_start(
                        agi["ag1a"][m * 128:(m + 1) * 128, :], t2a[:])

            gather("ag1", "a")

            # A-t2a group 1
            with (
                tc.tile_pool(name="ag1slab", bufs=2) as AS1,
                tc.tile_pool(name="ag1ps", bufs=1, space="PSUM") as APS1,
            ):
                acc1 = [APS1.tile([128, H], f32, name=f"acc1{m}",
                                  tag=f"acc1{m}") for m in range(4)]
                for q in range(KT // 4):
                    slabs = []
                    for t in range(4):
                        k = 4 * q + t
                        asl = AS1.tile([128, H], bf16, name=f"bsl{t}",
                                       tag=f"bsl{t}")
                        ring(t).dma_start(asl[:],
                                          adjT_d[k * 128:(k + 1) * 128,
                                                 H:ROWS])
                        slabs.append(asl)
                    for t in range(4):
                        k = 4 * q + t
                        for m in range(4):
                            nc.tensor.matmul(
                                acc1[m][:], slabs[t][:, m * 128:(m + 1) * 128],
                                v2sb[k][:], start=(k == 0),
                                stop=(k == KT - 1))
                for m in range(4):
                    t2a = DR.tile([128, H], bf16, name="t2a", tag="t2a")
                    nc.vector.tensor_copy(t2a[:], acc1[m][:])
                    nc.gpsimd.dma_start(
                        agi["ag1b"][m * 128:(m + 1) * 128, :], t2a[:])

            gather("ag1", "b")

        # hT in fp32r: tiles 0-3 hop0.T, 4-7 t1.T, 8-11 t2b.T.
        PH = st.enter_context(tc.tile_pool(name="hpool", bufs=1))
        hT = [PH.tile([128, ROWS], f32r, name=f"hT{t}") for t in range(HT)]

        # ========== hop0.T + its stats (runs under AllGather1) ===============
        with (
            tc.tile_pool(name="h0ps", bufs=2, space="PSUM") as H0PS,
            tc.tile_pool(name="sqps0", bufs=1, space="PSUM") as SQPS0,
        ):
            for mo in range(4):
                for n in range(2):
                    h0ps = H0PS.tile([128, H], f32, name="h0ps", tag="h0ps")
                    nc.tensor.matmul(h0ps[:],
                                     w1h0_sb[:, mo * 128:(mo + 1) * 128],
                                     xTloc_sb[:, n * H:(n + 1) * H],
                                     start=True, stop=True)
                    nc.vector.tensor_copy(hT[mo][:, n * H:(n + 1) * H],
                                          h0ps[:])
            stats_for(range(4), SQPS0, "sq0")

        # t1 natural (fp32), transposed as soon as each row-group lands
        PT1 = st.enter_context(tc.tile_pool(name="t1nat", bufs=1))
        t1_sb = [PT1.tile([128, H], f32, name=f"t1n{m}") for m in range(8)]

        # ========= A-t1 row-groups + transposes ==============================
        for g in range(2):
            with (
                tc.tile_pool(name=f"a2slab{g}", bufs=2) as AS2,
                tc.tile_pool(name=f"aps2{g}", bufs=1, space="PSUM") as APS2,
            ):
                acc2 = [APS2.tile([128, H], f32, name=f"ac2{g}{m}",
                                  tag=f"ac2{g}{m}") for m in range(4)]
                for q in range(KT // 4):
                    slabs = []
                    for t in range(4):
                        k = 4 * q + t
                        asl = AS2.tile([128, H], bf16, name=f"a2s{t}",
                                       tag=f"a2s{t}")
                        ring(t).dma_start(
                            asl[:], adjT_d[k * 128:(k + 1) * 128,
                                           g * H:(g + 1) * H])
                        slabs.append(asl)
                    v1p = AS2.tile([128, 2 * H], bf16, name="v1p", tag="v1p")
                    nc.scalar.dma_start(
                        v1p[:].rearrange("p (two f) -> p two f", two=2),
                        v_dram[:].rearrange("(a two p) f -> a p two f",
                                            two=2, p=128)[2 * q])
                    v1p2 = AS2.tile([128, 2 * H], bf16, name="v1p2",
                                    tag="v1p2")
                    nc.sync.dma_start(
                        v1p2[:].rearrange("p (two f) -> p two f", two=2),
                        v_dram[:].rearrange("(a two p) f -> a p two f",
                                            two=2, p=128)[2 * q + 1])
                    vv = [v1p[:, 0:H], v1p[:, H:2 * H],
                          v1p2[:, 0:H], v1p2[:, H:2 * H]]
                    for t in range(4):
                        k = 4 * q + t
                        for m in range(4):
                            nc.tensor.matmul(
                                acc2[m][:],
                                slabs[t][:, m * 128:(m + 1) * 128],
                                vv[t], start=(k == 0), stop=(k == KT - 1))
                for m in range(4):
                    nc.vector.tensor_copy(t1_sb[4 * g + m][:], acc2[m][:])
            with tc.tile_pool(name=f"tps{g}", bufs=4, space="PSUM") as TPS:
                for c in range(4):
                    for m in range(4 * g, 4 * g + 4):
                        tp = TPS.tile([128, 128], f32, name="tp", tag="tp")
                        nc.tensor.transpose(
                            tp[:], t1_sb[m][:, c * 128:(c + 1) * 128],
                            ident_sb[:])
                        nc.vector.tensor_copy(
                            hT[4 + c][:, m * 128:(m + 1) * 128], tp[:])

        # ========= t1 stats + AllReduce-a + normalize tiles 0..7 =============
        with tc.tile_pool(name="sqps1", bufs=1, space="PSUM") as SQPS1:
            stats_for(range(4, 8), SQPS1, "sq1")
        nc.gpsimd.dma_start(ar_a_in[:, 0:8], sumc[:, 0:8])
        nc.gpsimd.dma_start(ar_a_in[:, 8:16], sqc[:, 0:8])
        nc.gpsimd.collective_compute(
            "AllReduce", AluOpType.add, replica_groups=rg,
            ins=[ar_a_in[:].opt()], outs=[ar_a_out[:].opt()])
        nc.sync.dma_start(stat_a[:], ar_a_out[:, :])
        bn_affine(stat_a, 0, 8)
        for t in range(8):
            nc.scalar.activation(hT[t][:], hT[t][:], AF.Relu,
                                 bias=shift_c[:, t:t + 1],
                                 scale=scale_c[:, t:t + 1])

        # layer-2 weights, loaded during pass B
        PW2 = st.enter_context(tc.tile_pool(name="w2pool", bufs=1))
        w2cat_sb = [PW2.tile([128, 2 * OUT], f32r, name=f"w2cat{k}")
                    for k in range(HT)]
        for k in range(HT):
            nc.scalar.dma_start(w2cat_sb[k][:],
                                w2cat_d[k * 128:(k + 1) * 128, :])
        w2h0_sb = [PW2.tile([128, OUT], f32r, name=f"w2h0{k}")
                   for k in range(HT)]
        for k in range(HT):
            nc.scalar.dma_start(w2h0_sb[k][:],
                                w2h0_d[k * 128:(k + 1) * 128, :])
        b2bc_sb = PW2.tile([128, 2 * OUT], f32, name="b2bc_sb")
        nc.scalar.dma_start(b2bc_sb[:], b2bc_d[:, :])

        # ================= B: t2b.T = (adj_loc @ t2a_full).T =================
        with (
            tc.tile_pool(name="bslabs", bufs=2) as BS,
            tc.tile_pool(name="bps", bufs=1, space="PSUM") as BPS,
        ):
            psb = [BPS.tile([128, H], f32, name=f"psb{i}", tag=f"psb{i}")
                   for i in range(8)]  # i = mo*2+n
            for q in range(KT // 2):
                aslab = BS.tile([128, ROWS], bf16, name="aslab", tag="aslab")
                nc.sync.dma_start(aslab[:],
                                  adjT_d[2 * q * 128:(2 * q + 1) * 128, :])
                aslab2 = BS.tile([128, ROWS], bf16, name="aslab2",
                                 tag="aslab2")
                nc.scalar.dma_start(
                    aslab2[:], adjT_d[(2 * q + 1) * 128:(2 * q + 2) * 128, :])
                tsp = BS.tile([128, 2 * H], bf16, name="tsp", tag="tsp")
                nc.scalar.dma_start(
                    tsp[:].rearrange("p (two f) -> p two f", two=2),
                    gsrc_pair("ag1", q))
                for t, asl in ((0, aslab), (1, aslab2)):
                    k = 2 * q + t
                    for mo in range(4):
                        for n in range(2):
                            nc.tensor.matmul(
                                psb[mo * 2 + n][:],
                                tsp[:, t * H + mo * 128:
                                    t * H + (mo + 1) * 128],
                                asl[:, n * H:(n + 1) * H],
                                start=(k == 0), stop=(k == KT - 1))
            for mo in range(4):
                for n in range(2):
                    nc.vector.tensor_copy(hT[8 + mo][:, n * H:(n + 1) * H],
                                          psb[mo * 2 + n][:])

        # ========== stats for t2b + AllReduce-b ==============================
        with tc.tile_pool(name="sqps2", bufs=1, space="PSUM") as SQPS2:
            stats_for(range(8, HT), SQPS2, "sq2")
        nc.gpsimd.dma_start(ar_b_in[:, 0:4], sumc[:, 8:12])
        nc.gpsimd.dma_start(ar_b_in[:, 4:8], sqc[:, 8:12])
        nc.gpsimd.collective_compute(
            "AllReduce", AluOpType.add, replica_groups=rg,
            ins=[ar_b_in[:].opt()], outs=[ar_b_out[:].opt()])

        # ========== T2 phase 1 (k=0..7) under AllReduce-b ====================
        with tc.tile_pool(name="ypsb", bufs=1, space="PSUM") as YPSb:
            ypss = [None] * 8
            for m in range(4, 8):
                ypss[m] = YPSb.tile([128, 2 * OUT], f32, name=f"ypsb{m}",
                                    tag=f"ypsb{m}")

            def t2_phase2(half):
                for m in range(4 * half, 4 * half + 4):
                    for k in range(8, HT):
                        nc.tensor.matmul(ypss[m][:],
                                         hT[k][:, m * 128:(m + 1) * 128],
                                         w2cat_sb[k][:],
                                         start=False, stop=(k == HT - 1))
                    ysb = DR.tile([128, 2 * OUT], bf16, name="ysb", tag="ysb")
                    nc.vector.tensor_tensor(ysb[:], ypss[m][:], b2bc_sb[:],
                                            AluOpType.add)
                    nc.gpsimd.dma_start(
                        agi["ag2" + "ab"[half]][
                            (m - 4 * half) * 128:(m - 4 * half + 1) * 128, :],
                        ysb[:])
                gather("ag2", "ab"[half])

            with tc.tile_pool(name="ypsa", bufs=1, space="PSUM") as YPSa:
                for m in range(4):
                    ypss[m] = YPSa.tile([128, 2 * OUT], f32, name=f"ypsa{m}",
                                        tag=f"ypsa{m}")
                for m in range(8):
                    for k in range(8):
                        nc.tensor.matmul(ypss[m][:],
                                         hT[k][:, m * 128:(m + 1) * 128],
                                         w2cat_sb[k][:],
                                         start=(k == 0), stop=False)

                # AllReduce-b lands: finish BN for t2b tiles
                nc.sync.dma_start(stat_b[:], ar_b_out[:, :])
                bn_affine(stat_b, 8, HT)
                for t in range(8, HT):
                    nc.scalar.activation(hT[t][:], hT[t][:], AF.Relu,
                                         bias=shift_c[:, t:t + 1],
                                         scale=scale_c[:, t:t + 1])
                t2_phase2(0)

            # YPSa closed: its banks free for y0T while phase 2b runs
            with tc.tile_pool(name="y0ps", bufs=2, space="PSUM") as Y0PS:
                t2_phase2(1)
                for mo in range(2):
                    for n in range(2):
                        y0ps = Y0PS.tile([128, H], f32, name="y0ps",
                                         tag="y0ps")
                        for k in range(HT):
                            nc.tensor.matmul(
                                y0ps[:],
                                w2h0_sb[k][:, mo * 128:(mo + 1) * 128],
                                hT[k][:, n * H:(n + 1) * H],
                                start=(k == 0), stop=(k == HT - 1))
                        nc.vector.tensor_scalar_add(
                            h2T[mo][:, n * H:(n + 1) * H], y0ps[:],
                            b2h0T_sb[:, mo:mo + 1])

        # s1 natural (fp32), transposed before AllGather3 completes
        PS1 = st.enter_context(tc.tile_pool(name="s1nat", bufs=1))
        s1_sb = [PS1.tile([128, OUT], f32, name=f"s1n{m}") for m in range(8)]

        # ========== C: [s1|s2a] = adj_loc @ [y1|y2] (natural) ================
        with tc.tile_pool(name="cpsb", bufs=1, space="PSUM") as CPSb:
            pscb = [CPSb.tile([128, 2 * OUT], f32, name=f"pscb{m}",
                              tag=f"pscb{m}") for m in range(4, 8)]
            with (
                tc.tile_pool(name="cslabs", bufs=2) as CS,
                tc.tile_pool(name="cpsa", bufs=1, space="PSUM") as CPSa,
            ):
                psc = [CPSa.tile([128, 2 * OUT], f32, name=f"psca{m}",
                                 tag=f"psca{m}") for m in range(4)] + pscb
                for q in range(KT // 2):
                    aslab = CS.tile([128, ROWS], bf16, name="aslab",
                                    tag="aslab")
                    nc.sync.dma_start(
                        aslab[:], adjT_d[2 * q * 128:(2 * q + 1) * 128, :])
                    aslab2 = CS.tile([128, ROWS], bf16, name="aslab2",
                                     tag="aslab2")
                    nc.scalar.dma_start(
                        aslab2[:],
                        adjT_d[(2 * q + 1) * 128:(2 * q + 2) * 128, :])
                    ysp = CS.tile([128, 4 * OUT], bf16, name="ysp", tag="ysp")
                    nc.sync.dma_start(
                        ysp[:].rearrange("p (two f) -> p two f", two=2),
                        gsrc_pair("ag2", q))
                    for t, asl in ((0, aslab), (1, aslab2)):
                        for m in range(8):
                            nc.tensor.matmul(
                                psc[m][:], asl[:, m * 128:(m + 1) * 128],
                                ysp[:, t * 2 * OUT:(t + 1) * 2 * OUT],
                                start=(2 * q + t == 0),
                                stop=(2 * q + t == KT - 1))
                # drains half a + AllGather3a
                for m in range(4):
                    nc.vector.tensor_copy(s1_sb[m][:], psc[m][:, :OUT])
                    s2a = DR.tile([128, OUT], bf16, name="s2a", tag="s2a")
                    nc.vector.tensor_copy(s2a[:], psc[m][:, OUT:])
                    nc.gpsimd.dma_start(
                        agi["ag3a"][m * 128:(m + 1) * 128, :], s2a[:])
                gather("ag3", "a")
            # CPSa closed: transposes for half a run during half-b drains
            with tc.tile_pool(name="tps2a", bufs=4, space="PSUM") as TPS2a:
                for m in range(4, 8):
                    nc.vector.tensor_copy(s1_sb[m][:], pscb[m - 4][:, :OUT])
                    s2a = DR.tile([128, OUT], bf16, name="s2a", tag="s2a")
                    nc.vector.tensor_copy(s2a[:], pscb[m - 4][:, OUT:])
                    nc.gpsimd.dma_start(
                        agi["ag3b"][(m - 4) * 128:(m - 3) * 128, :], s2a[:])
                for c in range(2):
                    for m in range(4):
                        tp2 = TPS2a.tile([128, 128], f32, name="tp2a",
                                         tag="tp2a")
                        nc.tensor.transpose(
                            tp2[:], s1_sb[m][:, c * 128:(c + 1) * 128],
                            ident_sb[:])
                        nc.vector.tensor_copy(
                            h2T[2 + c][:, m * 128:(m + 1) * 128], tp2[:])
                gather("ag3", "b")

        # ========== s1 transposes (half b, under AllGather3) + D =============
        with (
            tc.tile_pool(name="tps2", bufs=4, space="PSUM") as TPS2,
            tc.tile_pool(name="dslabs", bufs=2) as DS,
            tc.tile_pool(name="dps", bufs=1, space="PSUM") as DPS,
        ):
            for c in range(2):
                for m in range(4, 8):
                    tp2 = TPS2.tile([128, 128], f32, name="tp2", tag="tp2")
                    nc.tensor.transpose(tp2[:],
                                        s1_sb[m][:, c * 128:(c + 1) * 128],
                                        ident_sb[:])
                    nc.vector.tensor_copy(h2T[2 + c][:, m * 128:(m + 1) * 128],
                                          tp2[:])
            # D: s2b.T = (adj_loc @ s2a_full).T; 4 k-slabs per iteration
            psd = [DPS.tile([128, H], f32, name=f"psd{i}", tag=f"psd{i}")
                   for i in range(4)]  # i = mo*2+n
            for q in range(KT // 4):
                slabs = []
                for t in range(4):
                    k = 4 * q + t
                    asl = DS.tile([128, ROWS], bf16, name=f"dsl{t}",
                                  tag=f"dsl{t}")
                    ring(t).dma_start(asl[:],
                                      adjT_d[k * 128:(k + 1) * 128, :])
                    slabs.append(asl)
                sp1 = DS.tile([128, 2 * OUT], bf16, name="sp1", tag="sp1")
                nc.sync.dma_start(
                    sp1[:].rearrange("p (two f) -> p two f", two=2),
                    gsrc_pair("ag3", 2 * q))
                sp2 = DS.tile([128, 2 * OUT], bf16, name="sp2", tag="sp2")
                nc.scalar.dma_start(
                    sp2[:].rearrange("p (two f) -> p two f", two=2),
                    gsrc_pair("ag3", 2 * q + 1))
                ss = [sp1[:, 0:OUT], sp1[:, OUT:2 * OUT],
                      sp2[:, 0:OUT], sp2[:, OUT:2 * OUT]]
                for t in range(4):
                    k = 4 * q + t
                    for mo in range(2):
                        for n in range(2):
                            nc.tensor.matmul(
                                psd[mo * 2 + n][:],
                                ss[t][:, mo * 128:(mo + 1) * 128],
                                slabs[t][:, n * H:(n + 1) * H],
                                start=(k == 0), stop=(k == KT - 1))
            for mo in range(2):
                for n in range(2):
                    nc.vector.tensor_copy(h2T[4 + mo][:, n * H:(n + 1) * H],
                                          psd[mo * 2 + n][:])

        # ========== final: out.T = (h2 @ Wf).T + bf ==========================
        with tc.tile_pool(name="fps", bufs=2, space="PSUM") as FPS:
            for mo in range(2):
                for n in range(2):
                    fps = FPS.tile([128, H], f32, name="fps", tag="fps")
                    for k in range(H2T):
                        nc.tensor.matmul(
                            fps[:], wf_sb[k][:, mo * 128:(mo + 1) * 128],
                            h2T[k][:, n * H:(n + 1) * H],
                            start=(k == 0), stop=(k == H2T - 1))
                    osb = DR.tile([128, H], f32, name="osb", tag="osb")
                    nc.vector.tensor_scalar_add(osb[:], fps[:],
                                                bfT_sb[:, mo:mo + 1])
                    nc.sync.dma_start(
                        outT_d[mo * 128:(mo + 1) * 128, n * H:(n + 1) * H],
                        osb[:])

    nc.compile()
    _BUILT["nc"] = nc
    return nc


def _half_major_perm():
    """Slab permutation: k' -> global 128-row slab index, half-major order:
    [r0 rows0:512 | r1 rows0:512 | ... | r7 rows0:512 | r0 rows512:1024...]"""
    perm = []
    for g in range(2):
        for r in range(NC):
            for j in range(4):
                perm.append(r * 8 + g * 4 + j)
    return perm


def prep_in_maps(x, adj, W1, b1, W2, b2, gamma, beta, Wf, bf):
    """Host-side sharding / layout prep. Returns one input dict per core."""
    import ml_dtypes

    x = np.asarray(x, dtype=np.float32)
    adj = np.asarray(adj, dtype=np.float32)
    W1 = np.asarray(W1, dtype=np.float32)
    b1 = np.asarray(b1, dtype=np.float32)
    W2 = np.asarray(W2, dtype=np.float32)
    b2 = np.asarray(b2, dtype=np.float32)
    gamma = np.asarray(gamma, dtype=np.float32)
    beta = np.asarray(beta, dtype=np.float32)
    Wf = np.asarray(Wf, dtype=np.float32)
    bf = np.asarray(bf, dtype=np.float32)

    perm = _half_major_perm()
    row_perm = np.concatenate(
        [np.arange(s * 128, (s + 1) * 128) for s in perm])

    xTp = np.ascontiguousarray(x.T[:, row_perm])         # [128, 8192]
    w1cat = np.ascontiguousarray(
        np.concatenate([W1[1], W1[2]], axis=1))          # [128, 1024]
    b1cat = np.concatenate([b1[1], b1[2]])               # [1024]
    b1bc = np.ascontiguousarray(
        np.broadcast_to(b1cat[None, :], (128, 2 * H)))
    w2cat = np.ascontiguousarray(
        np.concatenate([W2[1], W2[2]], axis=1))          # [1536, 512]
    b2cat = np.concatenate([b2[1], b2[2]])               # [512]
    b2bc = np.ascontiguousarray(
        np.broadcast_to(b2cat[None, :], (128, 2 * OUT)))
    gcol = np.ascontiguousarray(gamma.reshape(HT, 128).T)
    bcol = np.ascontiguousarray(beta.reshape(HT, 128).T)
    ident = np.eye(128, dtype=np.float32)

    shared = {
        "xT": xTp,
        "w1cat": w1cat,
        "w1h0": np.ascontiguousarray(W1[0]),
        "b1bc": b1bc,
        "w2cat": w2cat,
        "w2h0": np.ascontiguousarray(W2[0]),
        "b2bc": b2bc,
        "b2h0T": np.ascontiguousarray(b2[0].reshape(2, 128).T),
        "wf": np.ascontiguousarray(Wf),
        "bfT": np.ascontiguousarray(bf.reshape(2, 128).T),
        "gcol": gcol,
        "bcol": bcol,
        "ident": ident,
    }
    in_maps = []
    for d in range(NC):
        r0, r1 = d * ROWS, (d + 1) * ROWS
        m = dict(shared)
        adjT = adj[r0:r1].T[row_perm]                    # [8192, 1024]
        m["adjT"] = np.ascontiguousarray(adjT.astype(ml_dtypes.bfloat16))
        m["xTloc"] = np.ascontiguousarray(x[r0:r1].T)    # [128, 1024]
        in_maps.append(m)
    return in_maps


def run_on_hw(in_maps, trace=False):
    from concourse import bass_utils
    nc = build_program()
    return bass_utils.run_bass_kernel_spmd(
        nc, in_maps, core_ids=list(range(NC)), trace=trace)


def kernel(x, adj, W1, b1, W2, b2, gamma, beta, Wf, bf):
    in_maps = prep_in_maps(x, adj, W1, b1, W2, b2, gamma, beta, Wf, bf)
    res = run_on_hw(in_maps)
    out = np.concatenate(
        [np.ascontiguousarray(res.results[d]["outT"].T) for d in range(NC)],
        axis=0)
    return out.astype(np.float32)


# revision 25
# speedup vs baseline: 1.5991x; 1.0172x over previous
"""MixHop (2-hop) GNN forward on 8 TRN2 NeuronCores.

Sharding: adj and the output are row-sharded over N=8192 across 8 cores
(1024 rows each); x and all weights are replicated. Each propagation
adj_loc @ v is a local [1024,8192]@[8192,F] matmul; v is produced
row-sharded and AllGathered between hops.

Precision: propagation matmuls (adj-sided, the bulk of bytes+flops) run
in bf16 with fp32 PSUM accumulation; dense-layer transforms and BN run
in fp32r (full-rate reduced fp32). Measured end-to-end relative error
~2e-3.

Orientation notes:
- "natural"   = rows on partitions (needed for AllGather row-concat and
  as the K axis of the next propagation)
- "transposed" = features on partitions (needed as lhsT of the next
  dense layer; makes BatchNorm affine per-partition)
Pass B and D emit transposed outputs directly; pass A's t1 half and
pass C's s1 half are transposed on the PE with an identity matmul.
b1[0] (hop-0 bias of layer 1) is dropped: a per-column constant shift
is exactly cancelled by the training-mode BatchNorm that follows.

Scheduling notes:
- Every AllGather is split into two row-half chunks. Producer passes
  emit their first row-half, trigger chunk-a, and compute the second
  half under it; consumer passes contract chunk-a's rows while chunk-b
  is still gathering. To keep the contraction k-axis contiguous per
  chunk, the host permutes adj's columns (and x's rows) into
  "half-major" order: [r0 rows0:512 | r1 rows0:512 | ... | r0 rows
  512:1024 | ...]. Local row order (outputs) is unpermuted.
- Propagation k-loops process 4 k-slabs per iteration -> ~4.3us
  contiguous matmul bursts, which hold the PE HAM clock-gate at full
  rate (short bursts leave the PE throttled to 1.2 GHz).
- BN stats/AllReduce/normalize for h tiles 0..7 run under pass B; T2's
  first 8 k-tiles accumulate under AllReduce-b so only a short tail
  waits on it. y0.T runs under AllGather2b; s1 transposes run before
  AllGather3.
- DMA loads alternate between the two HWDGE rings (sync/scalar);
  SBUF->DRAM drains go via SWDGE (gpsimd).
"""
import sys
from contextlib import ExitStack

sys.path.insert(0, "/opt/trn_rl_repo")

import numpy as np

N, IN, H, OUT = 8192, 128, 512, 256
NC = 8
ROWS = N // NC          # 1024 rows per core
KT = N // 128           # 64 k-tiles of the propagation contraction
KH = KT // 2            # 32 k-tiles per gather chunk
HT = 3 * H // 128       # 12 feature tiles of h.T
H2T = 3 * OUT // 128    # 6 feature tiles of h2.T
EPS = 1e-5

_BUILT = {}


def build_program():
    """Build and compile the Bass program (cached)."""
    if "nc" in _BUILT:
        return _BUILT["nc"]

    import concourse.bacc as bacc
    import concourse.tile as tile
    import concourse.mybir as mybir
    from concourse.alu_op_type import AluOpType

    f32 = mybir.dt.float32
    f32r = mybir.dt.float32r
    bf16 = mybir.dt.bfloat16
    AF = mybir.ActivationFunctionType
    AX = mybir.AxisListType

    nc = bacc.Bacc("TRN2", target_bir_lowering=False, debug=False,
                   num_devices=NC)

    # ---- external inputs (per-core values supplied by the host) ----
    adjT_d = nc.dram_tensor("adjT", [N, ROWS], bf16, kind="ExternalInput")
    xT_d = nc.dram_tensor("xT", [IN, N], f32r, kind="ExternalInput")
    xTloc_d = nc.dram_tensor("xTloc", [IN, ROWS], f32r, kind="ExternalInput")
    w1cat_d = nc.dram_tensor("w1cat", [IN, 2 * H], f32r, kind="ExternalInput")
    w1h0_d = nc.dram_tensor("w1h0", [IN, H], f32r, kind="ExternalInput")
    b1bc_d = nc.dram_tensor("b1bc", [128, 2 * H], f32, kind="ExternalInput")
    w2cat_d = nc.dram_tensor("w2cat", [3 * H, 2 * OUT], f32r, kind="ExternalInput")
    w2h0_d = nc.dram_tensor("w2h0", [3 * H, OUT], f32r, kind="ExternalInput")
    b2bc_d = nc.dram_tensor("b2bc", [128, 2 * OUT], f32, kind="ExternalInput")
    b2h0T_d = nc.dram_tensor("b2h0T", [128, 2], f32, kind="ExternalInput")
    wf_d = nc.dram_tensor("wf", [3 * OUT, OUT], f32r, kind="ExternalInput")
    bfT_d = nc.dram_tensor("bfT", [128, 2], f32, kind="ExternalInput")
    gcol_d = nc.dram_tensor("gcol", [128, HT], f32, kind="ExternalInput")
    bcol_d = nc.dram_tensor("bcol", [128, HT], f32, kind="ExternalInput")
    ident_d = nc.dram_tensor("ident", [128, 128], f32, kind="ExternalInput")

    outT_d = nc.dram_tensor("outT", [OUT, ROWS], f32, kind="ExternalOutput")

    rg = [list(range(NC))]

    def ring(k):
        return nc.sync if k % 2 == 0 else nc.scalar

    with tile.TileContext(nc) as tc, ExitStack() as st:
        dram = st.enter_context(tc.tile_pool(name="dram", bufs=1, space="DRAM"))
        P = st.enter_context(tc.tile_pool(name="persist", bufs=1))
        DR = st.enter_context(tc.tile_pool(name="drain", bufs=2))

        # ---- DRAM intermediates ----
        v_dram = dram.tile([N, H], bf16, name="v_dram")   # x1 transform only
        agi = {}
        ago = {}
        for nm, rows, cols in (("ag1", ROWS // 2, H), ("ag2", ROWS // 2,
                                                       2 * OUT),
                               ("ag3", ROWS // 2, OUT)):
            for ch in "ab":
                agi[nm + ch] = dram.tile([rows, cols], bf16,
                                         name=f"{nm}{ch}_in")
                ago[nm + ch] = dram.tile([rows * NC, cols], bf16,
                                         name=f"{nm}{ch}_out",
                                         addr_space="Shared")
        ar_a_in = dram.tile([128, 16], f32, name="ar_a_in")
        ar_a_out = dram.tile([128, 16], f32, name="ar_a_out",
                             addr_space="Shared")
        ar_b_in = dram.tile([128, 8], f32, name="ar_b_in")
        ar_b_out = dram.tile([128, 8], f32, name="ar_b_out",
                             addr_space="Shared")

        def gather(nm, ch):
            nc.gpsimd.collective_compute(
                "AllGather", AluOpType.bypass, replica_groups=rg,
                ins=[agi[nm + ch][:].opt()], outs=[ago[nm + ch][:].opt()])

        def gsrc(nm, k):
            """k-slab [128, cols] of the gathered tensor, half-major order."""
            t = ago[nm + ("a" if k < KH else "b")]
            kk = k if k < KH else k - KH
            return t[kk * 128:(kk + 1) * 128, :]

        def gsrc_pair(nm, q):
            """Pair-slab AP [128, 2, cols] for k-slabs 2q, 2q+1."""
            t = ago[nm + ("a" if 2 * q < KH else "b")]
            qq = q if 2 * q < KH else q - KH // 2
            return t[:].rearrange("(a two p) f -> a p two f",
                                  two=2, p=128)[qq]

        # ---- small persistents (to the end) ----
        xTloc_sb = P.tile([IN, ROWS], f32r, name="xTloc_sb")
        nc.scalar.dma_start(xTloc_sb[:], xTloc_d[:, :])
        w1h0_sb = P.tile([IN, H], f32r, name="w1h0_sb")
        nc.scalar.dma_start(w1h0_sb[:], w1h0_d[:, :])
        b2h0T_sb = P.tile([128, 2], f32, name="b2h0T_sb")
        nc.scalar.dma_start(b2h0T_sb[:], b2h0T_d[:, :])
        bfT_sb = P.tile([128, 2], f32, name="bfT_sb")
        nc.scalar.dma_start(bfT_sb[:], bfT_d[:, :])
        gcol_sb = P.tile([128, HT], f32, name="gcol_sb")
        nc.scalar.dma_start(gcol_sb[:], gcol_d[:, :])
        bcol_sb = P.tile([128, HT], f32, name="bcol_sb")
        nc.scalar.dma_start(bcol_sb[:], bcol_d[:, :])
        ident_sb = P.tile([128, 128], f32, name="ident_sb")
        nc.scalar.dma_start(ident_sb[:], ident_d[:, :])
        wf_sb = [P.tile([128, OUT], f32r, name=f"wf{k}") for k in range(H2T)]
        for k in range(H2T):
            nc.scalar.dma_start(wf_sb[k][:], wf_d[k * 128:(k + 1) * 128, :])
        sumc = P.tile([128, HT], f32, name="sumc")
        sqc = P.tile([128, HT], f32, name="sqc")
        scale_c = P.tile([128, HT], f32, name="scale_c")
        shift_c = P.tile([128, HT], f32, name="shift_c")
        stat_a = P.tile([128, 16], f32, name="stat_a")
        stat_b = P.tile([128, 8], f32, name="stat_b")
        # h2.T (fp32r): tiles 0-1 y0.T, 2-3 s1.T, 4-5 s2b.T
        h2T = [P.tile([128, ROWS], f32r, name=f"h2T{t}") for t in range(H2T)]

        def bn_affine(stat, lo, hi):
            """Compute scale/shift columns [lo,hi) from gathered stats."""
            w = hi - lo
            mu = DR.tile([128, w], f32, name="mu", tag=f"mu{lo}")
            nc.vector.tensor_scalar_mul(mu[:], stat[:, :w], 1.0 / N)
            ex2 = DR.tile([128, w], f32, name="ex2", tag=f"ex2{lo}")
            nc.vector.tensor_scalar(ex2[:], stat[:, w:2 * w], 1.0 / N, EPS,
                                    AluOpType.mult, AluOpType.add)
            var = DR.tile([128, w], f32, name="var", tag=f"var{lo}")
            nc.vector.scalar_tensor_tensor(var[:], mu[:], -1.0, mu[:],
                                           AluOpType.mult, AluOpType.mult)
            nc.vector.tensor_add(var[:], var[:], ex2[:])
            std = DR.tile([128, w], f32, name="std", tag=f"std{lo}")
            nc.scalar.activation(std[:], var[:], AF.Sqrt)
            rstd = DR.tile([128, w], f32, name="rstd", tag=f"rstd{lo}")
            nc.vector.reciprocal(rstd[:], std[:])
            nc.vector.tensor_mul(scale_c[:, lo:hi], gcol_sb[:, lo:hi], rstd[:])
            nc.vector.scalar_tensor_tensor(shift_c[:, lo:hi], mu[:], -1.0,
                                           scale_c[:, lo:hi],
                                           AluOpType.mult, AluOpType.mult)
            nc.vector.tensor_add(shift_c[:, lo:hi], shift_c[:, lo:hi],
                                 bcol_sb[:, lo:hi])

        def stats_for(tiles, sq_pool, tag):
            for t in tiles:
                nc.vector.reduce_sum(sumc[:, t:t + 1], hT[t][:], axis=AX.X)
                sq = sq_pool.tile([128, ROWS], f32, name=tag, tag=tag)
                nc.vector.scalar_tensor_tensor(
                    sq[:], hT[t][:], 1.0, hT[t][:],
                    AluOpType.mult, AluOpType.mult,
                    accum_out=sqc[:, t:t + 1])

        # ============ T1 + pass A (t2a), pipelined ===========================
        with (
            tc.tile_pool(name="v2pool", bufs=1) as V2P,
            tc.tile_pool(name="xkpool", bufs=1) as XKP,
            tc.tile_pool(name="w1pool", bufs=1) as W1P,
        ):
            v2sb = [V2P.tile([128, H], bf16, name=f"v2s{k}")
                    for k in range(KT)]
            xk_sb = [XKP.tile([128, 128], f32r, name=f"xk{k}")
                     for k in range(KT)]
            w1cat_sb = W1P.tile([IN, 2 * H], f32r, name="w1cat_sb")
            nc.scalar.dma_start(w1cat_sb[:], w1cat_d[:, :])
            b1bc_sb = W1P.tile([128, 2 * H], f32, name="b1bc_sb")
            nc.scalar.dma_start(b1bc_sb[:], b1bc_d[:, :])

            # T1-x2: v2 = x @ W1[2] + b1[2]  (SBUF-resident, bf16)
            with tc.tile_pool(name="t1ps2", bufs=3, space="PSUM") as T1PS2:
                for k in range(KT):
                    nc.gpsimd.dma_start(xk_sb[k][:],
                                        xT_d[:, k * 128:(k + 1) * 128])
                    vps2 = T1PS2.tile([128, H], f32, name="vps2", tag="vps2")
                    nc.tensor.matmul(vps2[:], xk_sb[k][:],
                                     w1cat_sb[:, H:2 * H],
                                     start=True, stop=True)
                    nc.vector.tensor_tensor(v2sb[k][:], vps2[:],
                                            b1bc_sb[:, H:2 * H],
                                            AluOpType.add)

            # A-t2a group 0 with T1-x1 interleaved into its dense PE stream
            with (
                tc.tile_pool(name="ag0slab", bufs=2) as AS,
                tc.tile_pool(name="ag0ps", bufs=1, space="PSUM") as APS,
                tc.tile_pool(name="t1ps1", bufs=2, space="PSUM") as T1PS1,
            ):
                acc = [APS.tile([128, H], f32, name=f"acc0{m}",
                                tag=f"acc0{m}") for m in range(4)]
                for q in range(KT // 4):
                    slabs = []
                    for t in range(4):
                        k = 4 * q + t
                        asl = AS.tile([128, H], bf16, name=f"asl{t}",
                                      tag=f"asl{t}")
                        ring(t).dma_start(asl[:],
                                          adjT_d[k * 128:(k + 1) * 128, 0:H])
                        slabs.append(asl)
                    for t in range(4):
                        k = 4 * q + t
                        for m in range(4):
                            nc.tensor.matmul(
                                acc[m][:], slabs[t][:, m * 128:(m + 1) * 128],
                                v2sb[k][:], start=(k == 0),
                                stop=(k == KT - 1))
                    # T1-x1 for 4 slabs, slotted into the same stream
                    for t in range(4):
                        k = 4 * q + t
                        vps1 = T1PS1.tile([128, H], f32, name="vps1",
                                          tag="vps1")
                        nc.tensor.matmul(vps1[:], xk_sb[k][:],
                                         w1cat_sb[:, 0:H],
                                         start=True, stop=True)
                        v1sb = DR.tile([128, H], bf16, name="v1sb",
                                       tag="v1sb")
                        nc.vector.tensor_tensor(v1sb[:], vps1[:],
                                                b1bc_sb[:, 0:H],
                                                AluOpType.add)
                        nc.gpsimd.dma_start(
                            v_dram[k * 128:(k + 1) * 128, :], v1sb[:])
                for m in range(4):
                    t2a = DR.tile([128, H], bf16, name="t2a", tag="t2a")
                    nc.vector.tensor_copy(t2a[:], acc[m][:])
                    nc.gpsimd.dma_start(
                        agi["ag1a"][m * 128:(m + 1) * 128, :], t2a[:])

            gather("ag1", "a")

            # A-t2a group 1
            with (
                tc.tile_pool(name="ag1slab", bufs=2) as AS1,
                tc.tile_pool(name="ag1ps", bufs=1, space="PSUM") as APS1,
            ):
                acc1 = [APS1.tile([128, H], f32, name=f"acc1{m}",
                                  tag=f"acc1{m}") for m in range(4)]
                for q in range(KT // 4):
                    slabs = []
                    for t in range(4):
                        k = 4 * q + t
                        asl = AS1.tile([128, H], bf16, name=f"bsl{t}",
                                       tag=f"bsl{t}")
                        ring(t).dma_start(asl[:],
                                          adjT_d[k * 128:(k + 1) * 128,
                                                 H:ROWS])
                        slabs.append(asl)
                    for t in range(4):
                        k = 4 * q + t
                        for m in range(4):
                            nc.tensor.matmul(
                                acc1[m][:], slabs[t][:, m * 128:(m + 1) * 128],
                                v2sb[k][:], start=(k == 0),
                                stop=(k == KT - 1))
                for m in range(4):
                    t2a = DR.tile([128, H], bf16, name="t2a", tag="t2a")
                    nc.vector.tensor_copy(t2a[:], acc1[m][:])
                    nc.gpsimd.dma_start(
                        agi["ag1b"][m * 128:(m + 1) * 128, :], t2a[:])

            gather("ag1", "b")

        # hT in fp32r: tiles 0-3 hop0.T, 4-7 t1.T, 8-11 t2b.T.
        PH = st.enter_context(tc.tile_pool(name="hpool", bufs=1))
        hT = [PH.tile([128, ROWS], f32r, name=f"hT{t}") for t in range(HT)]

        # ========== hop0.T + its stats (runs under AllGather1) ===============
        with (
            tc.tile_pool(name="h0ps", bufs=2, space="PSUM") as H0PS,
            tc.tile_pool(name="sqps0", bufs=1, space="PSUM") as SQPS0,
        ):
            for mo in range(4):
                for n in range(2):
                    h0ps = H0PS.tile([128, H], f32, name="h0ps", tag="h0ps")
                    nc.tensor.matmul(h0ps[:],
                                     w1h0_sb[:, mo * 128:(mo + 1) * 128],
                                     xTloc_sb[:, n * H:(n + 1) * H],
                                     start=True, stop=True)
                    nc.vector.tensor_copy(hT[mo][:, n * H:(n + 1) * H],
                                          h0ps[:])
            stats_for(range(4), SQPS0, "sq0")

        # t1 natural (fp32), transposed as soon as each row-group lands
        PT1 = st.enter_context(tc.tile_pool(name="t1nat", bufs=1))
        t1_sb = [PT1.tile([128, H], f32, name=f"t1n{m}") for m in range(8)]

        # ========= A-t1 row-groups + transposes ==============================
        for g in range(2):
            with (
                tc.tile_pool(name=f"a2slab{g}", bufs=2) as AS2,
                tc.tile_pool(name=f"aps2{g}", bufs=1, space="PSUM") as APS2,
            ):
                acc2 = [APS2.tile([128, H], f32, name=f"ac2{g}{m}",
                                  tag=f"ac2{g}{m}") for m in range(4)]
                for q in range(KT // 4):
                    slabs = []
                    for t in range(4):
                        k = 4 * q + t
                        asl = AS2.tile([128, H], bf16, name=f"a2s{t}",
                                       tag=f"a2s{t}")
                        ring(t).dma_start(
                            asl[:], adjT_d[k * 128:(k + 1) * 128,
                                           g * H:(g + 1) * H])
                        slabs.append(asl)
                    v1p = AS2.tile([128, 2 * H], bf16, name="v1p", tag="v1p")
                    nc.scalar.dma_start(
                        v1p[:].rearrange("p (two f) -> p two f", two=2),
                        v_dram[:].rearrange("(a two p) f -> a p two f",
                                            two=2, p=128)[2 * q])
                    v1p2 = AS2.tile([128, 2 * H], bf16, name="v1p2",
                                    tag="v1p2")
                    nc.sync.dma_start(
                        v1p2[:].rearrange("p (two f) -> p two f", two=2),
                        v_dram[:].rearrange("(a two p) f -> a p two f",
                                            two=2, p=128)[2 * q + 1])
                    vv = [v1p[:, 0:H], v1p[:, H:2 * H],
                          v1p2[:, 0:H], v1p2[:, H:2 * H]]
                    for t in range(4):
                        k = 4 * q + t
                        for m in range(4):
                            nc.tensor.matmul(
                                acc2[m][:],
                                slabs[t][:, m * 128:(m + 1) * 128],
                                vv[t], start=(k == 0), stop=(k == KT - 1))
                for m in range(4):
                    nc.vector.tensor_copy(t1_sb[4 * g + m][:], acc2[m][:])
            with tc.tile_pool(name=f"tps{g}", bufs=4, space="PSUM") as TPS:
                for c in range(4):
                    for m in range(4 * g, 4 * g + 4):
                        tp = TPS.tile([128, 128], f32, name="tp", tag="tp")
                        nc.tensor.transpose(
                            tp[:], t1_sb[m][:, c * 128:(c + 1) * 128],
                            ident_sb[:])
                        nc.vector.tensor_copy(
                            hT[4 + c][:, m * 128:(m + 1) * 128], tp[:])

        # ========= t1 stats + AllReduce-a + normalize tiles 0..7 =============
        with tc.tile_pool(name="sqps1", bufs=1, space="PSUM") as SQPS1:
            stats_for(range(4, 8), SQPS1, "sq1")
        nc.gpsimd.dma_start(ar_a_in[:, 0:8], sumc[:, 0:8])
        nc.gpsimd.dma_start(ar_a_in[:, 8:16], sqc[:, 0:8])
        nc.gpsimd.collective_compute(
            "AllReduce", AluOpType.add, replica_groups=rg,
            ins=[ar_a_in[:].opt()], outs=[ar_a_out[:].opt()])
        nc.sync.dma_start(stat_a[:], ar_a_out[:, :])
        bn_affine(stat_a, 0, 8)
        for t in range(8):
            nc.scalar.activation(hT[t][:], hT[t][:], AF.Relu,
                                 bias=shift_c[:, t:t + 1],
                                 scale=scale_c[:, t:t + 1])

        # layer-2 weights, loaded during pass B
        PW2 = st.enter_context(tc.tile_pool(name="w2pool", bufs=1))
        w2cat_sb = [PW2.tile([128, 2 * OUT], f32r, name=f"w2cat{k}")
                    for k in range(HT)]
        for k in range(HT):
            nc.scalar.dma_start(w2cat_sb[k][:],
                                w2cat_d[k * 128:(k + 1) * 128, :])
        w2h0_sb = [PW2.tile([128, OUT], f32r, name=f"w2h0{k}")
                   for k in range(HT)]
        for k in range(HT):
            nc.scalar.dma_start(w2h0_sb[k][:],
                                w2h0_d[k * 128:(k + 1) * 128, :])
        b2bc_sb = PW2.tile([128, 2 * OUT], f32, name="b2bc_sb")
        nc.scalar.dma_start(b2bc_sb[:], b2bc_d[:, :])

        # ================= B: t2b.T = (adj_loc @ t2a_full).T =================
        with (
            tc.tile_pool(name="bslabs", bufs=2) as BS,
            tc.tile_pool(name="bps", bufs=1, space="PSUM") as BPS,
        ):
            psb = [BPS.tile([128, H], f32, name=f"psb{i}", tag=f"psb{i}")
                   for i in range(8)]  # i = mo*2+n
            for q in range(KT // 2):
                aslab = BS.tile([128, ROWS], bf16, name="aslab", tag="aslab")
                nc.sync.dma_start(aslab[:],
                                  adjT_d[2 * q * 128:(2 * q + 1) * 128, :])
                aslab2 = BS.tile([128, ROWS], bf16, name="aslab2",
                                 tag="aslab2")
                nc.scalar.dma_start(
                    aslab2[:], adjT_d[(2 * q + 1) * 128:(2 * q + 2) * 128, :])
                tsp = BS.tile([128, 2 * H], bf16, name="tsp", tag="tsp")
                nc.scalar.dma_start(
                    tsp[:].rearrange("p (two f) -> p two f", two=2),
                    gsrc_pair("ag1", q))
                for t, asl in ((0, aslab), (1, aslab2)):
                    k = 2 * q + t
                    for mo in range(4):
                        for n in range(2):
                            nc.tensor.matmul(
                                psb[mo * 2 + n][:],
                                tsp[:, t * H + mo * 128:
                                    t * H + (mo + 1) * 128],
                                asl[:, n * H:(n + 1) * H],
                                start=(k == 0), stop=(k == KT - 1))
            for mo in range(4):
                for n in range(2):
                    nc.vector.tensor_copy(hT[8 + mo][:, n * H:(n + 1) * H],
                                          psb[mo * 2 + n][:])

        # ========== stats for t2b + AllReduce-b ==============================
        with tc.tile_pool(name="sqps2", bufs=1, space="PSUM") as SQPS2:
            stats_for(range(8, HT), SQPS2, "sq2")
        nc.gpsimd.dma_start(ar_b_in[:, 0:4], sumc[:, 8:12])
        nc.gpsimd.dma_start(ar_b_in[:, 4:8], sqc[:, 8:12])
        nc.gpsimd.collective_compute(
            "AllReduce", AluOpType.add, replica_groups=rg,
            ins=[ar_b_in[:].opt()], outs=[ar_b_out[:].opt()])

        # ========== T2 phase 1 (k=0..7) under AllReduce-b ====================
        with tc.tile_pool(name="ypsb", bufs=1, space="PSUM") as YPSb:
            ypss = [None] * 8
            for m in range(4, 8):
                ypss[m] = YPSb.tile([128, 2 * OUT], f32, name=f"ypsb{m}",
                                    tag=f"ypsb{m}")

            def t2_phase2(half):
                for m in range(4 * half, 4 * half + 4):
                    for k in range(8, HT):
                        nc.tensor.matmul(ypss[m][:],
                                         hT[k][:, m * 128:(m + 1) * 128],
                                         w2cat_sb[k][:],
                                         start=False, stop=(k == HT - 1))
                    ysb = DR.tile([128, 2 * OUT], bf16, name="ysb", tag="ysb")
                    nc.vector.tensor_tensor(ysb[:], ypss[m][:], b2bc_sb[:],
                                            AluOpType.add)
                    nc.gpsimd.dma_start(
                        agi["ag2" + "ab"[half]][
                            (m - 4 * half) * 128:(m - 4 * half + 1) * 128, :],
                        ysb[:])
                gather("ag2", "ab"[half])

            with tc.tile_pool(name="ypsa", bufs=1, space="PSUM") as YPSa:
                for m in range(4):
                    ypss[m] = YPSa.tile([128, 2 * OUT], f32, name=f"ypsa{m}",
                                        tag=f"ypsa{m}")
                for m in range(8):
                    for k in range(8):
                        nc.tensor.matmul(ypss[m][:],
                                         hT[k][:, m * 128:(m + 1) * 128],
                                         w2cat_sb[k][:],
                                         start=(k == 0), stop=False)

                # AllReduce-b lands: finish BN for t2b tiles
                nc.sync.dma_start(stat_b[:], ar_b_out[:, :])
                bn_affine(stat_b, 8, HT)
                for t in range(8, HT):
                    nc.scalar.activation(hT[t][:], hT[t][:], AF.Relu,
                                         bias=shift_c[:, t:t + 1],
                                         scale=scale_c[:, t:t + 1])
                t2_phase2(0)

            # YPSa closed: its banks free for y0T, which fills the PE while
            # AllGather2a runs; phase 2b follows.
            with tc.tile_pool(name="y0ps", bufs=2, space="PSUM") as Y0PS:
                for mo in range(2):
                    for n in range(2):
                        y0ps = Y0PS.tile([128, H], f32, name="y0ps",
                                         tag="y0ps")
                        for k in range(HT):
                            nc.tensor.matmul(
                                y0ps[:],
                                w2h0_sb[k][:, mo * 128:(mo + 1) * 128],
                                hT[k][:, n * H:(n + 1) * H],
                                start=(k == 0), stop=(k == HT - 1))
                        nc.vector.tensor_scalar_add(
                            h2T[mo][:, n * H:(n + 1) * H], y0ps[:],
                            b2h0T_sb[:, mo:mo + 1])
                t2_phase2(1)

        # s1 natural (fp32), transposed before AllGather3 completes
        PS1 = st.enter_context(tc.tile_pool(name="s1nat", bufs=1))
        s1_sb = [PS1.tile([128, OUT], f32, name=f"s1n{m}") for m in range(8)]

        # ========== C: [s1|s2a] = adj_loc @ [y1|y2] (natural) ================
        with tc.tile_pool(name="cpsb", bufs=1, space="PSUM") as CPSb:
            pscb = [CPSb.tile([128, 2 * OUT], f32, name=f"pscb{m}",
                              tag=f"pscb{m}") for m in range(4, 8)]
            with (
                tc.tile_pool(name="cslabs", bufs=2) as CS,
                tc.tile_pool(name="cpsa", bufs=1, space="PSUM") as CPSa,
            ):
                psc = [CPSa.tile([128, 2 * OUT], f32, name=f"psca{m}",
                                 tag=f"psca{m}") for m in range(4)] + pscb
                for q in range(KT // 2):
                    aslab = CS.tile([128, ROWS], bf16, name="aslab",
                                    tag="aslab")
                    nc.sync.dma_start(
                        aslab[:], adjT_d[2 * q * 128:(2 * q + 1) * 128, :])
                    aslab2 = CS.tile([128, ROWS], bf16, name="aslab2",
                                     tag="aslab2")
                    nc.scalar.dma_start(
                        aslab2[:],
                        adjT_d[(2 * q + 1) * 128:(2 * q + 2) * 128, :])
                    ysp = CS.tile([128, 4 * OUT], bf16, name="ysp", tag="ysp")
                    nc.sync.dma_start(
                        ysp[:].rearrange("p (two f) -> p two f", two=2),
                        gsrc_pair("ag2", q))
                    for t, asl in ((0, aslab), (1, aslab2)):
                        for m in range(8):
                            nc.tensor.matmul(
                                psc[m][:], asl[:, m * 128:(m + 1) * 128],
                                ysp[:, t * 2 * OUT:(t + 1) * 2 * OUT],
                                start=(2 * q + t == 0),
                                stop=(2 * q + t == KT - 1))
                # drains half a + AllGather3a
                for m in range(4):
                    nc.vector.tensor_copy(s1_sb[m][:], psc[m][:, :OUT])
                    s2a = DR.tile([128, OUT], bf16, name="s2a", tag="s2a")
                    nc.vector.tensor_copy(s2a[:], psc[m][:, OUT:])
                    nc.gpsimd.dma_start(
                        agi["ag3a"][m * 128:(m + 1) * 128, :], s2a[:])
                gather("ag3", "a")
            # CPSa closed: transposes for half a run during half-b drains
            with tc.tile_pool(name="tps2a", bufs=4, space="PSUM") as TPS2a:
                for m in range(4, 8):
                    nc.vector.tensor_copy(s1_sb[m][:], pscb[m - 4][:, :OUT])
                    s2a = DR.tile([128, OUT], bf16, name="s2a", tag="s2a")
                    nc.vector.tensor_copy(s2a[:], pscb[m - 4][:, OUT:])
                    nc.gpsimd.dma_start(
                        agi["ag3b"][(m - 4) * 128:(m - 3) * 128, :], s2a[:])
                for c in range(2):
                    for m in range(4):
                        tp2 = TPS2a.tile([128, 128], f32, name="tp2a",
                                         tag="tp2a")
                        nc.tensor.transpose(
                            tp2[:], s1_sb[m][:, c * 128:(c + 1) * 128],
                            ident_sb[:])
                        nc.vector.tensor_copy(
                            h2T[2 + c][:, m * 128:(m + 1) * 128], tp2[:])
                gather("ag3", "b")

        # ========== s1 transposes (half b, under AllGather3) + D + final =====
        with (
            tc.tile_pool(name="dslabs", bufs=2) as DS,
            tc.tile_pool(name="dps", bufs=1, space="PSUM") as DPS,
        ):
            with tc.tile_pool(name="tps2", bufs=4, space="PSUM") as TPS2:
                for c in range(2):
                    for m in range(4, 8):
                        tp2 = TPS2.tile([128, 128], f32, name="tp2",
                                        tag="tp2")
                        nc.tensor.transpose(
                            tp2[:], s1_sb[m][:, c * 128:(c + 1) * 128],
                            ident_sb[:])
                        nc.vector.tensor_copy(
                            h2T[2 + c][:, m * 128:(m + 1) * 128], tp2[:])
            # final partial: y0/s1 k-tiles of out.T accumulate during the
            # AllGather3 window; the s2b tiles are added after pass D.
            fstack = ExitStack()
            FPS = fstack.enter_context(tc.tile_pool(name="fps", bufs=1,
                                                    space="PSUM"))
            fq = [FPS.tile([128, H], f32, name=f"fq{i}", tag=f"fq{i}")
                  for i in range(4)]  # i = mo*2+n
            for mo in range(2):
                for n in range(2):
                    for k in range(4):
                        nc.tensor.matmul(
                            fq[mo * 2 + n][:],
                            wf_sb[k][:, mo * 128:(mo + 1) * 128],
                            h2T[k][:, n * H:(n + 1) * H],
                            start=(k == 0), stop=False)
            # D: s2b.T = (adj_loc @ s2a_full).T; 4 k-slabs per iteration
            psd = [DPS.tile([128, H], f32, name=f"psd{i}", tag=f"psd{i}")
                   for i in range(4)]  # i = mo*2+n
            for q in range(KT // 4):
                slabs = []
                for t in range(4):
                    k = 4 * q + t
                    asl = DS.tile([128, ROWS], bf16, name=f"dsl{t}",
                                  tag=f"dsl{t}")
                    ring(t).dma_start(asl[:],
                                      adjT_d[k * 128:(k + 1) * 128, :])
                    slabs.append(asl)
                sp1 = DS.tile([128, 2 * OUT], bf16, name="sp1", tag="sp1")
                nc.sync.dma_start(
                    sp1[:].rearrange("p (two f) -> p two f", two=2),
                    gsrc_pair("ag3", 2 * q))
                sp2 = DS.tile([128, 2 * OUT], bf16, name="sp2", tag="sp2")
                nc.scalar.dma_start(
                    sp2[:].rearrange("p (two f) -> p two f", two=2),
                    gsrc_pair("ag3", 2 * q + 1))
                ss = [sp1[:, 0:OUT], sp1[:, OUT:2 * OUT],
                      sp2[:, 0:OUT], sp2[:, OUT:2 * OUT]]
                for t in range(4):
                    k = 4 * q + t
                    for mo in range(2):
                        for n in range(2):
                            nc.tensor.matmul(
                                psd[mo * 2 + n][:],
                                ss[t][:, mo * 128:(mo + 1) * 128],
                                slabs[t][:, n * H:(n + 1) * H],
                                start=(k == 0), stop=(k == KT - 1))
            for mo in range(2):
                for n in range(2):
                    nc.vector.tensor_copy(h2T[4 + mo][:, n * H:(n + 1) * H],
                                          psd[mo * 2 + n][:])

            # ========== final tail: add s2b k-tiles + bias, store ============
            for mo in range(2):
                for n in range(2):
                    for k in range(4, H2T):
                        nc.tensor.matmul(
                            fq[mo * 2 + n][:],
                            wf_sb[k][:, mo * 128:(mo + 1) * 128],
                            h2T[k][:, n * H:(n + 1) * H],
                            start=False, stop=(k == H2T - 1))
                    osb = DR.tile([128, H], f32, name="osb", tag="osb")
                    nc.vector.tensor_scalar_add(osb[:], fq[mo * 2 + n][:],
                                                bfT_sb[:, mo:mo + 1])
                    nc.sync.dma_start(
                        outT_d[mo * 128:(mo + 1) * 128, n * H:(n + 1) * H],
                        osb[:])
            fstack.close()

    nc.compile()
    _BUILT["nc"] = nc
    return nc


def _half_major_perm():
    """Slab permutation: k' -> global 128-row slab index, half-major order:
    [r0 rows0:512 | r1 rows0:512 | ... | r7 rows0:512 | r0 rows512:1024...]"""
    perm = []
    for g in range(2):
        for r in range(NC):
            for j in range(4):
                perm.append(r * 8 + g * 4 + j)
    return perm


def prep_in_maps(x, adj, W1, b1, W2, b2, gamma, beta, Wf, bf):
    """Host-side sharding / layout prep. Returns one input dict per core."""
    import ml_dtypes

    x = np.asarray(x, dtype=np.float32)
    adj = np.asarray(adj, dtype=np.float32)
    W1 = np.asarray(W1, dtype=np.float32)
    b1 = np.asarray(b1, dtype=np.float32)
    W2 = np.asarray(W2, dtype=np.float32)
    b2 = np.asarray(b2, dtype=np.float32)
    gamma = np.asarray(gamma, dtype=np.float32)
    beta = np.asarray(beta, dtype=np.float32)
    Wf = np.asarray(Wf, dtype=np.float32)
    bf = np.asarray(bf, dtype=np.float32)

    perm = _half_major_perm()
    row_perm = np.concatenate(
        [np.arange(s * 128, (s + 1) * 128) for s in perm])

    xTp = np.ascontiguousarray(x.T[:, row_perm])         # [128, 8192]
    w1cat = np.ascontiguousarray(
        np.concatenate([W1[1], W1[2]], axis=1))          # [128, 1024]
    b1cat = np.concatenate([b1[1], b1[2]])               # [1024]
    b1bc = np.ascontiguousarray(
        np.broadcast_to(b1cat[None, :], (128, 2 * H)))
    w2cat = np.ascontiguousarray(
        np.concatenate([W2[1], W2[2]], axis=1))          # [1536, 512]
    b2cat = np.concatenate([b2[1], b2[2]])               # [512]
    b2bc = np.ascontiguousarray(
        np.broadcast_to(b2cat[None, :], (128, 2 * OUT)))
    gcol = np.ascontiguousarray(gamma.reshape(HT, 128).T)
    bcol = np.ascontiguousarray(beta.reshape(HT, 128).T)
    ident = np.eye(128, dtype=np.float32)

    shared = {
        "xT": xTp,
        "w1cat": w1cat,
        "w1h0": np.ascontiguousarray(W1[0]),
        "b1bc": b1bc,
        "w2cat": w2cat,
        "w2h0": np.ascontiguousarray(W2[0]),
        "b2bc": b2bc,
        "b2h0T": np.ascontiguousarray(b2[0].reshape(2, 128).T),
        "wf": np.ascontiguousarray(Wf),
        "bfT": np.ascontiguousarray(bf.reshape(2, 128).T),
        "gcol": gcol,
        "bcol": bcol,
        "ident": ident,
    }
    in_maps = []
    for d in range(NC):
        r0, r1 = d * ROWS, (d + 1) * ROWS
        m = dict(shared)
        adjT = adj[r0:r1].T[row_perm]                    # [8192, 1024]
        m["adjT"] = np.ascontiguousarray(adjT.astype(ml_dtypes.bfloat16))
        m["xTloc"] = np.ascontiguousarray(x[r0:r1].T)    # [128, 1024]
        in_maps.append(m)
    return in_maps


def run_on_hw(in_maps, trace=False):
    from concourse import bass_utils
    nc = build_program()
    return bass_utils.run_bass_kernel_spmd(
        nc, in_maps, core_ids=list(range(NC)), trace=trace)


def kernel(x, adj, W1, b1, W2, b2, gamma, beta, Wf, bf):
    in_maps = prep_in_maps(x, adj, W1, b1, W2, b2, gamma, beta, Wf, bf)
    res = run_on_hw(in_maps)
    out = np.concatenate(
        [np.ascontiguousarray(res.results[d]["outT"].T) for d in range(NC)],
        axis=0)
    return out.astype(np.float32)
